# revision 1
# baseline (speedup 1.0000x reference)
"""BioZorro sparse-attention kernel for 8 Trainium2 NeuronCores.

Sharding: 8 cores = 2 batches x 4 token-quarters (384 own tokens each).
The zorro mask makes all non-fusion query rows fully masked -> uniform
softmax -> their attention output is mean(V) over all tokens; only the 16
fusion tokens do real attention (over the 1536 non-fusion keys). Fusion
rows are replicated on all 4 cores of a batch; the only cross-core data
is a per-layer AllGather of flash-softmax partials + V column sums
(~67 KB), plus one tiny AllGather for the final pooling.

Layout: the residual stream lives feature-major (tok^T [512, 400]) so
every matmul consumes natural-layout weights as lhsT with zero big
transposes. LayerNorm stats are computed via ones-matmul partition
reductions; rstd = exp(-0.5*ln(var+eps)) keeps ACT on one table set.
"""
import sys
sys.path.insert(0, "/opt/trn_rl_repo")
import numpy as np
import ml_dtypes

BF = ml_dtypes.bfloat16
OWN, FUS, TOK = 384, 16, 400
D, RIN, H, DH, IFF, DEPTH = 512, 1024, 8, 64, 1365, 4
NALL = 1552
B, NR, NA = 2, 768, 768
N_CORES = 8

_built = {}


def build(num_devices=8, use_cc=True):
    key = (num_devices, use_cc)
    if key in _built:
        return _built[key]
    import concourse.tile as tile
    from concourse import bacc, mybir
    from concourse.masks import make_identity

    # Force Exp to resolve to natural_log_exp_and_others so the Ln/Exp
    # pairs in the LN rstd chain share one ACT table set (otherwise the
    # table-load pass ping-pongs between natural_log and exp_and_others,
    # costing ~2.6us per layernorm on the critical path).
    if not getattr(bacc, "_act_tables_patched", False):
        _orig_gat = bacc.get_activation_tables

        def _patched_gat(arch):
            tabs = _orig_gat(arch)
            exp_t = mybir.ActivationFunctionType.Exp
            for nm, fns in tabs.items():
                if nm != "natural_log_exp_and_others":
                    fns.discard(exp_t)
            return tabs

        bacc.get_activation_tables = _patched_gat
        bacc._act_tables_patched = True

    f32 = mybir.dt.float32
    bf16 = mybir.dt.bfloat16
    AF = mybir.ActivationFunctionType
    OP = mybir.AluOpType

    nc = bacc.Bacc("TRN2", target_bir_lowering=False, debug=False,
                   enable_asserts=True, num_devices=num_devices)

    def din(name, shape, dt=f32):
        return nc.dram_tensor(name, shape, dt, kind="ExternalInput").ap()

    x_t = din("x_t", [RIN, OWN], bf16)
    ew_t = din("emb_w", [RIN, D], bf16); ebias_t = din("emb_b", [D, 1])
    eg2 = din("eln2_g", [D, 1]); eb2 = din("eln2_b", [D, 1])
    fus_t = din("fus_t", [D, FUS], bf16)

    wq_t = din("wq", [DEPTH, D, D], bf16)
    wkv_t = din("wkv", [DEPTH, D, 2 * D], bf16)
    woh_t = din("wo_h", [DEPTH, H, DH, D], bf16)
    won_t = din("wo_n", [DEPTH, D, D], bf16)
    w1_t = din("w1", [DEPTH, D, 2 * 1408], bf16)
    w2_t = din("w2", [DEPTH, 1408, D], bf16)
    pq2_t = din("pool_q2", [D, 1])
    pwkv_t = din("pool_wkv", [D, 2 * D], bf16)
    pwoh_t = din("pool_wo_h", [H, DH, D], bf16)
    pwon_t = din("pool_wo_n", [D, D], bf16)
    out_u = nc.dram_tensor("out_u", [D, 1], f32, kind="ExternalOutput").ap()
    out_f = nc.dram_tensor("out_f", [1, D], f32, kind="ExternalOutput").ap()

    W2T = 11  # k-tiles of padded IFF (11x128)
    IFFP = 11 * 128  # 1408, zero-padded from 1365

    with tile.TileContext(nc) as tc:
        with tc.tile_pool(name="cst", bufs=1) as cst, \
             tc.tile_pool(name="wp", bufs=2) as wp, \
             tc.tile_pool(name="ac", bufs=2) as ac, \
             tc.tile_pool(name="pgen", bufs=4, space="PSUM") as pgen, \
             tc.tile_pool(name="pacc", bufs=4, space="PSUM") as pacc, \
             tc.tile_pool(name="dramp", bufs=2, space="DRAM") as dramp:

            ident = cst.tile([128, 128], bf16, name="ident")
            make_identity(nc, ident[:])
            ones128 = cst.tile([128, 1], bf16, name="ones128")
            nc.vector.memset(ones128[:], 1.0)
            ones1 = cst.tile([1, 128], bf16, name="ones1")
            nc.vector.memset(ones1[:], 1.0)
            epsc = cst.tile([128, 1], f32, name="epsc")
            nc.vector.memset(epsc[:], 1e-5)
            oi512 = cst.tile([128, 1], bf16, name="oi512")
            nc.vector.memset(oi512[:], 1.0 / 512)
            oi1024 = cst.tile([128, 1], bf16, name="oi1024")
            nc.vector.memset(oi1024[:], 1.0 / 1024)

            # ---------- helpers ----------
            def ln_fold(pres, T, tag="y"):
                """Folded feature-major LN (gain folded into weights on host).

                pres: list of C callables; pres[c](dst) emits the write of the
                bf16 pre-LN input chunk into dst [128, T] (plain cast or fused
                residual-add + cast). Returns (y tiles, none) where
                y[c] = xb[c]*rstd - mu*rstd broadcast.
                """
                C = len(pres)
                oi = oi512 if C == 4 else oi1024
                xbs = []
                S = pgen.tile([1, T], f32, tag="g", name="lnS")
                Q = pgen.tile([1, T], f32, tag="g", name="lnQ")
                for c in range(C):
                    if callable(pres[c]):
                        xb = ac.tile([128, T], bf16, tag="lnxb", bufs=10,
                                     name="lnxb")
                        pres[c](xb)
                        xb = xb[:]
                    else:
                        xb = pres[c]
                    xbs.append(xb)
                    nc.tensor.matmul(S[:], oi[:], xb,
                                     start=(c == 0), stop=(c == C - 1))
                for c in range(C):
                    x2 = ac.tile([128, T], bf16, tag="lnx2", bufs=2, name="lnx2")
                    nc.scalar.activation(out=x2[:], in_=xbs[c], func=AF.Square)
                    nc.tensor.matmul(Q[:], oi[:], x2[:],
                                     start=(c == 0), stop=(c == C - 1))
                # S = mu, Q = E[x^2] directly (1/N folded into the ones)
                m2 = ac.tile([1, T], f32, tag="lnst", bufs=6, name="lnm2")
                nc.scalar.activation(out=m2[:], in_=S[:], func=AF.Square)
                var = ac.tile([1, T], f32, tag="lnst", bufs=6, name="lnvar")
                nc.vector.tensor_sub(out=var[:], in0=Q[:], in1=m2[:])
                pair = ac.tile([1, 2 * T], f32, tag="lnpr", bufs=2, name="lnpr")
                rstd = pair[:, 0:T]
                nc.scalar.activation(out=rstd, in_=var[:], func=AF.Ln,
                                     bias=epsc[0:1, :])
                nc.scalar.activation(out=rstd, in_=rstd, func=AF.Exp, scale=-0.5)
                nc.vector.tensor_mul(out=pair[:, T:2 * T], in0=S[:], in1=rstd)
                pairb = ac.tile([1, 2 * T], bf16, tag="lnprb", bufs=2, name="lnprb")
                nc.vector.tensor_copy(out=pairb[:], in_=pair[:])
                BR = pgen.tile([128, T], f32, tag="g", name="lnBR")
                nc.tensor.matmul(BR[:], ones1[:], pairb[:, 0:T], start=True,
                                 stop=True)
                BM = pgen.tile([128, T], f32, tag="g", name="lnBM")
                nc.tensor.matmul(BM[:], ones1[:], pairb[:, T:2 * T], start=True,
                                 stop=True)
                rB = ac.tile([128, T], bf16, tag="lnrB", bufs=2, name="lnrB")
                nc.vector.tensor_copy(out=rB[:], in_=BR[:])
                mB = ac.tile([128, T], bf16, tag="lnmB", bufs=2, name="lnmB")
                nc.vector.tensor_copy(out=mB[:], in_=BM[:])
                ys = []
                for c in range(C):
                    y = ac.tile([128, T], bf16, tag=f"{tag}{c}", bufs=1,
                                name=f"{tag}{c}")
                    nc.vector.tensor_mul(out=y[:], in0=xbs[c], in1=rB[:])
                    nc.vector.tensor_sub(out=y[:], in0=y[:], in1=mB[:])
                    ys.append(y)
                return ys, None

            def cp(src_ap):
                return lambda dst: nc.vector.tensor_copy(out=dst[:], in_=src_ap)

            def ln_fm(xs, T, gs, bs, out_views=None, out_dtype=bf16, tag="xn"):
                """Feature-major layernorm over C*128 features.

                xs: list of C sbuf AP views [128, T] (f32 or bf16-castable).
                gs/bs: per-chunk [128,1] f32 scalar tiles (bs may be None).
                Returns list of C output APs ([128, T] out_dtype).
                """
                C = len(xs)
                inv = 1.0 / (128 * C)
                xbs = []
                S = pgen.tile([1, T], f32, tag="g", name="lnS")
                Q = pgen.tile([1, T], f32, tag="g", name="lnQ")
                for c in range(C):
                    xb = ac.tile([128, T], bf16, tag="lnxb", bufs=10, name="lnxb")
                    nc.vector.tensor_copy(out=xb[:], in_=xs[c])
                    xbs.append(xb)
                    nc.tensor.matmul(S[:], ones128[:], xb[:],
                                     start=(c == 0), stop=(c == C - 1))
                for c in range(C):
                    x2 = ac.tile([128, T], bf16, tag="lnx2", bufs=2, name="lnx2")
                    nc.vector.tensor_mul(out=x2[:], in0=xbs[c][:], in1=xbs[c][:])
                    nc.tensor.matmul(Q[:], ones128[:], x2[:],
                                     start=(c == 0), stop=(c == C - 1))
                mu = ac.tile([1, T], f32, tag="lnst", bufs=6, name="lnmu")
                nc.scalar.mul(out=mu[:], in_=S[:], mul=inv)
                m2 = ac.tile([1, T], f32, tag="lnst", bufs=6, name="lnm2")
                nc.vector.tensor_mul(out=m2[:], in0=mu[:], in1=mu[:])
                var = ac.tile([1, T], f32, tag="lnst", bufs=6, name="lnvar")
                nc.scalar.mul(out=var[:], in_=Q[:], mul=inv)
                nc.vector.tensor_sub(out=var[:], in0=var[:], in1=m2[:])
                rstd = ac.tile([1, T], f32, tag="lnst", bufs=6, name="lnrstd")
                nc.scalar.activation(out=rstd[:], in_=var[:], func=AF.Ln,
                                     bias=epsc[0:1, :])
                nc.scalar.activation(out=rstd[:], in_=rstd[:], func=AF.Exp, scale=-0.5)
                murs = ac.tile([1, T], f32, tag="lnst", bufs=6, name="lnmurs")
                nc.vector.tensor_mul(out=murs[:], in0=mu[:], in1=rstd[:])
                rb = ac.tile([1, T], bf16, tag="lnsb", bufs=4, name="lnrb")
                nc.vector.tensor_copy(out=rb[:], in_=rstd[:])
                mb = ac.tile([1, T], bf16, tag="lnsb", bufs=4, name="lnmb")
                nc.vector.tensor_copy(out=mb[:], in_=murs[:])
                BR = pgen.tile([128, T], f32, tag="g", name="lnBR")
                nc.tensor.matmul(BR[:], ones1[:], rb[:], start=True, stop=True)
                BM = pgen.tile([128, T], f32, tag="g", name="lnBM")
                nc.tensor.matmul(BM[:], ones1[:], mb[:], start=True, stop=True)
                rB = ac.tile([128, T], bf16, tag="lnrB", bufs=2, name="lnrB")
                nc.vector.tensor_copy(out=rB[:], in_=BR[:])
                mB = ac.tile([128, T], bf16, tag="lnmB", bufs=2, name="lnmB")
                nc.vector.tensor_copy(out=mB[:], in_=BM[:])
                outs = []
                for c in range(C):
                    t1 = ac.tile([128, T], bf16, tag="lnt1", bufs=2, name="lnt1")
                    nc.vector.tensor_mul(out=t1[:], in0=xbs[c][:], in1=rB[:])
                    nc.vector.tensor_sub(out=t1[:], in0=t1[:], in1=mB[:])
                    if out_views is not None:
                        o = out_views[c]
                    else:
                        o = ac.tile([128, T], out_dtype, tag=f"{tag}{c}", bufs=1,
                                    name=f"{tag}{c}")[:]
                    if bs is not None:
                        nc.vector.tensor_scalar(out=o, in0=t1[:], scalar1=gs[c][:],
                                                scalar2=bs[c][:], op0=OP.mult,
                                                op1=OP.add)
                    else:
                        nc.vector.tensor_scalar_mul(out=o, in0=t1[:], scalar1=gs[c][:])
                    outs.append(o)
                return outs

            def load_cols(dram_ap, n, tag, rows=128):
                """Load [n*rows, 1]-style f32 column into n [rows,1] tiles."""
                ts = []
                for c in range(n):
                    t = wp.tile([rows, 1], f32, tag=f"{tag}{c}", bufs=1,
                                name=f"{tag}{c}")
                    nc.sync.dma_start(out=t[:], in_=dram_ap[rows * c:rows * (c + 1), :])
                    ts.append(t)
                return ts

            # ---------- embed ----------
            xeT = ac.tile([128, 8, OWN], bf16, tag="xe", bufs=1, name="xeT")
            nc.sync.dma_start(out=xeT[:],
                              in_=x_t.rearrange("(c p) t -> p c t", c=8))
            xe = [xeT[:, c, :] for c in range(8)]
            ewT = wp.tile([128, 8, D], bf16, tag="ew", bufs=1, name="ewT")
            nc.sync.dma_start(out=ewT[:],
                              in_=ew_t.rearrange("(c p) f -> p c f", c=8))
            ews = [ewT[:, c, :] for c in range(8)]
            ebs = load_cols(ebias_t, 4, "ebias")
            eg2s = load_cols(eg2, 4, "eg2")
            eb2s = load_cols(eb2, 4, "eb2")

            xn1e, _ = ln_fold(list(xe), OWN, tag="xne")

            t2 = []
            for mc in range(4):
                ps = pgen.tile([128, OWN], f32, tag="g", name=f"embp{mc}")
                for kc in range(8):
                    nc.tensor.matmul(ps[:], ews[kc][:, 128 * mc:128 * (mc + 1)],
                                     xn1e[kc][:], start=(kc == 0), stop=(kc == 7))
                t = ac.tile([128, OWN], f32, tag="t2", bufs=4, name=f"t2{mc}")
                nc.vector.tensor_scalar_add(out=t[:], in0=ps[:], scalar1=ebs[mc][:])
                t2.append(t[:])

            tok = [ac.tile([128, TOK], bf16, tag=f"tok{c}", bufs=1, name=f"tok{c}")
                   for c in range(4)]
            ln_fm(t2, OWN, eg2s, eb2s,
                  out_views=[tok[c][:, 0:OWN] for c in range(4)], out_dtype=f32)
            for c in range(4):
                nc.sync.dma_start(out=tok[c][:, OWN:TOK],
                                  in_=fus_t[128 * c:128 * (c + 1), :])

            tok_chunks = [(0, 128), (128, 256), (256, 384), (384, 400)]
            rg = [[0, 1, 2, 3], [4, 5, 6, 7]]

            # ---------- layers ----------
            psO_prev = None
            for l in range(DEPTH):
                wqT = wp.tile([128, 4, D], bf16, tag="wq", bufs=1, name="wqT")
                nc.sync.dma_start(out=wqT[:],
                                  in_=wq_t[l].rearrange("(c p) f -> p c f", c=4))
                wq = [wqT[:, c, :] for c in range(4)]
                wkvT = wp.tile([128, 4, 2 * D], bf16, tag="wkv", bufs=2,
                               name="wkvT")
                nc.sync.dma_start(out=wkvT[:],
                                  in_=wkv_t[l].rearrange("(c p) f -> p c f", c=4))
                wkv = [wkvT[:, c, :] for c in range(4)]
                wohT = wp.tile([DH, H, D], bf16, tag="woh", bufs=1, name="wohT")
                nc.sync.dma_start(out=wohT[:],
                                  in_=woh_t[l].rearrange("h d f -> d h f"))
                woh = [wohT[:, h, :] for h in range(H)]
                wonT = wp.tile([128, 4, D], bf16, tag="won", bufs=1, name="wonT")
                nc.sync.dma_start(out=wonT[:],
                                  in_=won_t[l].rearrange("(c p) f -> p c f", c=4))
                won = [wonT[:, c, :] for c in range(4)]
                w1T = wp.tile([128, 4, 2 * IFFP], bf16, tag="w1", bufs=1,
                              name="w1T")
                nc.sync.dma_start(out=w1T[:],
                                  in_=w1_t[l].rearrange("(c p) f -> p c f", c=4))
                w1 = [w1T[:, c, :] for c in range(4)]
                w2T_ = wp.tile([128, W2T, D], bf16, tag="w2", bufs=1, name="w2T_")
                nc.sync.dma_start(out=w2T_[:],
                                  in_=w2_t[l].rearrange("(j p) f -> p j f", j=W2T))
                w2 = [w2T_[:, j, :] for j in range(W2T)]
                # LN1 -> xn1 (folded; fuse previous layer's FF2 residual)
                if psO_prev is not None:
                    for c in range(4):
                        nc.vector.tensor_add(out=tok[c][:], in0=tok[c][:],
                                             in1=psO_prev[c][:])
                    psO_prev = None
                xn1, _ = ln_fold([tok[c][:] for c in range(4)], TOK, tag="x1")

                # K^T (own cols only)
                kt = []
                for mc in range(4):
                    ps = pgen.tile([128, OWN], f32, tag="g", name=f"kt{mc}")
                    for kc in range(4):
                        nc.tensor.matmul(ps[:], wkv[kc][:, 128 * mc:128 * (mc + 1)],
                                         xn1[kc][:, 0:OWN],
                                         start=(kc == 0), stop=(kc == 3))
                    s = ac.tile([128, OWN], bf16, tag=f"kt{mc}", bufs=1,
                                name=f"ktb{mc}")
                    nc.vector.tensor_copy(out=s[:], in_=ps[:])
                    kt.append(s)

                # qf^T padded to 32 cols
                qf = []
                for mc in range(4):
                    ps = pgen.tile([128, FUS], f32, tag="g", name=f"qf{mc}")
                    for kc in range(4):
                        nc.tensor.matmul(ps[:], wq[kc][:, 128 * mc:128 * (mc + 1)],
                                         xn1[kc][:, OWN:TOK],
                                         start=(kc == 0), stop=(kc == 3))
                    s = ac.tile([128, 32], bf16, tag=f"qf{mc}", bufs=1,
                                name=f"qfb{mc}")
                    nc.vector.memset(s[:, FUS:32], 0.0)
                    nc.vector.tensor_copy(out=s[:, 0:FUS], in_=ps[:])
                    qf.append(s)

                # scores + exp (+row sums)
                E, lacc = [], []
                for t in range(2):
                    sp = pgen.tile([128, OWN], f32, tag="g", name=f"sp{t}")
                    for i in range(4):
                        h = 4 * t + i
                        ch, base = h // 2, (h % 2) * 64
                        nc.tensor.matmul(sp[32 * i:32 * i + 32, :],
                                         qf[ch][base:base + 64, 0:32],
                                         kt[ch][base:base + 64, :],
                                         start=True, stop=True,
                                         tile_position=(base, 32 * i))
                    e = ac.tile([128, OWN], bf16, tag=f"e{t}", bufs=1, name=f"e{t}")
                    la = ac.tile([128, 1], f32, tag=f"la{t}", bufs=2, name=f"la{t}")
                    nc.scalar.activation(out=e[:], in_=sp[:], func=AF.Exp,
                                         accum_out=la[:])
                    E.append(e)
                    lacc.append(la)

                # e^T via PE transpose
                ET = [[None] * 3 for _ in range(2)]
                for t in range(2):
                    for j in range(3):
                        pt = pgen.tile([128, 128], bf16, tag="g", name=f"et{t}{j}")
                        nc.tensor.transpose(pt[:], E[t][:, 128 * j:128 * (j + 1)],
                                            ident[:])
                        s = ac.tile([128, 128], bf16, tag=f"ET{t}{j}", bufs=1,
                                    name=f"ETb{t}{j}")
                        nc.vector.tensor_copy(out=s[:], in_=pt[:])
                        ET[t][j] = s

                # V token-major
                V = []
                for i, (a, b) in enumerate(tok_chunks):
                    m = b - a
                    ps = pgen.tile([128, D], f32, tag="g", name=f"v{i}")
                    for kc in range(4):
                        nc.tensor.matmul(ps[0:m, :], xn1[kc][:, a:b],
                                         wkv[kc][:, D:2 * D],
                                         start=(kc == 0), stop=(kc == 3))
                    s = ac.tile([128, D], bf16, tag=f"V{i}", bufs=1, name=f"Vb{i}")
                    nc.vector.tensor_copy(out=s[0:m, :], in_=ps[0:m, :])
                    V.append(s)

                # payload P = [l0, l1, ACC0, ACC1, vsumT_own]
                P = ac.tile([128, 134], f32, tag="P", bufs=2, name="P")
                nc.vector.tensor_copy(out=P[:, 0:1], in_=lacc[0][:])
                nc.vector.tensor_copy(out=P[:, 1:2], in_=lacc[1][:])
                for t in range(2):
                    acc = pacc.tile([128, 64], f32, tag="a", name=f"acc{t}")
                    for i in range(4):
                        h = 4 * t + i
                        for j in range(3):
                            nc.tensor.matmul(acc[32 * i:32 * i + 32, :],
                                             ET[t][j][:, 32 * i:32 * i + 32],
                                             V[j][:, DH * h:DH * (h + 1)],
                                             start=(j == 0), stop=(j == 2),
                                             tile_position=(0, 32 * i))
                    nc.vector.tensor_copy(out=P[:, 2 + 64 * t:66 + 64 * t], in_=acc[:])
                for c in range(4):
                    ps = pgen.tile([128, 1], f32, tag="g", name=f"vs{c}")
                    for j in range(3):
                        nc.tensor.matmul(ps[:], V[j][:, 128 * c:128 * (c + 1)],
                                         ones128[:], start=(j == 0), stop=(j == 2))
                    nc.vector.tensor_copy(out=P[:, 130 + c:131 + c], in_=ps[:])
                vfu = ac.tile([128, 4], f32, tag="vfu", bufs=2, name="vfu")
                for c in range(4):
                    ps = pgen.tile([128, 1], f32, tag="g", name=f"vf{c}")
                    nc.tensor.matmul(ps[:], V[3][0:FUS, 128 * c:128 * (c + 1)],
                                     ones128[0:FUS, :], start=True, stop=True)
                    nc.vector.tensor_copy(out=vfu[:, c:c + 1], in_=ps[:])

                # exchange
                pin = dramp.tile([128, 134], f32, tag="pin", bufs=2, name="pin")
                nc.sync.dma_start(out=pin[:], in_=P[:])
                R = []
                if use_cc:
                    pout = dramp.tile([4 * 128, 134], f32, tag="pout", bufs=2,
                                      name="pout")
                    nc.gpsimd.collective_compute(
                        "AllGather", OP.bypass, replica_groups=rg,
                        ins=[pin.opt()], outs=[pout.opt()])
                    for r in range(4):
                        s = ac.tile([128, 134], f32, tag="R", bufs=4,
                                    name=f"R{r}")
                        nc.sync.dma_start(out=s[:], in_=pout[128 * r:128 * (r + 1), :])
                        R.append(s)
                else:
                    for r in range(4):
                        s = ac.tile([128, 134], f32, tag="R", bufs=4,
                                    name=f"R{r}")
                        nc.sync.dma_start(out=s[:], in_=pin[:])
                        R.append(s)
                T01 = ac.tile([128, 134], f32, tag="cmb", bufs=3, name="T01")
                nc.vector.tensor_add(out=T01[:], in0=R[0][:], in1=R[1][:])
                T23 = ac.tile([128, 134], f32, tag="cmb", bufs=3, name="T23")
                nc.vector.tensor_add(out=T23[:], in0=R[2][:], in1=R[3][:])
                PT = ac.tile([128, 134], f32, tag="cmb", bufs=3, name="PT")
                nc.vector.tensor_add(out=PT[:], in0=T01[:], in1=T23[:])

                # uniform delta
                vsb = ac.tile([128, 4], bf16, tag="vsb", bufs=2, name="vsb")
                nc.vector.tensor_add(out=vsb[:], in0=PT[:, 130:134], in1=vfu[:])
                dup = pgen.tile([128, 4], f32, tag="g", name="dup")
                for c in range(4):
                    for kc in range(4):
                        nc.tensor.matmul(dup[:, c:c + 1],
                                         won[kc][:, 128 * c:128 * (c + 1)],
                                         vsb[:, kc:kc + 1],
                                         start=(kc == 0), stop=(kc == 3))
                dub = ac.tile([128, 4], f32, tag="dub", bufs=2, name="dub")
                nc.vector.tensor_copy(out=dub[:], in_=dup[:])
                dus = [dub[:, c:c + 1] for c in range(4)]

                # fusion delta
                linv = ac.tile([128, 2], f32, tag="linv", bufs=2, name="linv")
                nc.vector.reciprocal(out=linv[:], in_=PT[:, 0:2])
                ofT = []
                for t in range(2):
                    of = ac.tile([128, 64], bf16, tag=f"of{t}", bufs=1,
                                 name=f"of{t}")
                    nc.vector.tensor_scalar_mul(out=of[:],
                                                in0=PT[:, 2 + 64 * t:66 + 64 * t],
                                                scalar1=linv[:, t:t + 1])
                    pt = pgen.tile([64, 128], bf16, tag="g", name=f"oft{t}")
                    nc.tensor.transpose(pt[:], of[:], ident[:])
                    s = ac.tile([64, 128], bf16, tag=f"ofT{t}", bufs=1,
                                name=f"ofTb{t}")
                    nc.vector.tensor_copy(out=s[:], in_=pt[:])
                    ofT.append(s)
                df = pacc.tile([FUS, D], f32, tag="a", name="df")
                for h in range(H):
                    t, i = h // 4, h % 4
                    nc.tensor.matmul(df[:], ofT[t][:, 32 * i:32 * i + FUS],
                                     woh[h][:], start=(h == 0), stop=(h == 7))
                dfb = ac.tile([FUS, D], bf16, tag="dfb", bufs=2, name="dfb")
                nc.vector.tensor_copy(out=dfb[:], in_=df[:])

                # transpose fusion delta; residual applied fused in LN2
                dftp = []
                for c in range(4):
                    pt = pacc.tile([128, FUS], bf16, tag="a", name=f"dft{c}")
                    nc.tensor.transpose(pt[:], dfb[0:FUS, 128 * c:128 * (c + 1)],
                                        ident[0:FUS, 0:FUS])
                    dftp.append(pt)

                # LN2 + GEGLU FF (folded LN; FF2 c-major so psO[0] lands
                # early and the next layer's LN1 overlaps FF2's tail)
                def mk2(c):
                    def pre(dst):
                        nc.vector.tensor_scalar_add(out=dst[:, 0:OWN],
                                                    in0=tok[c][:, 0:OWN],
                                                    scalar1=dus[c][:])
                        nc.vector.tensor_add(out=dst[:, OWN:TOK],
                                             in0=tok[c][:, OWN:TOK],
                                             in1=dftp[c][:])
                    return pre
                xn2, _ = ln_fold([mk2(c) for c in range(4)], TOK, tag="x2")
                for c in range(4):
                    nc.vector.tensor_scalar_add(out=tok[c][:, 0:OWN],
                                                in0=tok[c][:, 0:OWN],
                                                scalar1=dus[c][:])
                    nc.vector.tensor_add(out=tok[c][:, OWN:TOK],
                                         in0=tok[c][:, OWN:TOK], in1=dftp[c][:])
                gts = []
                for j in range(W2T):
                    a = 128 * j
                    wj = 128
                    px = pgen.tile([128, TOK], f32, tag="g", name=f"fx{j}")
                    pg = pgen.tile([128, TOK], f32, tag="g", name=f"fg{j}")
                    for kc in range(4):
                        nc.tensor.matmul(px[0:wj, :], w1[kc][:, a:a + wj],
                                         xn2[kc][:], start=(kc == 0), stop=(kc == 3))
                    for kc in range(4):
                        nc.tensor.matmul(pg[0:wj, :],
                                         w1[kc][:, IFFP + a:IFFP + a + wj],
                                         xn2[kc][:], start=(kc == 0), stop=(kc == 3))
                    gg = ac.tile([128, TOK], bf16, tag="gg", bufs=3, name=f"gg{j}")
                    nc.scalar.activation(out=gg[0:wj, :], in_=pg[0:wj, :],
                                         func=AF.Gelu)
                    gt = ac.tile([128, TOK], bf16, tag="gt", bufs=12, name=f"gt{j}")
                    nc.vector.tensor_mul(out=gt[0:wj, :], in0=gg[0:wj, :],
                                         in1=px[0:wj, :])
                    gts.append(gt)
                psO_prev = []
                for c in range(4):
                    psO = pacc.tile([128, TOK], f32, tag="a", name=f"fo{c}")
                    for j in range(W2T):
                        nc.tensor.matmul(psO[:], w2[j][:, 128 * c:128 * (c + 1)],
                                         gts[j][:], start=(j == 0),
                                         stop=(j == W2T - 1))
                    psO_prev.append(psO)

            # ---------- pool ----------
            pwkv = []
            for c in range(4):
                t = wp.tile([128, 2 * D], bf16, tag=f"wkv{c}", bufs=1,
                            name=f"pwkv{c}")
                nc.sync.dma_start(out=t[:], in_=pwkv_t[128 * c:128 * (c + 1), :])
                pwkv.append(t)
            pwoh = []
            for h in range(H):
                t = wp.tile([DH, D], bf16, tag=f"woh{h}", bufs=1, name=f"pwoh{h}")
                nc.sync.dma_start(out=t[:], in_=pwoh_t[h])
                pwoh.append(t)
            pwon = []
            for c in range(4):
                t = wp.tile([128, D], bf16, tag=f"won{c}", bufs=1, name=f"pwon{c}")
                nc.sync.dma_start(out=t[:], in_=pwon_t[128 * c:128 * (c + 1), :])
                pwon.append(t)
            pq2s = load_cols(pq2_t, 4, "pq2")

            for c in range(4):
                nc.vector.tensor_add(out=tok[c][:], in0=tok[c][:],
                                     in1=psO_prev[c][:])
            tokn, _ = ln_fold([tok[c][:] for c in range(4)], TOK, tag="tn")

            # V_pool token-major
            Vp = []
            for i, (a, b) in enumerate(tok_chunks):
                m = b - a
                ps = pgen.tile([128, D], f32, tag="g", name=f"pv{i}")
                for kc in range(4):
                    nc.tensor.matmul(ps[0:m, :], tokn[kc][:, a:b],
                                     pwkv[kc][:, D:2 * D],
                                     start=(kc == 0), stop=(kc == 3))
                s = ac.tile([128, D], bf16, tag=f"V{i}", bufs=1, name=f"pVb{i}")
                nc.vector.tensor_copy(out=s[0:m, :], in_=ps[0:m, :])
                Vp.append(s)

            # pool vsum exchange
            Pp = ac.tile([128, 4], f32, tag="Pp", bufs=2, name="Pp")
            for c in range(4):
                ps = pgen.tile([128, 1], f32, tag="g", name=f"pvs{c}")
                for j in range(3):
                    nc.tensor.matmul(ps[:], Vp[j][:, 128 * c:128 * (c + 1)],
                                     ones128[:], start=(j == 0), stop=(j == 2))
                nc.vector.tensor_copy(out=Pp[:, c:c + 1], in_=ps[:])
            pvfu = ac.tile([128, 4], f32, tag="vfu", bufs=2, name="pvfu")
            for c in range(4):
                ps = pgen.tile([128, 1], f32, tag="g", name=f"pvf{c}")
                nc.tensor.matmul(ps[:], Vp[3][0:FUS, 128 * c:128 * (c + 1)],
                                 ones128[0:FUS, :], start=True, stop=True)
                nc.vector.tensor_copy(out=pvfu[:, c:c + 1], in_=ps[:])
            pinp = dramp.tile([128, 4], f32, tag="pinp", bufs=1, name="pinp")
            nc.sync.dma_start(out=pinp[:], in_=Pp[:])
            Rpa = ac.tile([128, 4, 4], f32, tag="Ra", bufs=2, name="Rpa")
            if use_cc:
                poutp = dramp.tile([4 * 128, 4], f32, tag="poutp", bufs=1,
                                   name="poutp")
                nc.gpsimd.collective_compute(
                    "AllGather", OP.bypass, replica_groups=rg,
                    ins=[pinp.opt()], outs=[poutp.opt()])
                nc.sync.dma_start(
                    out=Rpa[:],
                    in_=poutp.rearrange("(r p) f -> p r f", r=4))
            else:
                nc.sync.dma_start(
                    out=Rpa[:],
                    in_=pinp.rearrange("(r p) f -> p r f", r=1)
                    .to_broadcast((128, 4, 4)))
            pT2 = ac.tile([128, 2, 4], f32, tag="cmb", bufs=3, name="pT2")
            nc.vector.tensor_add(out=pT2[:], in0=Rpa[:, 0:2, :], in1=Rpa[:, 2:4, :])
            pvs = ac.tile([128, 4], f32, tag="vsall", bufs=2, name="pvs")
            nc.vector.tensor_add(out=pvs[:], in0=pT2[:, 0, :], in1=pT2[:, 1, :])
            nc.vector.tensor_add(out=pvs[:], in0=pvs[:], in1=pvfu[:])
            pvsb = ac.tile([128, 4], bf16, tag="vsb", bufs=2, name="pvsb")
            nc.vector.tensor_copy(out=pvsb[:], in_=pvs[:])
            for c in range(4):
                ps = pgen.tile([128, 1], f32, tag="g", name=f"pdu{c}")
                for kc in range(4):
                    nc.tensor.matmul(ps[:], pwon[kc][:, 128 * c:128 * (c + 1)],
                                     pvsb[:, kc:kc + 1],
                                     start=(kc == 0), stop=(kc == 3))
                s = ac.tile([128, 1], f32, tag=f"du{c}", bufs=2, name=f"pdub{c}")
                nc.vector.tensor_copy(out=s[:], in_=ps[:])
                nc.sync.dma_start(out=out_u[128 * c:128 * (c + 1), :], in_=s[:])

            # fusion-key attention for return token 2 (all local)
            kf = []
            for mc in range(4):
                ps = pgen.tile([128, FUS], f32, tag="g", name=f"pkf{mc}")
                for kc in range(4):
                    nc.tensor.matmul(ps[:], pwkv[kc][:, 128 * mc:128 * (mc + 1)],
                                     tokn[kc][:, OWN:TOK],
                                     start=(kc == 0), stop=(kc == 3))
                s = ac.tile([128, FUS], bf16, tag=f"kf{mc}", bufs=1, name=f"kfb{mc}")
                nc.vector.tensor_copy(out=s[:], in_=ps[:])
                kf.append(s)
            q2 = []
            for mc in range(4):
                s = ac.tile([128, 32], bf16, tag=f"qf{mc}", bufs=1, name=f"q2b{mc}")
                nc.vector.memset(s[:, 1:32], 0.0)
                nc.vector.tensor_copy(out=s[:, 0:1], in_=pq2s[mc][:])
                q2.append(s)
            e2, l2 = [], []
            for t in range(2):
                sp = pgen.tile([128, FUS], f32, tag="g", name=f"ps2{t}")
                for i in range(4):
                    h = 4 * t + i
                    ch, base = h // 2, (h % 2) * 64
                    nc.tensor.matmul(sp[32 * i:32 * i + 32, :],
                                     q2[ch][base:base + 64, 0:32],
                                     kf[ch][base:base + 64, :],
                                     start=True, stop=True,
                                     tile_position=(base, 32 * i))
                e = ac.tile([128, FUS], bf16, tag=f"e2{t}", bufs=1, name=f"e2{t}")
                la = ac.tile([128, 1], f32, tag=f"la{t}", bufs=2, name=f"pla{t}")
                nc.scalar.activation(out=e[:], in_=sp[:], func=AF.Exp,
                                     accum_out=la[:])
                e2.append(e)
                l2.append(la)
            e2T = []
            for t in range(2):
                pt = pgen.tile([FUS, 128], bf16, tag="g", name=f"pet{t}")
                nc.tensor.transpose(pt[:], e2[t][:], ident[:])
                s = ac.tile([FUS, 128], bf16, tag=f"e2T{t}", bufs=1, name=f"e2Tb{t}")
                nc.vector.tensor_copy(out=s[:], in_=pt[:])
                e2T.append(s)
            ofT2 = []
            for t in range(2):
                acc = pacc.tile([128, 64], f32, tag="a", name=f"pacc2{t}")
                for i in range(4):
                    h = 4 * t + i
                    nc.tensor.matmul(acc[32 * i:32 * i + 32, :],
                                     e2T[t][:, 32 * i:32 * i + 32],
                                     Vp[3][0:FUS, DH * h:DH * (h + 1)],
                                     start=True, stop=True,
                                     tile_position=(0, 32 * i))
                li = ac.tile([128, 1], f32, tag="linv", bufs=2, name=f"pli{t}")
                nc.vector.reciprocal(out=li[:], in_=l2[t][:])
                of = ac.tile([128, 64], bf16, tag=f"of{t}", bufs=1, name=f"pof{t}")
                nc.vector.tensor_scalar_mul(out=of[:], in0=acc[:], scalar1=li[:])
                pt = pgen.tile([64, 128], bf16, tag="g", name=f"poft{t}")
                nc.tensor.transpose(pt[:], of[:], ident[:])
                s = ac.tile([64, 128], bf16, tag=f"ofT{t}", bufs=1, name=f"pofTb{t}")
                nc.vector.tensor_copy(out=s[:], in_=pt[:])
                ofT2.append(s)
            P2 = pacc.tile([1, D], f32, tag="a", name="P2")
            for h in range(H):
                t, i = h // 4, h % 4
                nc.tensor.matmul(P2[:], ofT2[t][:, 32 * i:32 * i + 1],
                                 pwoh[h][:], start=(h == 0), stop=(h == 7))
            p2s = ac.tile([1, D], f32, tag="p2s", bufs=1, name="p2s")
            nc.vector.tensor_copy(out=p2s[:], in_=P2[:])
            nc.sync.dma_start(out=out_f[:], in_=p2s[:])

    nc.compile()
    _built[key] = nc
    return nc


def _pad_w1(w1f):
    """[DEPTH, D, 2*IFF] -> [DEPTH, D, 2*1408] with x1/gate blocks padded."""
    out = np.zeros((DEPTH, D, 2 * 1408), np.float64)
    out[:, :, 0:IFF] = w1f[:, :, 0:IFF]
    out[:, :, 1408:1408 + IFF] = w1f[:, :, IFF:2 * IFF]
    return out


def _prep_inputs(inputs):
    """Host-side prep: slice/transpose/cast per-core input dicts."""
    I = {k: np.asarray(v) for k, v in inputs.items()}
    f32 = np.float32

    def bf(x):
        return np.ascontiguousarray(x).astype(BF)

    def col(x):
        return np.ascontiguousarray(np.asarray(x, f32).reshape(-1, 1))

    scale = DH ** -0.5
    # fold LN gains into the consuming weights; bar = column sums for the
    # rank-1 (-mu*rstd) correction applied in PSUM by the kernel
    wqf = I["layers_wq"].astype(np.float64) * scale \
        * I["layers_attn_g"].astype(np.float64)[:, :, None]
    wkvf = I["layers_wkv"].astype(np.float64) \
        * I["layers_attn_g"].astype(np.float64)[:, :, None]
    w1f = I["layers_ff_w1"].astype(np.float64) \
        * I["layers_ff_g"].astype(np.float64)[:, :, None]
    pkvf = I["pool_wkv"].astype(np.float64) * I["final_g"].astype(np.float64)[:, None]
    shared = {
        "fus_t": bf(I["fusion_tokens"].astype(np.float64).T),
        "wq": bf(wqf),
        "wkv": bf(wkvf),
        "wo_h": bf(I["layers_wo"].reshape(DEPTH, H, DH, D)),
        "wo_n": bf(I["layers_wo"] * (1.0 / NALL)),
        "w1": bf(_pad_w1(w1f)),
        "w2": bf(np.pad(I["layers_ff_w2"].astype(np.float64),
                        ((0, 0), (0, 1408 - IFF), (0, 0)))),
        "pool_wkv": bf(pkvf),
        "pool_wo_h": bf(I["pool_wo"].reshape(H, DH, D)),
        "pool_wo_n": bf(I["pool_wo"] * (1.0 / NALL)),
    }
    # host-side pool query for return token 2 (row 2 = FUSION)
    ret = I["return_tokens"].astype(f32)
    g = I["pool_g"].astype(f32)
    mu = ret.mean(-1, keepdims=True)
    var = ((ret - mu) ** 2).mean(-1, keepdims=True)
    retn = (ret - mu) / np.sqrt(var + 1e-5) * g
    q2 = (retn[2] @ I["pool_wq"].astype(f32)) * scale
    shared["pool_q2"] = col(q2)

    in_maps = []
    for c in range(N_CORES):
        b, q = c // 4, c % 4
        mod = "rna" if q < 2 else "atac"
        x = I[mod][b, (q % 2) * OWN:(q % 2 + 1) * OWN, :]  # [384, 1024]
        m = dict(shared)
        m["x_t"] = bf(x.astype(np.float64).T)
        ewf = I[f"{mod}_w"].astype(np.float64) \
            * I[f"{mod}_ln1_g"].astype(np.float64)[:, None]
        m["emb_w"] = bf(ewf)
        m["emb_b"] = col(I[f"{mod}_b"].astype(np.float64)
                         + I[f"{mod}_ln1_b"].astype(np.float64)
                         @ I[f"{mod}_w"].astype(np.float64))
        m["eln2_g"] = col(I[f"{mod}_ln2_g"])
        m["eln2_b"] = col(I[f"{mod}_ln2_b"])
        in_maps.append(m)
    return in_maps, ret


def kernel(**inputs):
    from concourse import bass_utils
    nc = build(num_devices=N_CORES, use_cc=True)
    in_maps, ret = _prep_inputs(inputs)
    res = bass_utils.run_bass_kernel_spmd(nc, in_maps,
                                          core_ids=list(range(N_CORES)))
    out = np.zeros((B, 3, D), np.float32)
    for b in range(2):
        r = res.results[4 * b]
        u = r["out_u"][:, 0]
        f = r["out_f"][0]
        out[b, 0] = u + ret[0]
        out[b, 1] = u + ret[1]
        out[b, 2] = f + ret[2]
    return out



# revision 11
# speedup vs baseline: 1.2450x; 1.2450x over previous
"""BioZorro sparse-attention kernel for 8 Trainium2 NeuronCores.

Sharding: 8 cores = 2 batches x 4 token-quarters (384 own tokens each).
The zorro mask makes all non-fusion query rows fully masked -> uniform
softmax -> their attention output is mean(V); only the 16 fusion tokens
attend (over the 1536 non-fusion keys). Cross-core data per layer is two
small AllGathers: (A) V column sums (2KB) issued early, (B) fusion
flash-softmax partials (66KB), plus one tiny AllGather for pooling.

Compute layout: residual stream feature-major (tok^T [512, 400] f32).
All heavy matmuls run in fp8e4 DoubleRow (contract 256/instr, 2x rate):
activations are cast to paired [128,2,T] tiles; weights are host-packed
into one fp8 buffer per layer (single DMA, double-buffered). LayerNorms
are folded into consumers: raw-cast -> matmul immediately; the -mu
correction enters PSUM as a rank-1 matmul (host-precomputed column sums
x the device S row); rstd is applied at PSUM eviction (column-broadcast
or per-token scalars). Per-tensor power-of-2 fp8 scales are descaled via
free immediate-scale slots (exp/gelu/copy activations).
"""
import sys
sys.path.insert(0, "/opt/trn_rl_repo")
import numpy as np
import ml_dtypes

BF = ml_dtypes.bfloat16
F8 = ml_dtypes.float8_e4m3
OWN, FUS, TOK = 384, 16, 400
D, RIN, H, DH, IFF, DEPTH = 512, 1024, 8, 64, 1365, 4
NALL = 1552
B, NR, NA = 2, 768, 768
N_CORES = 8
IFFP = 1408           # x/gate block padding (11 x 128)
IFF2 = 1536           # FF2 contract padding (6 x 256)

# fp8 packed-weight segment offsets (cols in the per-layer [128, FCOLS])
SEG_WQ = 0            # [2kp][2sub][512]
SEG_WKV = 2048        # [2kp][2sub][1024]
SEG_WO = SEG_WKV + 4096   # [2kp][2sub][512]
SEG_W1 = SEG_WO + 2048    # [2kp][2sub][2*1408]
SEG_W2 = SEG_W1 + 11264   # [6jp][2sub][512]
FCOLS = SEG_W2 + 6144

_built = {}


def _pow2_scale(w, target=120.0):
    m = float(np.abs(w).max())
    if m <= 0:
        return 1.0
    return float(2.0 ** np.floor(np.log2(target / m)))


def _pack_pairs(w, scale):
    """[K, N] f64 -> [128, K//256, 2, N] fp8 DoubleRow lhsT layout."""
    K, N = w.shape
    assert K % 256 == 0
    out = (w * scale).astype(F8).reshape(K // 128, 128, N)
    # chunk k = rows 128k..128k+128; pair kp = (2kp, 2kp+1)
    out = out.transpose(1, 0, 2).reshape(128, K // 256, 2, N)
    return np.ascontiguousarray(out)


def build(num_devices=8, use_cc=True, scales=None):
    key = (num_devices, use_cc, scales)
    if key in _built:
        return _built[key]
    import concourse.tile as tile
    from concourse import bacc, mybir
    from concourse.masks import make_identity

    # Force Exp to resolve to natural_log_exp_and_others so Ln/Exp/Square
    # live in one ACT table set (Gelu still needs its own set; those two
    # swaps per layer are prefetched off the critical path with dummy ops).
    if not getattr(bacc, "_act_tables_patched", False):
        _orig_gat = bacc.get_activation_tables

        def _patched_gat(arch):
            tabs = _orig_gat(arch)
            exp_t = mybir.ActivationFunctionType.Exp
            for nm, fns in tabs.items():
                if nm != "natural_log_exp_and_others":
                    fns.discard(exp_t)
            return tabs

        bacc.get_activation_tables = _patched_gat
        bacc._act_tables_patched = True

    sq, skv, so, s1x, s1g, s2, se, spl, spo = scales
    f32 = mybir.dt.float32
    bf16 = mybir.dt.bfloat16
    f8 = mybir.dt.float8e4
    AF = mybir.ActivationFunctionType
    OP = mybir.AluOpType
    DR = mybir.MatmulPerfMode.DoubleRow

    nc = bacc.Bacc("TRN2", target_bir_lowering=False, debug=False,
                   enable_asserts=True, num_devices=num_devices)

    def din(name, shape, dt=f32):
        return nc.dram_tensor(name, shape, dt, kind="ExternalInput").ap()

    x8_t = din("x8", [128, 4, 2, OWN], f8)
    ew8_t = din("ew8", [128, 4, 2, D], f8)
    ebias_t = din("emb_b", [D, 1])
    eg2_t = din("eln2_g", [D, 1])
    eb2_t = din("eln2_b", [D, 1])
    fus_t = din("fus_t", [128, 4, FUS], f32)
    wpk_t = din("wpk", [DEPTH, 128, FCOLS], f8)
    # host rank-1 rows: per layer [wksum, wvsum, wqsum] each [512]
    rows_t = din("rows", [1, DEPTH * 3 * D], bf16)
    erow_t = din("erow", [1, D], bf16)        # embed -se*colsum(ew')/128
    prow_t = din("prow", [1, 2 * D], bf16)    # pool [pwksum, pwvsum]
    pwkv8_t = din("pwkv8", [128, 2, 2, 2 * D], f8)
    pwo8_t = din("pwo8", [128, 2, 2, D], f8)
    pq2_t = din("pool_q2", [D, 1])
    out_u = nc.dram_tensor("out_u", [D, 1], f32, kind="ExternalOutput").ap()
    out_f = nc.dram_tensor("out_f", [1, D], f32, kind="ExternalOutput").ap()

    with tile.TileContext(nc) as tc:
        with tc.tile_pool(name="cst", bufs=1) as cst, \
             tc.tile_pool(name="wp", bufs=2) as wp, \
             tc.tile_pool(name="ac", bufs=2) as ac, \
             tc.tile_pool(name="pgen", bufs=4, space="PSUM") as pgen, \
             tc.tile_pool(name="pacc", bufs=4, space="PSUM") as pacc, \
             tc.tile_pool(name="dramp", bufs=2, space="DRAM") as dramp:

            ident = cst.tile([128, 128], bf16, name="ident")
            make_identity(nc, ident[:])
            ones128 = cst.tile([128, 1], bf16, name="ones128")
            nc.vector.memset(ones128[:], 1.0)
            ones1 = cst.tile([1, 128], bf16, name="ones1")
            nc.vector.memset(ones1[:], 1.0)
            epsc = cst.tile([128, 1], f32, name="epsc")
            nc.vector.memset(epsc[:], 1e-5)
            oi512 = cst.tile([128, 1], bf16, name="oi512")
            nc.vector.memset(oi512[:], 1.0 / 512)
            ones8p = cst.tile([128, 1], f8, name="ones8p")
            nc.vector.memset(ones8p[:], 0.125)
            cinv = cst.tile([128, 1], f32, name="cinv")
            nc.vector.memset(cinv[:], 1.0 / (s1x * s2))

            def load_cols(dram_ap, n, tag, rows=128):
                ts = []
                for c in range(n):
                    t = wp.tile([rows, 1], f32, tag=f"{tag}{c}", bufs=1,
                                name=f"{tag}{c}")
                    nc.sync.dma_start(out=t[:],
                                      in_=dram_ap[rows * c:rows * (c + 1), :])
                    ts.append(t)
                return ts

            # ---------- one-time loads ----------
            x8 = ac.tile([128, 4, 2, OWN], f8, tag="x8", bufs=1, name="x8")
            nc.sync.dma_start(out=x8[:], in_=x8_t)
            ew8 = wp.tile([128, 4, 2, D], f8, tag="ew8", bufs=1, name="ew8")
            nc.sync.dma_start(out=ew8[:], in_=ew8_t)
            rows = wp.tile([1, DEPTH * 3 * D], bf16, tag="rows", bufs=1,
                           name="rows")
            nc.sync.dma_start(out=rows[:], in_=rows_t)
            erow = wp.tile([1, D], bf16, tag="erow", bufs=1, name="erow")
            nc.sync.dma_start(out=erow[:], in_=erow_t)
            ebs = load_cols(ebias_t, 4, "ebias")
            eg2s = load_cols(eg2_t, 4, "eg2")
            eb2s = load_cols(eb2_t, 4, "eb2")

            # layer-0 weights DMA starts immediately
            wts = []
            w0 = wp.tile([128, FCOLS], f8, tag="wpk", bufs=2, name="wpk0")
            nc.sync.dma_start(out=w0[:], in_=wpk_t[0])

            # dummy exp to preload the nlexp ACT table during initial DMAs
            dtab = ac.tile([1, 1], f32, tag="dtab", bufs=2, name="dtab")
            nc.scalar.activation(out=dtab[:], in_=epsc[0:1, :], func=AF.Exp)

            # ---------- embed ----------
            # stats over raw x8 (LN1 folded into the embed matmul)
            Se = pgen.tile([1, OWN], f32, tag="g", name="Se")
            for i in range(8):
                nc.tensor.matmul(Se[:], ones8p[:], x8[:, i // 2, i % 2, :],
                                 start=(i == 0), stop=(i == 7))
            Seb = ac.tile([1, OWN], bf16, tag="rowb", bufs=4, name="Seb")
            nc.scalar.activation(out=Seb[:], in_=Se[:], func=AF.Copy)
            xsqe = []
            for kp in range(4):
                t = ac.tile([128, 2, OWN], bf16, tag="xsq", bufs=4,
                            name=f"xsqe{kp}")
                nc.scalar.activation(out=t[:], in_=x8[:, kp, :, :],
                                     func=AF.Square)
                xsqe.append(t)
            Qe = pgen.tile([1, OWN], f32, tag="g", name="Qe")
            oi1024 = cst.tile([128, 1], bf16, name="oi1024")
            nc.vector.memset(oi1024[:], 1.0 / 1024)
            for i in range(8):
                nc.tensor.matmul(Qe[:], oi1024[:], xsqe[i // 2][:, i % 2, :],
                                 start=(i == 0), stop=(i == 7))
            m2e = ac.tile([1, OWN], f32, tag="rowf", bufs=6, name="m2e")
            nc.scalar.activation(out=m2e[:], in_=Se[:], func=AF.Square,
                                 scale=1.0 / 128)
            vare = ac.tile([1, OWN], f32, tag="rowf", bufs=6, name="vare")
            nc.vector.tensor_sub(out=vare[:], in0=Qe[:], in1=m2e[:])
            rstde = ac.tile([1, OWN], f32, tag="rowf", bufs=6, name="rstde")
            nc.scalar.activation(out=rstde[:], in_=vare[:], func=AF.Ln,
                                 bias=epsc[0:1, :])
            nc.scalar.activation(out=rstde[:], in_=rstde[:], func=AF.Exp,
                                 scale=-0.5)
            rstdeb = ac.tile([1, OWN], bf16, tag="rowb", bufs=4, name="rstdeb")
            nc.scalar.activation(out=rstdeb[:], in_=rstde[:], func=AF.Copy,
                                 scale=1.0 / se)
            bRe_p = pgen.tile([128, OWN], f32, tag="g", name="bRe")
            nc.tensor.matmul(bRe_p[:], ones1[:], rstdeb[:], start=True,
                             stop=True)
            bRe = ac.tile([128, OWN], bf16, tag="bR", bufs=2, name="bReb")
            nc.vector.tensor_copy(out=bRe[:], in_=bRe_p[:])

            hb = []
            for mc in range(4):
                ps = pgen.tile([128, OWN], f32, tag="g", name=f"embp{mc}")
                for kp in range(4):
                    nc.tensor.matmul(ps[:], ew8[:, kp, :, 128 * mc:128 * (mc + 1)],
                                     x8[:, kp, :, :], start=(kp == 0),
                                     stop=False, perf_mode=DR)
                nc.tensor.matmul(ps[:], erow[:, 128 * mc:128 * (mc + 1)],
                                 Seb[:], start=False, stop=True)
                t1 = ac.tile([128, OWN], bf16, tag="embt", bufs=2,
                             name=f"embt{mc}")
                nc.vector.tensor_mul(out=t1[:], in0=ps[:], in1=bRe[:])
                t2 = ac.tile([128, OWN], bf16, tag=f"hb{mc}", bufs=1,
                             name=f"hb{mc}")
                nc.vector.tensor_scalar_add(out=t2[:], in0=t1[:],
                                            scalar1=ebs[mc][:])
                hb.append(t2)

            # embed LN2 (explicit normalize into f32 tok)
            S2e = pgen.tile([1, OWN], f32, tag="g", name="S2e")
            for c in range(4):
                nc.tensor.matmul(S2e[:], oi512[:], hb[c][:],
                                 start=(c == 0), stop=(c == 3))
            x2e = []
            for c in range(4):
                t = ac.tile([128, OWN], bf16, tag="xsq", bufs=4,
                            name=f"x2e{c}")
                nc.vector.tensor_mul(out=t[:], in0=hb[c][:], in1=hb[c][:])
                x2e.append(t)
            Q2e = pgen.tile([1, OWN], f32, tag="g", name="Q2e")
            for c in range(4):
                nc.tensor.matmul(Q2e[:], oi512[:], x2e[c][:],
                                 start=(c == 0), stop=(c == 3))
            m22 = ac.tile([1, OWN], f32, tag="rowf", bufs=6, name="m22")
            nc.scalar.activation(out=m22[:], in_=S2e[:], func=AF.Square)
            var2 = ac.tile([1, OWN], f32, tag="rowf", bufs=6, name="var2e")
            nc.vector.tensor_sub(out=var2[:], in0=Q2e[:], in1=m22[:])
            rstd2e = ac.tile([1, OWN], f32, tag="rowf", bufs=6, name="rstd2e")
            nc.scalar.activation(out=rstd2e[:], in_=var2[:], func=AF.Ln,
                                 bias=epsc[0:1, :])
            nc.scalar.activation(out=rstd2e[:], in_=rstd2e[:], func=AF.Exp,
                                 scale=-0.5)
            pr2 = ac.tile([1, 2 * OWN], bf16, tag="rowb2", bufs=2, name="pr2e")
            nc.vector.tensor_copy(out=pr2[:, 0:OWN], in_=rstd2e[:])
            mre = ac.tile([1, OWN], f32, tag="rowf", bufs=6, name="mre")
            nc.vector.tensor_mul(out=mre[:], in0=S2e[:], in1=rstd2e[:])
            nc.vector.tensor_copy(out=pr2[:, OWN:2 * OWN], in_=mre[:])
            bR2e_p = pgen.tile([128, OWN], f32, tag="g", name="bR2e")
            nc.tensor.matmul(bR2e_p[:], ones1[:], pr2[:, 0:OWN], start=True,
                             stop=True)
            bR2e = ac.tile([128, OWN], bf16, tag="bR", bufs=2, name="bR2eb")
            nc.vector.tensor_copy(out=bR2e[:], in_=bR2e_p[:])
            bM2e_p = pgen.tile([128, OWN], f32, tag="g", name="bM2e")
            nc.tensor.matmul(bM2e_p[:], ones1[:], pr2[:, OWN:2 * OWN],
                             start=True, stop=True)
            bM2e = ac.tile([128, OWN], bf16, tag="bM", bufs=2, name="bM2eb")
            nc.vector.tensor_copy(out=bM2e[:], in_=bM2e_p[:])

            tok = [ac.tile([128, TOK], f32, tag=f"tok{c}", bufs=1,
                           name=f"tok{c}") for c in range(4)]
            for c in range(4):
                t1 = ac.tile([128, OWN], bf16, tag="embt", bufs=2,
                             name=f"eln{c}")
                nc.vector.tensor_mul(out=t1[:], in0=hb[c][:], in1=bR2e[:])
                nc.vector.tensor_sub(out=t1[:], in0=t1[:], in1=bM2e[:])
                nc.vector.tensor_scalar(out=tok[c][:, 0:OWN], in0=t1[:],
                                        scalar1=eg2s[c][:], scalar2=eb2s[c][:],
                                        op0=OP.mult, op1=OP.add)
                nc.sync.dma_start(out=tok[c][:, OWN:TOK], in_=fus_t[:, c, :])

            tok_chunks = [(0, 128), (128, 256), (256, 384), (384, 400)]
            rg = [[0, 1, 2, 3], [4, 5, 6, 7]]
            psO_prev = None

            # rank-1 row views per layer
            def lrow(l, which):
                base = (l * 3 + which) * D
                return rows[:, base:base + D]

            # ---------- layers ----------
            for l in range(DEPTH):
                wT = w0 if l == 0 else wp.tile([128, FCOLS], f8, tag="wpk",
                                               bufs=2, name=f"wpk{l}")
                if l > 0:
                    nc.sync.dma_start(out=wT[:], in_=wpk_t[l])
                wq8 = wT[:, SEG_WQ:SEG_WKV].rearrange(
                    "p (kp s o) -> p kp s o", kp=2, s=2)
                wkv8 = wT[:, SEG_WKV:SEG_WO].rearrange(
                    "p (kp s o) -> p kp s o", kp=2, s=2)
                wo8 = wT[:, SEG_WO:SEG_W1].rearrange(
                    "p (kp s o) -> p kp s o", kp=2, s=2)
                w18 = wT[:, SEG_W1:SEG_W2].rearrange(
                    "p (kp s o) -> p kp s o", kp=2, s=2)
                w28 = wT[:, SEG_W2:FCOLS].rearrange(
                    "p (jp s o) -> p jp s o", jp=6, s=2)

                # --- LN1 prep: residual + fp8 raw cast ---
                tr8 = [ac.tile([128, 2, TOK], f8, tag=f"tr8{kp}", bufs=1,
                               name=f"tr8_{kp}") for kp in range(2)]
                for c in range(4):
                    if psO_prev is not None:
                        t = ac.tile([128, TOK], bf16, tag="psot", bufs=2,
                                    name=f"psot{c}")
                        nc.vector.tensor_scalar_mul(out=t[:],
                                                    in0=psO_prev[c][:],
                                                    scalar1=cinv[:])
                        nc.vector.tensor_add(out=tok[c][:], in0=tok[c][:],
                                             in1=t[:])
                    nc.vector.tensor_copy(out=tr8[c // 2][:, c % 2, :],
                                          in_=tok[c][:])
                psO_prev = None

                # --- stats1 (rank-1 style folded LN) ---
                S1 = pgen.tile([1, TOK], f32, tag="g", name="S1")
                for i in range(4):
                    nc.tensor.matmul(S1[:], ones8p[:], tr8[i // 2][:, i % 2, :],
                                     start=(i == 0), stop=(i == 3))
                S1b = ac.tile([1, TOK], bf16, tag="rowb", bufs=4, name="S1b")
                nc.scalar.activation(out=S1b[:], in_=S1[:], func=AF.Copy)
                xsq = []
                for kp in range(2):
                    t = ac.tile([128, 2, TOK], bf16, tag="xsq", bufs=4,
                                name=f"xsq{kp}")
                    nc.scalar.activation(out=t[:], in_=tr8[kp][:],
                                         func=AF.Square)
                    xsq.append(t)
                Q1 = pgen.tile([1, TOK], f32, tag="g", name="Q1")
                for i in range(4):
                    nc.tensor.matmul(Q1[:], oi512[:], xsq[i // 2][:, i % 2, :],
                                     start=(i == 0), stop=(i == 3))
                m2 = ac.tile([1, TOK], f32, tag="rowf", bufs=6, name="m2")
                nc.scalar.activation(out=m2[:], in_=S1[:], func=AF.Square,
                                     scale=1.0 / 64)
                var = ac.tile([1, TOK], f32, tag="rowf", bufs=6, name="var")
                nc.vector.tensor_sub(out=var[:], in0=Q1[:], in1=m2[:])
                rstd = ac.tile([1, TOK], f32, tag="rowf", bufs=6, name="rstd")
                nc.scalar.activation(out=rstd[:], in_=var[:], func=AF.Ln,
                                     bias=epsc[0:1, :])
                nc.scalar.activation(out=rstd[:], in_=rstd[:], func=AF.Exp,
                                     scale=-0.5)
                rstdb = ac.tile([1, TOK], bf16, tag="rowb", bufs=4,
                                name="rstdb")
                nc.vector.tensor_copy(out=rstdb[:], in_=rstd[:])
                bR1_p = pgen.tile([128, TOK], f32, tag="g", name="bR1")
                nc.tensor.matmul(bR1_p[:], ones1[:], rstdb[:], start=True,
                                 stop=True)
                bR1 = ac.tile([128, TOK], bf16, tag="bR", bufs=2, name="bR1b")
                nc.vector.tensor_copy(out=bR1[:], in_=bR1_p[:])
                rT_p = pacc.tile([128, 4], f32, tag="a", name="rTp")
                for c, (a_, b_) in enumerate(tok_chunks):
                    nc.tensor.matmul(rT_p[0:b_ - a_, c:c + 1],
                                     rstdb[:, a_:b_],
                                     ones1[0:1, 0:1], start=True, stop=True)
                rT = ac.tile([128, 4], f32, tag="rT", bufs=2, name="rT")
                nc.scalar.activation(out=rT[:], in_=rT_p[:], func=AF.Copy,
                                     scale=1.0 / skv)

                # --- V (+ vsums -> exchange A) ---
                V = []
                for i, (a, b) in enumerate(tok_chunks):
                    m = b - a
                    ps = pacc.tile([128, D], f32, tag="a", name=f"v{i}")
                    for kp in range(2):
                        nc.tensor.matmul(ps[0:m, :], tr8[kp][:, :, a:b],
                                         wkv8[:, kp, :, D:2 * D],
                                         start=(kp == 0), stop=False,
                                         perf_mode=DR)
                    nc.tensor.matmul(ps[0:m, :], S1b[:, a:b],
                                     lrow(l, 1), start=False, stop=True)
                    s = ac.tile([128, D], bf16, tag=f"V{i}", bufs=1,
                                name=f"Vb{i}")
                    nc.vector.tensor_scalar_mul(out=s[0:m, :], in0=ps[0:m, :],
                                                scalar1=rT[0:m, i:i + 1])
                    V.append(s)
                PA = ac.tile([128, 4], f32, tag="PA", bufs=2, name="PA")
                for c in range(4):
                    ps = pgen.tile([128, 1], f32, tag="g", name=f"vs{c}")
                    for j in range(3):
                        nc.tensor.matmul(ps[:], V[j][:, 128 * c:128 * (c + 1)],
                                         ones128[:], start=(j == 0),
                                         stop=(j == 2))
                    nc.vector.tensor_copy(out=PA[:, c:c + 1], in_=ps[:])
                vfu = ac.tile([128, 4], f32, tag="vfu", bufs=2, name="vfu")
                for c in range(4):
                    ps = pgen.tile([128, 1], f32, tag="g", name=f"vf{c}")
                    nc.tensor.matmul(ps[:], V[3][0:FUS, 128 * c:128 * (c + 1)],
                                     ones128[0:FUS, :], start=True, stop=True)
                    nc.vector.tensor_copy(out=vfu[:, c:c + 1], in_=ps[:])
                pinA = dramp.tile([128, 4], f32, tag="pinA", bufs=2,
                                  name="pinA")
                nc.sync.dma_start(out=pinA[:], in_=PA[:])
                RA = ac.tile([128, 4, 4], f32, tag="RA", bufs=2, name="RA")
                if use_cc:
                    poutA = dramp.tile([4 * 128, 4], f32, tag="poutA", bufs=2,
                                       name="poutA")
                    nc.gpsimd.collective_compute(
                        "AllGather", OP.bypass, replica_groups=rg,
                        ins=[pinA.opt()], outs=[poutA.opt()])
                    nc.sync.dma_start(
                        out=RA[:], in_=poutA.rearrange("(r p) f -> p r f", r=4))
                else:
                    nc.sync.dma_start(
                        out=RA[:],
                        in_=pinA.rearrange("(r p) f -> p r f", r=1)
                        .to_broadcast((128, 4, 4)))

                # --- K^T, Qf ---
                kt = []
                for mc in range(4):
                    ps = pgen.tile([128, OWN], f32, tag="g", name=f"kt{mc}")
                    for kp in range(2):
                        nc.tensor.matmul(ps[:],
                                         wkv8[:, kp, :, 128 * mc:128 * (mc + 1)],
                                         tr8[kp][:, :, 0:OWN],
                                         start=(kp == 0), stop=False,
                                         perf_mode=DR)
                    nc.tensor.matmul(ps[:], lrow(l, 0)[:, 128 * mc:128 * (mc + 1)],
                                     S1b[:, 0:OWN], start=False, stop=True)
                    s = ac.tile([128, OWN], bf16, tag=f"kt{mc}", bufs=1,
                                name=f"ktb{mc}")
                    nc.vector.tensor_mul(out=s[:], in0=ps[:],
                                         in1=bR1[:, 0:OWN])
                    kt.append(s)
                qf = []
                for mc in range(4):
                    ps = pgen.tile([128, FUS], f32, tag="g", name=f"qf{mc}")
                    for kp in range(2):
                        nc.tensor.matmul(ps[:],
                                         wq8[:, kp, :, 128 * mc:128 * (mc + 1)],
                                         tr8[kp][:, :, OWN:TOK],
                                         start=(kp == 0), stop=False,
                                         perf_mode=DR)
                    nc.tensor.matmul(ps[:], lrow(l, 2)[:, 128 * mc:128 * (mc + 1)],
                                     S1b[:, OWN:TOK], start=False, stop=True)
                    s = ac.tile([128, 32], bf16, tag=f"qf{mc}", bufs=1,
                                name=f"qfb{mc}")
                    nc.vector.memset(s[:, FUS:32], 0.0)
                    nc.vector.tensor_mul(out=s[:, 0:FUS], in0=ps[:],
                                         in1=bR1[:, OWN:TOK])
                    qf.append(s)

                # --- scores + exp ---
                E, lacc = [], []
                for t in range(2):
                    sp = pgen.tile([128, OWN], f32, tag="g", name=f"sp{t}")
                    for i in range(4):
                        h = 4 * t + i
                        ch, base = h // 2, (h % 2) * 64
                        nc.tensor.matmul(sp[32 * i:32 * i + 32, :],
                                         qf[ch][base:base + 64, 0:32],
                                         kt[ch][base:base + 64, :],
                                         start=True, stop=True,
                                         tile_position=(base, 32 * i))
                    e = ac.tile([128, OWN], bf16, tag=f"e{t}", bufs=1,
                                name=f"e{t}")
                    la = ac.tile([128, 1], f32, tag=f"la{t}", bufs=2,
                                 name=f"la{t}")
                    nc.scalar.activation(out=e[:], in_=sp[:], func=AF.Exp,
                                         scale=1.0 / (sq * skv),
                                         accum_out=la[:])
                    E.append(e)
                    lacc.append(la)

                # --- E^T + ACC -> exchange B ---
                ET = [[None] * 3 for _ in range(2)]
                for t in range(2):
                    for j in range(3):
                        pt = pgen.tile([128, 128], bf16, tag="g",
                                       name=f"et{t}{j}")
                        nc.tensor.transpose(pt[:], E[t][:, 128 * j:128 * (j + 1)],
                                            ident[:])
                        s = ac.tile([128, 128], bf16, tag=f"ET{t}{j}", bufs=1,
                                    name=f"ETb{t}{j}")
                        nc.vector.tensor_copy(out=s[:], in_=pt[:])
                        ET[t][j] = s
                PB = ac.tile([128, 130], f32, tag="PB", bufs=2, name="PB")
                nc.vector.tensor_copy(out=PB[:, 0:1], in_=lacc[0][:])
                nc.vector.tensor_copy(out=PB[:, 1:2], in_=lacc[1][:])
                for t in range(2):
                    acc = pacc.tile([128, 64], f32, tag="a", name=f"acc{t}")
                    for i in range(4):
                        h = 4 * t + i
                        for j in range(3):
                            nc.tensor.matmul(acc[32 * i:32 * i + 32, :],
                                             ET[t][j][:, 32 * i:32 * i + 32],
                                             V[j][:, DH * h:DH * (h + 1)],
                                             start=(j == 0), stop=(j == 2),
                                             tile_position=(0, 32 * i))
                    nc.vector.tensor_copy(out=PB[:, 2 + 64 * t:66 + 64 * t],
                                          in_=acc[:])
                pinB = dramp.tile([128, 130], f32, tag="pinB", bufs=2,
                                  name="pinB")
                nc.sync.dma_start(out=pinB[:], in_=PB[:])
                RB = ac.tile([128, 4, 130], f32, tag="RB", bufs=2, name="RB")
                if use_cc:
                    poutB = dramp.tile([4 * 128, 130], f32, tag="poutB",
                                       bufs=2, name="poutB")
                    nc.gpsimd.collective_compute(
                        "AllGather", OP.bypass, replica_groups=rg,
                        ins=[pinB.opt()], outs=[poutB.opt()])
                    nc.sync.dma_start(
                        out=RB[:], in_=poutB.rearrange("(r p) f -> p r f", r=4))
                else:
                    nc.sync.dma_start(
                        out=RB[:],
                        in_=pinB.rearrange("(r p) f -> p r f", r=1)
                        .to_broadcast((128, 4, 130)))

                # --- A-combine -> uniform delta -> own-col LN2 prep ---
                vT2 = ac.tile([128, 2, 4], f32, tag="cmb", bufs=3, name="vT2")
                nc.vector.tensor_add(out=vT2[:], in0=RA[:, 0:2, :],
                                     in1=RA[:, 2:4, :])
                vsb = ac.tile([128, 4], f32, tag="vsb", bufs=2, name="vsb")
                nc.vector.tensor_add(out=vsb[:], in0=vT2[:, 0, :],
                                     in1=vT2[:, 1, :])
                nc.vector.tensor_add(out=vsb[:], in0=vsb[:], in1=vfu[:])
                vsb8 = ac.tile([128, 4], f8, tag="vsb8", bufs=2,
                               name="vsb8")
                nc.vector.tensor_copy(out=vsb8[:], in_=vsb[:])
                dup = pgen.tile([128, 4], f32, tag="g", name="dup")
                for c in range(4):
                    for kc in range(4):
                        nc.tensor.matmul(dup[:, c:c + 1],
                                         wo8[:, kc // 2, kc % 2,
                                             128 * c:128 * (c + 1)],
                                         vsb8[:, kc:kc + 1],
                                         start=(kc == 0), stop=(kc == 3))
                dus = ac.tile([128, 4], f32, tag="dus", bufs=2, name="dus")
                nc.scalar.activation(out=dus[:], in_=dup[:], func=AF.Copy,
                                     scale=1.0 / (so * NALL))
                t28 = [ac.tile([128, 2, TOK], f8, tag=f"t28_{kp}", bufs=1,
                               name=f"t28_{kp}") for kp in range(2)]
                for c in range(4):
                    nc.vector.tensor_scalar_add(out=tok[c][:, 0:OWN],
                                                in0=tok[c][:, 0:OWN],
                                                scalar1=dus[:, c:c + 1])
                    nc.vector.tensor_copy(out=t28[c // 2][:, c % 2, 0:OWN],
                                          in_=tok[c][:, 0:OWN])
                S2 = pgen.tile([1, TOK], f32, tag="g", name="S2")
                for i in range(4):
                    nc.tensor.matmul(S2[:, 0:OWN], ones8p[:],
                                     t28[i // 2][:, i % 2, 0:OWN],
                                     start=(i == 0), stop=(i == 3))
                xsq2 = []
                for kp in range(2):
                    t = ac.tile([128, 2, TOK], bf16, tag="xsq", bufs=4,
                                name=f"xsq2{kp}")
                    nc.scalar.activation(out=t[:, :, 0:OWN],
                                         in_=t28[kp][:, :, 0:OWN],
                                         func=AF.Square)
                    xsq2.append(t)
                Q2 = pgen.tile([1, TOK], f32, tag="g", name="Q2")
                for i in range(4):
                    nc.tensor.matmul(Q2[:, 0:OWN], oi512[:],
                                     xsq2[i // 2][:, i % 2, 0:OWN],
                                     start=(i == 0), stop=(i == 3))

                # --- B-combine -> fusion delta -> fusion-col LN2 prep ---
                T01 = ac.tile([128, 130], f32, tag="cmbB", bufs=3, name="T01")
                nc.vector.tensor_add(out=T01[:], in0=RB[:, 0, :],
                                     in1=RB[:, 1, :])
                T23 = ac.tile([128, 130], f32, tag="cmbB", bufs=3, name="T23")
                nc.vector.tensor_add(out=T23[:], in0=RB[:, 2, :],
                                     in1=RB[:, 3, :])
                PT = ac.tile([128, 130], f32, tag="cmbB", bufs=3, name="PT")
                nc.vector.tensor_add(out=PT[:], in0=T01[:], in1=T23[:])
                linv = ac.tile([128, 2], f32, tag="linv", bufs=2, name="linv")
                nc.vector.reciprocal(out=linv[:], in_=PT[:, 0:2])
                of = []
                for t in range(2):
                    s = ac.tile([128, 64], bf16, tag=f"of{t}", bufs=1,
                                name=f"of{t}")
                    nc.vector.tensor_scalar_mul(out=s[:],
                                                in0=PT[:, 2 + 64 * t:66 + 64 * t],
                                                scalar1=linv[:, t:t + 1])
                    of.append(s)
                ofI8 = [ac.tile([128, 2, FUS], f8, tag=f"ofI{kp}", bufs=1,
                                name=f"ofI{kp}") for kp in range(2)]
                for kc in range(4):
                    pt = pacc.tile([128, 32], bf16, tag="a", name=f"ofIp{kc}")
                    for half in range(2):
                        h = 2 * kc + half
                        t, i = h // 4, h % 4
                        nc.tensor.transpose(
                            pt[64 * half:64 * half + 64, :],
                            of[t][32 * i:32 * i + 32, :],
                            ident[32 * i:32 * i + 32, 32 * i:32 * i + 32],
                            tile_position=(32 * i, 64 * half))
                    nc.vector.tensor_copy(out=ofI8[kc // 2][:, kc % 2, :],
                                          in_=pt[:, 0:FUS])
                dfp = pacc.tile([FUS, D], f32, tag="a", name="dfp")
                for kp in range(2):
                    nc.tensor.matmul(dfp[:], ofI8[kp][:], wo8[:, kp, :, :],
                                     start=(kp == 0), stop=(kp == 1),
                                     perf_mode=DR)
                dfb = ac.tile([FUS, D], bf16, tag="dfb", bufs=2, name="dfb")
                nc.scalar.activation(out=dfb[:], in_=dfp[:], func=AF.Copy,
                                     scale=1.0 / so)
                for c in range(4):
                    pt = pacc.tile([128, FUS], bf16, tag="a", name=f"dft{c}")
                    nc.tensor.transpose(pt[:], dfb[0:FUS, 128 * c:128 * (c + 1)],
                                        ident[0:FUS, 0:FUS])
                    nc.vector.tensor_add(out=tok[c][:, OWN:TOK],
                                         in0=tok[c][:, OWN:TOK], in1=pt[:])
                    nc.vector.tensor_copy(out=t28[c // 2][:, c % 2, OWN:TOK],
                                          in_=tok[c][:, OWN:TOK])
                for i in range(4):
                    nc.tensor.matmul(S2[:, OWN:TOK], ones8p[:],
                                     t28[i // 2][:, i % 2, OWN:TOK],
                                     start=(i == 0), stop=(i == 3))
                for kp in range(2):
                    nc.scalar.activation(out=xsq2[kp][:, :, OWN:TOK],
                                         in_=t28[kp][:, :, OWN:TOK],
                                         func=AF.Square)
                for i in range(4):
                    nc.tensor.matmul(Q2[:, OWN:TOK], oi512[:],
                                     xsq2[i // 2][:, i % 2, OWN:TOK],
                                     start=(i == 0), stop=(i == 3))

                # --- stats2 tail + normalize-cast ---
                m2_2 = ac.tile([1, TOK], f32, tag="rowf", bufs=6, name="m2_2")
                nc.scalar.activation(out=m2_2[:], in_=S2[:], func=AF.Square,
                                     scale=1.0 / 64)
                var2 = ac.tile([1, TOK], f32, tag="rowf", bufs=6, name="var2")
                nc.vector.tensor_sub(out=var2[:], in0=Q2[:], in1=m2_2[:])
                rstd2 = ac.tile([1, TOK], f32, tag="rowf", bufs=6,
                                name="rstd2")
                nc.scalar.activation(out=rstd2[:], in_=var2[:], func=AF.Ln,
                                     bias=epsc[0:1, :])
                nc.scalar.activation(out=rstd2[:], in_=rstd2[:], func=AF.Exp,
                                     scale=-0.5)
                pr2l = ac.tile([1, 2 * TOK], bf16, tag="rowb2", bufs=2,
                               name="pr2l")
                nc.vector.tensor_copy(out=pr2l[:, 0:TOK], in_=rstd2[:])
                murs2 = ac.tile([1, TOK], f32, tag="rowf", bufs=6,
                                name="murs2")
                nc.vector.tensor_mul(out=murs2[:], in0=S2[:], in1=rstd2[:])
                nc.scalar.activation(out=pr2l[:, TOK:2 * TOK], in_=murs2[:],
                                     func=AF.Copy, scale=1.0 / 64)
                bR2_p = pgen.tile([128, TOK], f32, tag="g", name="bR2")
                nc.tensor.matmul(bR2_p[:], ones1[:], pr2l[:, 0:TOK],
                                 start=True, stop=True)
                bR2 = ac.tile([128, TOK], bf16, tag="bR", bufs=2, name="bR2b")
                nc.vector.tensor_copy(out=bR2[:], in_=bR2_p[:])
                bM2_p = pgen.tile([128, TOK], f32, tag="g", name="bM2")
                nc.tensor.matmul(bM2_p[:], ones1[:], pr2l[:, TOK:2 * TOK],
                                 start=True, stop=True)
                bM2 = ac.tile([128, TOK], bf16, tag="bM", bufs=2, name="bM2b")
                nc.vector.tensor_copy(out=bM2[:], in_=bM2_p[:])
                # dummy gelu: preload the gelu table while FF1 matmuls run
                dt2 = ac.tile([1, 1], f32, tag="dtab", bufs=2, name="dt2")
                nc.scalar.activation(out=dt2[:], in_=rstd2[0:1, 0:1],
                                     func=AF.Gelu)
                xc8 = [ac.tile([128, 2, TOK], f8, tag=f"xc8_{kp}", bufs=1,
                               name=f"xc8_{kp}") for kp in range(2)]
                for c in range(4):
                    t1 = ac.tile([128, TOK], bf16, tag="xct", bufs=2,
                                 name=f"xct{c}")
                    nc.vector.tensor_mul(out=t1[:], in0=tok[c][:], in1=bR2[:])
                    nc.vector.tensor_sub(out=xc8[c // 2][:, c % 2, :],
                                         in0=t1[:], in1=bM2[:])

                # --- FF1 / GEGLU / FF2 ---
                gt8 = [ac.tile([128, 2, TOK], f8, tag=f"gt{jp}", bufs=1,
                               name=f"gt{jp}") for jp in range(6)]
                nc.vector.memset(gt8[5][:, 1, :], 0.0)
                for j in range(11):
                    px = pgen.tile([128, TOK], f32, tag="g", name=f"fx{j}")
                    pg = pgen.tile([128, TOK], f32, tag="g", name=f"fg{j}")
                    for kp in range(2):
                        nc.tensor.matmul(px[:], w18[:, kp, :, 128 * j:128 * (j + 1)],
                                         xc8[kp][:], start=(kp == 0),
                                         stop=(kp == 1), perf_mode=DR)
                    for kp in range(2):
                        nc.tensor.matmul(
                            pg[:],
                            w18[:, kp, :, IFFP + 128 * j:IFFP + 128 * (j + 1)],
                            xc8[kp][:], start=(kp == 0), stop=(kp == 1),
                            perf_mode=DR)
                    gg = ac.tile([128, TOK], bf16, tag="gg", bufs=3,
                                 name=f"gg{j}")
                    nc.scalar.activation(out=gg[:], in_=pg[:], func=AF.Gelu,
                                         scale=1.0 / s1g)
                    nc.vector.tensor_mul(out=gt8[j // 2][:, j % 2, :],
                                         in0=gg[:], in1=px[:])
                # dummy exp: preload nlexp for the next layer during FF2
                dt3 = ac.tile([1, 1], f32, tag="dtab", bufs=2, name="dt3")
                nc.scalar.activation(out=dt3[:], in_=gt8[0][0:1, 0, 0:1],
                                     func=AF.Exp)
                psO_prev = []
                for c in range(4):
                    psO = pacc.tile([128, TOK], f32, tag="a", name=f"fo{c}")
                    for jp in range(6):
                        nc.tensor.matmul(psO[:], w28[:, jp, :, 128 * c:128 * (c + 1)],
                                         gt8[jp][:], start=(jp == 0),
                                         stop=(jp == 5), perf_mode=DR)
                    psO_prev.append(psO)

            # ---------- pool ----------
            pwkv8 = wp.tile([128, 2, 2, 2 * D], f8, tag="pwkv8", bufs=1,
                            name="pwkv8")
            nc.sync.dma_start(out=pwkv8[:], in_=pwkv8_t)
            pwo8 = wp.tile([128, 2, 2, D], f8, tag="pwo8", bufs=1,
                           name="pwo8")
            nc.sync.dma_start(out=pwo8[:], in_=pwo8_t)
            prow = wp.tile([1, 2 * D], bf16, tag="prow", bufs=1, name="prow")
            nc.sync.dma_start(out=prow[:], in_=prow_t)
            pq2s = load_cols(pq2_t, 4, "pq2")

            # final LN (rank-1 folded) over tok + last FF residual
            tf8 = [ac.tile([128, 2, TOK], f8, tag=f"tr8{kp}", bufs=1,
                           name=f"tf8_{kp}") for kp in range(2)]
            for c in range(4):
                t = ac.tile([128, TOK], bf16, tag="psot", bufs=2,
                            name=f"fpsot{c}")
                nc.vector.tensor_scalar_mul(out=t[:], in0=psO_prev[c][:],
                                            scalar1=cinv[:])
                nc.vector.tensor_add(out=tok[c][:], in0=tok[c][:], in1=t[:])
                nc.vector.tensor_copy(out=tf8[c // 2][:, c % 2, :],
                                      in_=tok[c][:])
            Sf = pgen.tile([1, TOK], f32, tag="g", name="Sf")
            for i in range(4):
                nc.tensor.matmul(Sf[:], ones8p[:], tf8[i // 2][:, i % 2, :],
                                 start=(i == 0), stop=(i == 3))
            Sfb = ac.tile([1, TOK], bf16, tag="rowb", bufs=4, name="Sfb")
            nc.scalar.activation(out=Sfb[:], in_=Sf[:], func=AF.Copy)
            xsqf = []
            for kp in range(2):
                t = ac.tile([128, 2, TOK], bf16, tag="xsq", bufs=4,
                            name=f"xsqf{kp}")
                nc.scalar.activation(out=t[:], in_=tf8[kp][:], func=AF.Square)
                xsqf.append(t)
            Qf_ = pgen.tile([1, TOK], f32, tag="g", name="Qf_")
            for i in range(4):
                nc.tensor.matmul(Qf_[:], oi512[:], xsqf[i // 2][:, i % 2, :],
                                 start=(i == 0), stop=(i == 3))
            m2f = ac.tile([1, TOK], f32, tag="rowf", bufs=6, name="m2f")
            nc.scalar.activation(out=m2f[:], in_=Sf[:], func=AF.Square,
                                 scale=1.0 / 64)
            varf = ac.tile([1, TOK], f32, tag="rowf", bufs=6, name="varf")
            nc.vector.tensor_sub(out=varf[:], in0=Qf_[:], in1=m2f[:])
            rstdf = ac.tile([1, TOK], f32, tag="rowf", bufs=6, name="rstdf")
            nc.scalar.activation(out=rstdf[:], in_=varf[:], func=AF.Ln,
                                 bias=epsc[0:1, :])
            nc.scalar.activation(out=rstdf[:], in_=rstdf[:], func=AF.Exp,
                                 scale=-0.5)
            rstdfb = ac.tile([1, TOK], bf16, tag="rowb", bufs=4,
                             name="rstdfb")
            nc.vector.tensor_copy(out=rstdfb[:], in_=rstdf[:])
            bRf_p = pgen.tile([128, TOK], f32, tag="g", name="bRf")
            nc.tensor.matmul(bRf_p[:], ones1[:], rstdfb[:], start=True,
                             stop=True)
            bRf = ac.tile([128, TOK], bf16, tag="bR", bufs=2, name="bRfb")
            nc.vector.tensor_copy(out=bRf[:], in_=bRf_p[:])
            rTf_p = pacc.tile([128, 4], f32, tag="a", name="rTfp")
            for c, (a_, b_) in enumerate(tok_chunks):
                nc.tensor.matmul(rTf_p[0:b_ - a_, c:c + 1],
                                 rstdfb[:, a_:b_],
                                 ones1[0:1, 0:1], start=True, stop=True)
            rTf = ac.tile([128, 4], f32, tag="rT", bufs=2, name="rTf")
            nc.scalar.activation(out=rTf[:], in_=rTf_p[:], func=AF.Copy,
                                 scale=1.0 / spl)

            # V_pool + vsums -> pool exchange
            Vp = []
            for i, (a, b) in enumerate(tok_chunks):
                m = b - a
                ps = pacc.tile([128, D], f32, tag="a", name=f"pv{i}")
                for kp in range(2):
                    nc.tensor.matmul(ps[0:m, :], tf8[kp][:, :, a:b],
                                     pwkv8[:, kp, :, D:2 * D],
                                     start=(kp == 0), stop=False,
                                     perf_mode=DR)
                nc.tensor.matmul(ps[0:m, :], Sfb[:, a:b], prow[:, D:2 * D],
                                 start=False, stop=True)
                s = ac.tile([128, D], bf16, tag=f"V{i}", bufs=1,
                            name=f"pVb{i}")
                nc.vector.tensor_scalar_mul(out=s[0:m, :], in0=ps[0:m, :],
                                            scalar1=rTf[0:m, i:i + 1])
                Vp.append(s)
            PpA = ac.tile([128, 4], f32, tag="PA", bufs=2, name="PpA")
            for c in range(4):
                ps = pgen.tile([128, 1], f32, tag="g", name=f"pvs{c}")
                for j in range(3):
                    nc.tensor.matmul(ps[:], Vp[j][:, 128 * c:128 * (c + 1)],
                                     ones128[:], start=(j == 0), stop=(j == 2))
                nc.vector.tensor_copy(out=PpA[:, c:c + 1], in_=ps[:])
            pvfu = ac.tile([128, 4], f32, tag="vfu", bufs=2, name="pvfu")
            for c in range(4):
                ps = pgen.tile([128, 1], f32, tag="g", name=f"pvf{c}")
                nc.tensor.matmul(ps[:], Vp[3][0:FUS, 128 * c:128 * (c + 1)],
                                 ones128[0:FUS, :], start=True, stop=True)
                nc.vector.tensor_copy(out=pvfu[:, c:c + 1], in_=ps[:])
            pinP = dramp.tile([128, 4], f32, tag="pinA", bufs=2, name="pinP")
            nc.sync.dma_start(out=pinP[:], in_=PpA[:])
            RP = ac.tile([128, 4, 4], f32, tag="RA", bufs=2, name="RP")
            if use_cc:
                poutP = dramp.tile([4 * 128, 4], f32, tag="poutA", bufs=2,
                                   name="poutP")
                nc.gpsimd.collective_compute(
                    "AllGather", OP.bypass, replica_groups=rg,
                    ins=[pinP.opt()], outs=[poutP.opt()])
                nc.sync.dma_start(
                    out=RP[:], in_=poutP.rearrange("(r p) f -> p r f", r=4))
            else:
                nc.sync.dma_start(
                    out=RP[:],
                    in_=pinP.rearrange("(r p) f -> p r f", r=1)
                    .to_broadcast((128, 4, 4)))

            # uniform pooled vector u
            pT2 = ac.tile([128, 2, 4], f32, tag="cmb", bufs=3, name="pT2")
            nc.vector.tensor_add(out=pT2[:], in0=RP[:, 0:2, :],
                                 in1=RP[:, 2:4, :])
            pvs = ac.tile([128, 4], f32, tag="vsb", bufs=2, name="pvs")
            nc.vector.tensor_add(out=pvs[:], in0=pT2[:, 0, :],
                                 in1=pT2[:, 1, :])
            nc.vector.tensor_add(out=pvs[:], in0=pvs[:], in1=pvfu[:])
            pvsb8 = ac.tile([128, 4], f8, tag="vsb8", bufs=2,
                            name="pvsb8")
            nc.vector.tensor_copy(out=pvsb8[:], in_=pvs[:])
            pup = pgen.tile([128, 4], f32, tag="g", name="pup")
            for c in range(4):
                for kc in range(4):
                    nc.tensor.matmul(pup[:, c:c + 1],
                                     pwo8[:, kc // 2, kc % 2,
                                          128 * c:128 * (c + 1)],
                                     pvsb8[:, kc:kc + 1],
                                     start=(kc == 0), stop=(kc == 3))
            pus = ac.tile([128, 4], f32, tag="dus", bufs=2, name="pus")
            nc.scalar.activation(out=pus[:], in_=pup[:], func=AF.Copy,
                                 scale=1.0 / (spo * NALL))
            nc.sync.dma_start(out=out_u.rearrange("(c p) one -> p (c one)", c=4),
                              in_=pus[:])

            # fusion-key attention for return token 2
            kf = []
            for mc in range(4):
                ps = pgen.tile([128, FUS], f32, tag="g", name=f"pkf{mc}")
                for kp in range(2):
                    nc.tensor.matmul(ps[:],
                                     pwkv8[:, kp, :, 128 * mc:128 * (mc + 1)],
                                     tf8[kp][:, :, OWN:TOK],
                                     start=(kp == 0), stop=False,
                                     perf_mode=DR)
                nc.tensor.matmul(ps[:], prow[:, 128 * mc:128 * (mc + 1)],
                                 Sfb[:, OWN:TOK], start=False, stop=True)
                s = ac.tile([128, FUS], bf16, tag=f"kf{mc}", bufs=1,
                            name=f"kfb{mc}")
                nc.vector.tensor_mul(out=s[:], in0=ps[:],
                                     in1=bRf[:, OWN:TOK])
                kf.append(s)
            q2 = []
            for mc in range(4):
                s = ac.tile([128, 32], bf16, tag=f"qf{mc}", bufs=1,
                            name=f"q2b{mc}")
                nc.vector.memset(s[:, 1:32], 0.0)
                nc.vector.tensor_copy(out=s[:, 0:1], in_=pq2s[mc][:])
                q2.append(s)
            e2, l2 = [], []
            for t in range(2):
                sp = pgen.tile([128, FUS], f32, tag="g", name=f"ps2{t}")
                for i in range(4):
                    h = 4 * t + i
                    ch, base = h // 2, (h % 2) * 64
                    nc.tensor.matmul(sp[32 * i:32 * i + 32, :],
                                     q2[ch][base:base + 64, 0:32],
                                     kf[ch][base:base + 64, :],
                                     start=True, stop=True,
                                     tile_position=(base, 32 * i))
                e = ac.tile([128, FUS], bf16, tag=f"e2{t}", bufs=1,
                            name=f"e2{t}")
                la = ac.tile([128, 1], f32, tag=f"la{t}", bufs=2,
                             name=f"pla{t}")
                nc.scalar.activation(out=e[:], in_=sp[:], func=AF.Exp,
                                     scale=1.0 / spl, accum_out=la[:])
                e2.append(e)
                l2.append(la)
            e2T = []
            for t in range(2):
                pt = pgen.tile([FUS, 128], bf16, tag="g", name=f"pet{t}")
                nc.tensor.transpose(pt[:], e2[t][:], ident[:])
                s = ac.tile([FUS, 128], bf16, tag=f"e2T{t}", bufs=1,
                            name=f"e2Tb{t}")
                nc.vector.tensor_copy(out=s[:], in_=pt[:])
                e2T.append(s)
            of2 = []
            for t in range(2):
                acc = pacc.tile([128, 64], f32, tag="a", name=f"pacc2{t}")
                for i in range(4):
                    h = 4 * t + i
                    nc.tensor.matmul(acc[32 * i:32 * i + 32, :],
                                     e2T[t][:, 32 * i:32 * i + 32],
                                     Vp[3][0:FUS, DH * h:DH * (h + 1)],
                                     start=True, stop=True,
                                     tile_position=(0, 32 * i))
                li = ac.tile([128, 1], f32, tag="linv", bufs=2,
                             name=f"pli{t}")
                nc.vector.reciprocal(out=li[:], in_=l2[t][:])
                s = ac.tile([128, 64], bf16, tag=f"of{t}", bufs=1,
                            name=f"pof{t}")
                nc.vector.tensor_scalar_mul(out=s[:], in0=acc[:],
                                            scalar1=li[:])
                of2.append(s)
            ofI2 = [ac.tile([128, 1], f8, tag=f"ofI2{kc}", bufs=1,
                            name=f"ofI2{kc}") for kc in range(4)]
            for kc in range(4):
                pt = pacc.tile([128, 32], bf16, tag="a", name=f"ofI2p{kc}")
                for half in range(2):
                    h = 2 * kc + half
                    t, i = h // 4, h % 4
                    nc.tensor.transpose(
                        pt[64 * half:64 * half + 64, :],
                        of2[t][32 * i:32 * i + 32, :],
                        ident[32 * i:32 * i + 32, 32 * i:32 * i + 32],
                        tile_position=(32 * i, 64 * half))
                nc.vector.tensor_copy(out=ofI2[kc][:], in_=pt[:, 0:1])
            P2 = pacc.tile([1, D], f32, tag="a", name="P2")
            for kc in range(4):
                nc.tensor.matmul(P2[:], ofI2[kc][:],
                                 pwo8[:, kc // 2, kc % 2, :],
                                 start=(kc == 0), stop=(kc == 3))
            p2s = ac.tile([1, D], f32, tag="p2s", bufs=1, name="p2s")
            nc.scalar.activation(out=p2s[:], in_=P2[:], func=AF.Copy,
                                 scale=1.0 / spo)
            nc.sync.dma_start(out=out_f, in_=p2s[:])

    nc.compile()
    _built[key] = nc
    return nc


def _prep_inputs(inputs):
    """Host-side prep: fold gains, pick fp8 scales, pack weights per layer."""
    I = {k: np.asarray(v, np.float64) for k, v in inputs.items()}
    f32 = np.float32

    def col(x):
        return np.ascontiguousarray(np.asarray(x, f32).reshape(-1, 1))

    scale_dh = DH ** -0.5
    g = I["layers_attn_g"][:, :, None]
    wqf = I["layers_wq"] * scale_dh * g            # [L, 512, 512]
    wkvf = I["layers_wkv"] * g                     # [L, 512, 1024]
    wof = I["layers_wo"]                           # [L, 512, 512]
    w1f = I["layers_ff_w1"] * I["layers_ff_g"][:, :, None]  # [L, 512, 2730]
    w2f = I["layers_ff_w2"]                        # [L, 1365, 512]

    sq = _pow2_scale(wqf)
    skv = _pow2_scale(wkvf)
    so = _pow2_scale(wof)
    s1x = 16.0
    s1g = _pow2_scale(w1f[:, :, IFF:])
    s2 = _pow2_scale(w2f)
    pkvf = I["pool_wkv"] * I["final_g"][:, None]
    spl = _pow2_scale(pkvf)
    spo = _pow2_scale(I["pool_wo"])
    sef = {}
    for mod in ("rna", "atac"):
        ewf = I[f"{mod}_w"] * I[f"{mod}_ln1_g"][:, None]
        sef[mod] = (ewf, _pow2_scale(ewf))
    se = min(sef["rna"][1], sef["atac"][1])
    scales = (sq, skv, so, s1x, s1g, s2, se, spl, spo)

    # packed per-layer fp8 weights
    wpk = np.zeros((DEPTH, 128, FCOLS), F8)
    rows = np.zeros((DEPTH, 3, D), np.float64)
    for l in range(DEPTH):
        wpk[l, :, SEG_WQ:SEG_WKV] = _pack_pairs(wqf[l], sq).reshape(128, -1)
        wpk[l, :, SEG_WKV:SEG_WO] = _pack_pairs(wkvf[l], skv).reshape(128, -1)
        wpk[l, :, SEG_WO:SEG_W1] = _pack_pairs(wof[l], so).reshape(128, -1)
        w1p = np.zeros((D, 2 * IFFP), np.float64)
        w1p[:, 0:IFF] = w1f[l][:, 0:IFF] * s1x
        w1p[:, IFFP:IFFP + IFF] = w1f[l][:, IFF:] * s1g
        wpk[l, :, SEG_W1:SEG_W2] = _pack_pairs(w1p, 1.0).reshape(128, -1)
        w2p = np.zeros((IFF2, D), np.float64)
        w2p[0:IFF, :] = w2f[l] * s2
        wpk[l, :, SEG_W2:FCOLS] = _pack_pairs(w2p, 1.0).reshape(128, -1)
        # rank-1 rows: -scale*colsum/64
        rows[l, 0] = -skv * wkvf[l][:, 0:D].sum(0) / 64     # wksum
        rows[l, 1] = -skv * wkvf[l][:, D:2 * D].sum(0) / 64  # wvsum
        rows[l, 2] = -sq * wqf[l].sum(0) / 64                # wqsum
    rows_b = np.ascontiguousarray(rows.reshape(1, -1)).astype(BF)

    prow = np.zeros((1, 2 * D), np.float64)
    prow[0, 0:D] = -spl * pkvf[:, 0:D].sum(0) / 64
    prow[0, D:2 * D] = -spl * pkvf[:, D:2 * D].sum(0) / 64

    pwkv8 = _pack_pairs(pkvf, spl)
    pwo8 = _pack_pairs(I["pool_wo"], spo)

    # pool query for return token 2 (host, tiny)
    ret = I["return_tokens"].astype(f32)
    gp = I["pool_g"].astype(f32)
    mu = ret.mean(-1, keepdims=True)
    var = ((ret - mu) ** 2).mean(-1, keepdims=True)
    retn = (ret - mu) / np.sqrt(var + 1e-5) * gp
    q2 = (retn[2] @ I["pool_wq"].astype(f32)) * scale_dh

    fus8 = I["fusion_tokens"].T.reshape(4, 128, FUS).transpose(1, 0, 2)

    shared = {
        "wpk": wpk,
        "rows": rows_b,
        "prow": prow.astype(BF),
        "pwkv8": pwkv8,
        "pwo8": pwo8,
        "pool_q2": col(q2),
        "fus_t": np.ascontiguousarray(fus8).astype(np.float32),
    }

    in_maps = []
    for c in range(N_CORES):
        b, q = c // 4, c % 4
        mod = "rna" if q < 2 else "atac"
        x = I[mod][b, (q % 2) * OWN:(q % 2 + 1) * OWN, :]   # [384, 1024]
        m = dict(shared)
        # x8: [128, 4kp, 2sub, 384]
        xT = np.ascontiguousarray(x.T)                      # [1024, 384]
        m["x8"] = _pack_pairs(xT, 1.0)
        ewf = sef[mod][0]
        m["ew8"] = _pack_pairs(ewf, se)
        m["emb_b"] = col(I[f"{mod}_b"]
                         + I[f"{mod}_ln1_b"] @ I[f"{mod}_w"])
        m["erow"] = (-se * ewf.sum(0) / 128).reshape(1, -1).astype(BF)
        m["eln2_g"] = col(I[f"{mod}_ln2_g"])
        m["eln2_b"] = col(I[f"{mod}_ln2_b"])
        in_maps.append(m)
    return in_maps, ret.astype(f32), scales


def kernel(**inputs):
    from concourse import bass_utils
    in_maps, ret, scales = _prep_inputs(inputs)
    nc = build(num_devices=N_CORES, use_cc=True, scales=scales)
    res = bass_utils.run_bass_kernel_spmd(nc, in_maps,
                                          core_ids=list(range(N_CORES)))
    out = np.zeros((B, 3, D), np.float32)
    for b in range(2):
        r = res.results[4 * b]
        u = r["out_u"][:, 0]
        f = r["out_f"][0]
        out[b, 0] = u + ret[0]
        out[b, 1] = u + ret[1]
        out[b, 2] = f + ret[2]
    return out


# revision 14
# speedup vs baseline: 1.5323x; 1.2308x over previous
"""BioZorro sparse-attention kernel for 8 Trainium2 NeuronCores.

Sharding: 8 cores = 2 batches x 4 token-quarters (384 own tokens each).
The zorro mask makes all non-fusion query rows fully masked -> uniform
softmax -> their attention output is mean(V); only the 16 fusion tokens
attend (over the 1536 non-fusion keys). Cross-core data per layer is two
small AllGathers: (A) V column sums (2KB) issued early, (B) fusion
flash-softmax partials (66KB), plus one tiny AllGather for pooling.

Compute layout: residual stream feature-major (tok^T [512, 400] f32).
All heavy matmuls run in fp8e4 DoubleRow (contract 256/instr, 2x rate):
activations are cast to paired [128,2,T] tiles; weights are host-packed
into one fp8 buffer per layer (single DMA, double-buffered). LayerNorms
are folded into consumers: raw-cast -> matmul immediately; the -mu
correction enters PSUM as a rank-1 matmul (host-precomputed column sums
x the device S row); rstd is applied at PSUM eviction (column-broadcast
or per-token scalars). Per-tensor power-of-2 fp8 scales are descaled via
free immediate-scale slots (exp/gelu/copy activations).
"""
import sys
sys.path.insert(0, "/opt/trn_rl_repo")
import numpy as np
import ml_dtypes

BF = ml_dtypes.bfloat16
F8 = ml_dtypes.float8_e4m3
OWN, FUS, TOK = 384, 16, 400
D, RIN, H, DH, IFF, DEPTH = 512, 1024, 8, 64, 1365, 4
NALL = 1552
B, NR, NA = 2, 768, 768
N_CORES = 8
IFFP = 1408           # x/gate block padding (11 x 128)
IFF2 = 1536           # FF2 contract padding (6 x 256)

# fp8 packed-weight segment offsets (cols in the per-layer [128, FCOLS])
SEG_WQ = 0            # [2kp][2sub][512]
SEG_WKV = 2048        # [2kp][2sub][1024]
SEG_WO = SEG_WKV + 4096   # [2kp][2sub][512]
SEG_W1 = SEG_WO + 2048    # [2kp][2sub][2*1408]
SEG_W2 = SEG_W1 + 11264   # [6jp][2sub][512]
FCOLS = SEG_W2 + 6144

_built = {}


def _pow2_scale(w, target=120.0):
    m = float(np.abs(w).max())
    if m <= 0:
        return 1.0
    return float(2.0 ** np.floor(np.log2(target / m)))


def _pack_pairs(w, scale):
    """[K, N] f64 -> [128, K//256, 2, N] fp8 DoubleRow lhsT layout."""
    K, N = w.shape
    assert K % 256 == 0
    out = (w * scale).astype(F8).reshape(K // 128, 128, N)
    # chunk k = rows 128k..128k+128; pair kp = (2kp, 2kp+1)
    out = out.transpose(1, 0, 2).reshape(128, K // 256, 2, N)
    return np.ascontiguousarray(out)


def build(num_devices=8, use_cc=True, scales=None):
    key = (num_devices, use_cc, scales)
    if key in _built:
        return _built[key]
    import concourse.tile as tile
    from concourse import bacc, mybir
    from concourse.masks import make_identity

    # Force Exp to resolve to natural_log_exp_and_others so Ln/Exp/Square
    # live in one ACT table set (Gelu still needs its own set; those two
    # swaps per layer are prefetched off the critical path with dummy ops).
    if not getattr(bacc, "_act_tables_patched", False):
        _orig_gat = bacc.get_activation_tables

        def _patched_gat(arch):
            tabs = _orig_gat(arch)
            exp_t = mybir.ActivationFunctionType.Exp
            for nm, fns in tabs.items():
                if nm != "natural_log_exp_and_others":
                    fns.discard(exp_t)
            return tabs

        bacc.get_activation_tables = _patched_gat
        bacc._act_tables_patched = True

    sq, skv, so, s1x, s1g, s2, se, spl, spo = scales
    f32 = mybir.dt.float32
    bf16 = mybir.dt.bfloat16
    f8 = mybir.dt.float8e4
    AF = mybir.ActivationFunctionType
    OP = mybir.AluOpType
    DR = mybir.MatmulPerfMode.DoubleRow

    nc = bacc.Bacc("TRN2", target_bir_lowering=False, debug=False,
                   enable_asserts=True, num_devices=num_devices)

    def din(name, shape, dt=f32):
        return nc.dram_tensor(name, shape, dt, kind="ExternalInput").ap()

    x8_t = din("x8", [128, 4, 2, OWN], f8)
    ew8_t = din("ew8", [128, 4, 2, D], f8)
    ebias_t = din("emb_b", [D, 1])
    eg2_t = din("eln2_g", [D, 1])
    eb2_t = din("eln2_b", [D, 1])
    fus_t = din("fus_t", [128, 4, FUS], f32)
    wpk_t = din("wpk", [DEPTH, 128, FCOLS], f8)
    # host rank-1 rows: per layer [wksum, wvsum, wqsum] each [512]
    rows_t = din("rows", [1, DEPTH * 3 * D], bf16)
    erow_t = din("erow", [1, D], bf16)        # embed -se*colsum(ew')/128
    prow_t = din("prow", [1, 2 * D], bf16)    # pool [pwksum, pwvsum]
    pwkv8_t = din("pwkv8", [128, 2, 2, 2 * D], f8)
    pwo8_t = din("pwo8", [128, 2, 2, D], f8)
    pq2_t = din("pool_q2", [D, 1])
    out_u = nc.dram_tensor("out_u", [D, 1], f32, kind="ExternalOutput").ap()
    out_f = nc.dram_tensor("out_f", [1, D], f32, kind="ExternalOutput").ap()

    with tile.TileContext(nc) as tc:
        with tc.tile_pool(name="cst", bufs=1) as cst, \
             tc.tile_pool(name="wp", bufs=2) as wp, \
             tc.tile_pool(name="ac", bufs=2) as ac, \
             tc.tile_pool(name="pgen", bufs=4, space="PSUM") as pgen, \
             tc.tile_pool(name="pacc", bufs=4, space="PSUM") as pacc, \
             tc.tile_pool(name="dramp", bufs=2, space="DRAM") as dramp:

            ident = cst.tile([128, 128], bf16, name="ident")
            make_identity(nc, ident[:])
            ones128 = cst.tile([128, 1], bf16, name="ones128")
            nc.vector.memset(ones128[:], 1.0)
            ones1 = cst.tile([1, 128], bf16, name="ones1")
            nc.vector.memset(ones1[:], 1.0)
            epsc = cst.tile([128, 1], f32, name="epsc")
            nc.vector.memset(epsc[:], 1e-5)
            oi512 = cst.tile([128, 1], bf16, name="oi512")
            nc.vector.memset(oi512[:], 1.0 / 512)
            ones8p = cst.tile([128, 1], f8, name="ones8p")
            nc.vector.memset(ones8p[:], 0.125)
            cinvf = 1.0 / (s1x * s2)

            def load_cols(dram_ap, n, tag, rows=128):
                ts = []
                for c in range(n):
                    t = wp.tile([rows, 1], f32, tag=f"{tag}{c}", bufs=1,
                                name=f"{tag}{c}")
                    nc.sync.dma_start(out=t[:],
                                      in_=dram_ap[rows * c:rows * (c + 1), :])
                    ts.append(t)
                return ts

            # ---------- one-time loads ----------
            x8 = ac.tile([128, 4, 2, OWN], f8, tag="x8", bufs=1, name="x8")
            nc.sync.dma_start(out=x8[:], in_=x8_t)
            ew8 = wp.tile([128, 4, 2, D], f8, tag="ew8", bufs=1, name="ew8")
            nc.sync.dma_start(out=ew8[:], in_=ew8_t)
            rows = wp.tile([1, DEPTH * 3 * D], bf16, tag="rows", bufs=1,
                           name="rows")
            nc.sync.dma_start(out=rows[:], in_=rows_t)
            erow = wp.tile([1, D], bf16, tag="erow", bufs=1, name="erow")
            nc.sync.dma_start(out=erow[:], in_=erow_t)
            ebs = load_cols(ebias_t, 4, "ebias")
            eg2s = load_cols(eg2_t, 4, "eg2")
            eb2s = load_cols(eb2_t, 4, "eb2")

            # layer-0 weights DMA starts immediately
            wts = []
            w0 = wp.tile([128, FCOLS], f8, tag="wpk", bufs=2, name="wpk0")
            nc.sync.dma_start(out=w0[:], in_=wpk_t[0])

            # dummy exp to preload the nlexp ACT table during initial DMAs
            dtab = ac.tile([1, 1], f32, tag="dtab", bufs=2, name="dtab")
            nc.scalar.activation(out=dtab[:], in_=epsc[0:1, :], func=AF.Exp)

            # ---------- embed ----------
            # stats over raw x8 (LN1 folded into the embed matmul)
            Se = pgen.tile([1, OWN], f32, tag="g", name="Se")
            for i in range(8):
                nc.tensor.matmul(Se[:], ones8p[:], x8[:, i // 2, i % 2, :],
                                 start=(i == 0), stop=(i == 7))
            Seb = ac.tile([1, OWN], bf16, tag="rowb", bufs=4, name="Seb")
            nc.scalar.activation(out=Seb[:], in_=Se[:], func=AF.Copy)
            xsqe = []
            for kp in range(4):
                t = ac.tile([128, 2, OWN], bf16, tag="xsq", bufs=4,
                            name=f"xsqe{kp}")
                nc.scalar.activation(out=t[:], in_=x8[:, kp, :, :],
                                     func=AF.Square)
                xsqe.append(t)
            Qe = pgen.tile([1, OWN], f32, tag="g", name="Qe")
            oi1024 = cst.tile([128, 1], bf16, name="oi1024")
            nc.vector.memset(oi1024[:], 1.0 / 1024)
            for i in range(8):
                nc.tensor.matmul(Qe[:], oi1024[:], xsqe[i // 2][:, i % 2, :],
                                 start=(i == 0), stop=(i == 7))
            m2e = ac.tile([1, OWN], f32, tag="rowf", bufs=6, name="m2e")
            nc.scalar.activation(out=m2e[:], in_=Se[:], func=AF.Square,
                                 scale=1.0 / 128)
            vare = ac.tile([1, OWN], f32, tag="rowf", bufs=6, name="vare")
            nc.vector.tensor_sub(out=vare[:], in0=Qe[:], in1=m2e[:])
            rstde = ac.tile([1, OWN], f32, tag="rowf", bufs=6, name="rstde")
            nc.scalar.activation(out=rstde[:], in_=vare[:], func=AF.Ln,
                                 bias=epsc[0:1, :])
            nc.scalar.activation(out=rstde[:], in_=rstde[:], func=AF.Exp,
                                 scale=-0.5)
            rstdeb = ac.tile([1, OWN], bf16, tag="rowb", bufs=4, name="rstdeb")
            nc.scalar.activation(out=rstdeb[:], in_=rstde[:], func=AF.Copy,
                                 scale=1.0 / se)
            bRe_p = pgen.tile([128, OWN], f32, tag="g", name="bRe")
            nc.tensor.matmul(bRe_p[:], ones1[:], rstdeb[:], start=True,
                             stop=True)
            bRe = ac.tile([128, OWN], bf16, tag="bR", bufs=2, name="bReb")
            nc.vector.tensor_copy(out=bRe[:], in_=bRe_p[:])

            hb = []
            for mc in range(4):
                ps = pgen.tile([128, OWN], f32, tag="g", name=f"embp{mc}")
                for kp in range(4):
                    nc.tensor.matmul(ps[:], ew8[:, kp, :, 128 * mc:128 * (mc + 1)],
                                     x8[:, kp, :, :], start=(kp == 0),
                                     stop=False, perf_mode=DR)
                nc.tensor.matmul(ps[:], erow[:, 128 * mc:128 * (mc + 1)],
                                 Seb[:], start=False, stop=True)
                t1 = ac.tile([128, OWN], bf16, tag="embt", bufs=2,
                             name=f"embt{mc}")
                nc.vector.tensor_mul(out=t1[:], in0=ps[:], in1=bRe[:])
                t2 = ac.tile([128, OWN], bf16, tag=f"hb{mc}", bufs=1,
                             name=f"hb{mc}")
                nc.vector.tensor_scalar_add(out=t2[:], in0=t1[:],
                                            scalar1=ebs[mc][:])
                hb.append(t2)

            # embed LN2 (explicit normalize into f32 tok)
            S2e = pgen.tile([1, OWN], f32, tag="g", name="S2e")
            for c in range(4):
                nc.tensor.matmul(S2e[:], oi512[:], hb[c][:],
                                 start=(c == 0), stop=(c == 3))
            x2e = []
            for c in range(4):
                t = ac.tile([128, OWN], bf16, tag="xsq", bufs=4,
                            name=f"x2e{c}")
                nc.vector.tensor_mul(out=t[:], in0=hb[c][:], in1=hb[c][:])
                x2e.append(t)
            Q2e = pgen.tile([1, OWN], f32, tag="g", name="Q2e")
            for c in range(4):
                nc.tensor.matmul(Q2e[:], oi512[:], x2e[c][:],
                                 start=(c == 0), stop=(c == 3))
            m22 = ac.tile([1, OWN], f32, tag="rowf", bufs=6, name="m22")
            nc.scalar.activation(out=m22[:], in_=S2e[:], func=AF.Square)
            var2 = ac.tile([1, OWN], f32, tag="rowf", bufs=6, name="var2e")
            nc.vector.tensor_sub(out=var2[:], in0=Q2e[:], in1=m22[:])
            rstd2e = ac.tile([1, OWN], f32, tag="rowf", bufs=6, name="rstd2e")
            nc.scalar.activation(out=rstd2e[:], in_=var2[:], func=AF.Ln,
                                 bias=epsc[0:1, :])
            nc.scalar.activation(out=rstd2e[:], in_=rstd2e[:], func=AF.Exp,
                                 scale=-0.5)
            pr2 = ac.tile([1, 2 * OWN], bf16, tag="rowb2", bufs=2, name="pr2e")
            nc.vector.tensor_copy(out=pr2[:, 0:OWN], in_=rstd2e[:])
            mre = ac.tile([1, OWN], f32, tag="rowf", bufs=6, name="mre")
            nc.vector.tensor_mul(out=mre[:], in0=S2e[:], in1=rstd2e[:])
            nc.vector.tensor_copy(out=pr2[:, OWN:2 * OWN], in_=mre[:])
            bR2e_p = pgen.tile([128, OWN], f32, tag="g", name="bR2e")
            nc.tensor.matmul(bR2e_p[:], ones1[:], pr2[:, 0:OWN], start=True,
                             stop=True)
            bR2e = ac.tile([128, OWN], bf16, tag="bR", bufs=2, name="bR2eb")
            nc.vector.tensor_copy(out=bR2e[:], in_=bR2e_p[:])
            bM2e_p = pgen.tile([128, OWN], f32, tag="g", name="bM2e")
            nc.tensor.matmul(bM2e_p[:], ones1[:], pr2[:, OWN:2 * OWN],
                             start=True, stop=True)
            bM2e = ac.tile([128, OWN], bf16, tag="bM", bufs=2, name="bM2eb")
            nc.vector.tensor_copy(out=bM2e[:], in_=bM2e_p[:])

            tok = [ac.tile([128, TOK], f32, tag=f"tok{c}", bufs=1,
                           name=f"tok{c}") for c in range(4)]
            for c in range(4):
                t1 = ac.tile([128, OWN], bf16, tag="embt", bufs=2,
                             name=f"eln{c}")
                nc.vector.tensor_mul(out=t1[:], in0=hb[c][:], in1=bR2e[:])
                nc.vector.tensor_sub(out=t1[:], in0=t1[:], in1=bM2e[:])
                nc.vector.tensor_scalar(out=tok[c][:, 0:OWN], in0=t1[:],
                                        scalar1=eg2s[c][:], scalar2=eb2s[c][:],
                                        op0=OP.mult, op1=OP.add)
                nc.sync.dma_start(out=tok[c][:, OWN:TOK], in_=fus_t[:, c, :])

            tok_chunks = [(0, 128), (128, 256), (256, 384), (384, 400)]
            rg = [[0, 1, 2, 3], [4, 5, 6, 7]]
            psO_prev = None

            # rank-1 row views per layer
            def lrow(l, which):
                base = (l * 3 + which) * D
                return rows[:, base:base + D]

            # ---------- layers ----------
            for l in range(DEPTH):
                wT = w0 if l == 0 else wp.tile([128, FCOLS], f8, tag="wpk",
                                               bufs=2, name=f"wpk{l}")
                if l > 0:
                    nc.sync.dma_start(out=wT[:], in_=wpk_t[l])
                wq8 = wT[:, SEG_WQ:SEG_WKV].rearrange(
                    "p (kp s o) -> p kp s o", kp=2, s=2)
                wkv8 = wT[:, SEG_WKV:SEG_WO].rearrange(
                    "p (kp s o) -> p kp s o", kp=2, s=2)
                wo8 = wT[:, SEG_WO:SEG_W1].rearrange(
                    "p (kp s o) -> p kp s o", kp=2, s=2)
                w18 = wT[:, SEG_W1:SEG_W2].rearrange(
                    "p (kp s o) -> p kp s o", kp=2, s=2)
                w28 = wT[:, SEG_W2:FCOLS].rearrange(
                    "p (jp s o) -> p jp s o", jp=6, s=2)

                # --- LN1 prep: residual + fp8 raw cast ---
                tr8 = [ac.tile([128, 2, TOK], f8, tag=f"tr8{kp}", bufs=1,
                               name=f"tr8_{kp}") for kp in range(2)]
                for c in range(4):
                    if psO_prev is not None:
                        t = ac.tile([128, TOK], bf16, tag="psot", bufs=2,
                                    name=f"psot{c}")
                        nc.scalar.activation(out=t[:], in_=psO_prev[c][:],
                                             func=AF.Copy, scale=cinvf)
                        nc.vector.tensor_add(out=tok[c][:], in0=tok[c][:],
                                             in1=t[:])
                    nc.vector.tensor_copy(out=tr8[c // 2][:, c % 2, :],
                                          in_=tok[c][:])
                psO_prev = None

                # --- stats1 (rank-1 style folded LN) ---
                S1 = pgen.tile([1, TOK], f32, tag="g", name="S1")
                for i in range(4):
                    nc.tensor.matmul(S1[:], ones8p[:], tr8[i // 2][:, i % 2, :],
                                     start=(i == 0), stop=(i == 3))
                S1b = ac.tile([1, TOK], bf16, tag="rowb", bufs=4, name="S1b")
                nc.scalar.activation(out=S1b[:], in_=S1[:], func=AF.Copy)
                xsq = []
                for kp in range(2):
                    t = ac.tile([128, 2, TOK], bf16, tag="xsq", bufs=4,
                                name=f"xsq{kp}")
                    if kp == 0:
                        nc.vector.tensor_mul(out=t[:], in0=tr8[kp][:],
                                             in1=tr8[kp][:])
                    else:
                        nc.scalar.activation(out=t[:], in_=tr8[kp][:],
                                             func=AF.Square)
                    xsq.append(t)
                Q1 = pgen.tile([1, TOK], f32, tag="g", name="Q1")
                for i in range(4):
                    nc.tensor.matmul(Q1[:], oi512[:], xsq[i // 2][:, i % 2, :],
                                     start=(i == 0), stop=(i == 3))
                m2 = ac.tile([1, TOK], f32, tag="rowf", bufs=6, name="m2")
                nc.scalar.activation(out=m2[:], in_=S1[:], func=AF.Square,
                                     scale=1.0 / 64)
                var = ac.tile([1, TOK], f32, tag="rowf", bufs=6, name="var")
                nc.vector.tensor_sub(out=var[:], in0=Q1[:], in1=m2[:])
                rstd = ac.tile([1, TOK], f32, tag="rowf", bufs=6, name="rstd")
                nc.scalar.activation(out=rstd[:], in_=var[:], func=AF.Ln,
                                     bias=epsc[0:1, :])
                nc.scalar.activation(out=rstd[:], in_=rstd[:], func=AF.Exp,
                                     scale=-0.5)
                rstdb = ac.tile([1, TOK], bf16, tag="rowb", bufs=4,
                                name="rstdb")
                nc.vector.tensor_copy(out=rstdb[:], in_=rstd[:])
                bR1_p = pgen.tile([128, TOK], f32, tag="g", name="bR1")
                nc.tensor.matmul(bR1_p[:], ones1[:], rstdb[:], start=True,
                                 stop=True)
                bR1 = ac.tile([128, TOK], bf16, tag="bR", bufs=2, name="bR1b")
                nc.vector.tensor_copy(out=bR1[:], in_=bR1_p[:])
                rT_p = pacc.tile([128, 4], f32, tag="a", name="rTp")
                for c, (a_, b_) in enumerate(tok_chunks):
                    nc.tensor.matmul(rT_p[0:b_ - a_, c:c + 1],
                                     rstdb[:, a_:b_],
                                     ones1[0:1, 0:1], start=True, stop=True)
                rT = ac.tile([128, 4], f32, tag="rT", bufs=2, name="rT")
                nc.scalar.activation(out=rT[:], in_=rT_p[:], func=AF.Copy,
                                     scale=1.0 / skv)

                # --- V (+ vsums -> exchange A) ---
                V = []
                for i, (a, b) in enumerate(tok_chunks):
                    m = b - a
                    ps = pacc.tile([128, D], f32, tag="a", name=f"v{i}")
                    for kp in range(2):
                        nc.tensor.matmul(ps[0:m, :], tr8[kp][:, :, a:b],
                                         wkv8[:, kp, :, D:2 * D],
                                         start=(kp == 0), stop=False,
                                         perf_mode=DR)
                    nc.tensor.matmul(ps[0:m, :], S1b[:, a:b],
                                     lrow(l, 1), start=False, stop=True)
                    s = ac.tile([128, D], bf16, tag=f"V{i}", bufs=1,
                                name=f"Vb{i}")
                    nc.scalar.activation(out=s[0:m, :], in_=ps[0:m, :],
                                         func=AF.Copy,
                                         scale=rT[0:m, i:i + 1])
                    V.append(s)
                PA = ac.tile([128, 4], f32, tag="PA", bufs=2, name="PA")
                for c in range(4):
                    ps = pgen.tile([128, 1], f32, tag="g", name=f"vs{c}")
                    for j in range(3):
                        nc.tensor.matmul(ps[:], V[j][:, 128 * c:128 * (c + 1)],
                                         ones128[:], start=(j == 0),
                                         stop=(j == 2))
                    nc.vector.tensor_copy(out=PA[:, c:c + 1], in_=ps[:])
                vfu = ac.tile([128, 4], f32, tag="vfu", bufs=2, name="vfu")
                for c in range(4):
                    ps = pgen.tile([128, 1], f32, tag="g", name=f"vf{c}")
                    nc.tensor.matmul(ps[:], V[3][0:FUS, 128 * c:128 * (c + 1)],
                                     ones128[0:FUS, :], start=True, stop=True)
                    nc.vector.tensor_copy(out=vfu[:, c:c + 1], in_=ps[:])
                pinA = dramp.tile([128, 4], f32, tag="pinA", bufs=2,
                                  name="pinA")
                nc.sync.dma_start(out=pinA[:], in_=PA[:])
                RA = ac.tile([128, 4, 4], f32, tag="RA", bufs=2, name="RA")
                if use_cc:
                    poutA = dramp.tile([4 * 128, 4], f32, tag="poutA", bufs=2,
                                       name="poutA")
                    nc.gpsimd.collective_compute(
                        "AllGather", OP.bypass, replica_groups=rg,
                        ins=[pinA.opt()], outs=[poutA.opt()])
                    nc.sync.dma_start(
                        out=RA[:], in_=poutA.rearrange("(r p) f -> p r f", r=4))
                else:
                    nc.sync.dma_start(
                        out=RA[:],
                        in_=pinA.rearrange("(r p) f -> p r f", r=1)
                        .to_broadcast((128, 4, 4)))

                # --- K^T, Qf ---
                kt = []
                for mc in range(4):
                    ps = pgen.tile([128, OWN], f32, tag="g", name=f"kt{mc}")
                    for kp in range(2):
                        nc.tensor.matmul(ps[:],
                                         wkv8[:, kp, :, 128 * mc:128 * (mc + 1)],
                                         tr8[kp][:, :, 0:OWN],
                                         start=(kp == 0), stop=False,
                                         perf_mode=DR)
                    nc.tensor.matmul(ps[:], lrow(l, 0)[:, 128 * mc:128 * (mc + 1)],
                                     S1b[:, 0:OWN], start=False, stop=True)
                    s = ac.tile([128, OWN], bf16, tag=f"kt{mc}", bufs=1,
                                name=f"ktb{mc}")
                    nc.vector.tensor_mul(out=s[:], in0=ps[:],
                                         in1=bR1[:, 0:OWN])
                    kt.append(s)
                qf = []
                for mc in range(4):
                    ps = pgen.tile([128, FUS], f32, tag="g", name=f"qf{mc}")
                    for kp in range(2):
                        nc.tensor.matmul(ps[:],
                                         wq8[:, kp, :, 128 * mc:128 * (mc + 1)],
                                         tr8[kp][:, :, OWN:TOK],
                                         start=(kp == 0), stop=False,
                                         perf_mode=DR)
                    nc.tensor.matmul(ps[:], lrow(l, 2)[:, 128 * mc:128 * (mc + 1)],
                                     S1b[:, OWN:TOK], start=False, stop=True)
                    s = ac.tile([128, 32], bf16, tag=f"qf{mc}", bufs=1,
                                name=f"qfb{mc}")
                    nc.vector.memset(s[:, FUS:32], 0.0)
                    nc.vector.tensor_mul(out=s[:, 0:FUS], in0=ps[:],
                                         in1=bR1[:, OWN:TOK])
                    qf.append(s)

                # --- scores + exp ---
                E, lacc = [], []
                for t in range(2):
                    sp = pgen.tile([128, OWN], f32, tag="g", name=f"sp{t}")
                    for i in range(4):
                        h = 4 * t + i
                        ch, base = h // 2, (h % 2) * 64
                        nc.tensor.matmul(sp[32 * i:32 * i + 32, :],
                                         qf[ch][base:base + 64, 0:32],
                                         kt[ch][base:base + 64, :],
                                         start=True, stop=True,
                                         tile_position=(base, 32 * i))
                    e = ac.tile([128, OWN], bf16, tag=f"e{t}", bufs=1,
                                name=f"e{t}")
                    la = ac.tile([128, 1], f32, tag=f"la{t}", bufs=2,
                                 name=f"la{t}")
                    nc.scalar.activation(out=e[:], in_=sp[:], func=AF.Exp,
                                         scale=1.0 / (sq * skv),
                                         accum_out=la[:])
                    E.append(e)
                    lacc.append(la)

                # --- E^T + ACC -> exchange B ---
                ET = [[None] * 3 for _ in range(2)]
                for t in range(2):
                    for j in range(3):
                        pt = pgen.tile([128, 128], bf16, tag="g",
                                       name=f"et{t}{j}")
                        nc.tensor.transpose(pt[:], E[t][:, 128 * j:128 * (j + 1)],
                                            ident[:])
                        s = ac.tile([128, 128], bf16, tag=f"ET{t}{j}", bufs=1,
                                    name=f"ETb{t}{j}")
                        nc.vector.tensor_copy(out=s[:], in_=pt[:])
                        ET[t][j] = s
                PB = ac.tile([128, 130], f32, tag="PB", bufs=2, name="PB")
                nc.vector.tensor_copy(out=PB[:, 0:1], in_=lacc[0][:])
                nc.vector.tensor_copy(out=PB[:, 1:2], in_=lacc[1][:])
                for t in range(2):
                    acc = pacc.tile([128, 64], f32, tag="a", name=f"acc{t}")
                    for i in range(4):
                        h = 4 * t + i
                        for j in range(3):
                            nc.tensor.matmul(acc[32 * i:32 * i + 32, :],
                                             ET[t][j][:, 32 * i:32 * i + 32],
                                             V[j][:, DH * h:DH * (h + 1)],
                                             start=(j == 0), stop=(j == 2),
                                             tile_position=(0, 32 * i))
                    nc.vector.tensor_copy(out=PB[:, 2 + 64 * t:66 + 64 * t],
                                          in_=acc[:])
                pinB = dramp.tile([128, 130], f32, tag="pinB", bufs=2,
                                  name="pinB")
                nc.sync.dma_start(out=pinB[:], in_=PB[:])
                RB = ac.tile([128, 4, 130], f32, tag="RB", bufs=2, name="RB")
                if use_cc:
                    poutB = dramp.tile([4 * 128, 130], f32, tag="poutB",
                                       bufs=2, name="poutB")
                    nc.gpsimd.collective_compute(
                        "AllGather", OP.bypass, replica_groups=rg,
                        ins=[pinB.opt()], outs=[poutB.opt()])
                    nc.sync.dma_start(
                        out=RB[:], in_=poutB.rearrange("(r p) f -> p r f", r=4))
                else:
                    nc.sync.dma_start(
                        out=RB[:],
                        in_=pinB.rearrange("(r p) f -> p r f", r=1)
                        .to_broadcast((128, 4, 130)))

                # --- A-combine -> uniform delta -> own-col LN2 prep ---
                vsb = ac.tile([128, 4], f32, tag="vsb", bufs=2, name="vsb")
                nc.vector.tensor_reduce(out=vsb[:],
                                        in_=RA[:].rearrange("p r f -> p f r"),
                                        axis=mybir.AxisListType.X,
                                        op=OP.add)
                nc.vector.tensor_add(out=vsb[:], in0=vsb[:], in1=vfu[:])
                vsb8 = ac.tile([128, 4], f8, tag="vsb8", bufs=2,
                               name="vsb8")
                nc.vector.tensor_copy(out=vsb8[:], in_=vsb[:])
                dup = pgen.tile([128, 4], f32, tag="g", name="dup")
                for c in range(4):
                    for kc in range(4):
                        nc.tensor.matmul(dup[:, c:c + 1],
                                         wo8[:, kc // 2, kc % 2,
                                             128 * c:128 * (c + 1)],
                                         vsb8[:, kc:kc + 1],
                                         start=(kc == 0), stop=(kc == 3))
                dus = ac.tile([128, 4], f32, tag="dus", bufs=2, name="dus")
                nc.scalar.activation(out=dus[:], in_=dup[:], func=AF.Copy,
                                     scale=1.0 / (so * NALL))
                t28 = [ac.tile([128, 2, TOK], f8, tag=f"t28_{kp}", bufs=1,
                               name=f"t28_{kp}") for kp in range(2)]
                for c in range(4):
                    nc.vector.tensor_scalar_add(out=tok[c][:, 0:OWN],
                                                in0=tok[c][:, 0:OWN],
                                                scalar1=dus[:, c:c + 1])
                    nc.vector.tensor_copy(out=t28[c // 2][:, c % 2, 0:OWN],
                                          in_=tok[c][:, 0:OWN])
                S2 = pgen.tile([1, TOK], f32, tag="g", name="S2")
                for i in range(4):
                    nc.tensor.matmul(S2[:, 0:OWN], ones8p[:],
                                     t28[i // 2][:, i % 2, 0:OWN],
                                     start=(i == 0), stop=(i == 3))
                mur2b = ac.tile([1, TOK], bf16, tag="rowb", bufs=4,
                                name="mur2b")
                nc.scalar.activation(out=mur2b[:, 0:OWN], in_=S2[:, 0:OWN],
                                     func=AF.Copy, scale=1.0 / 64)
                bMu_p = pgen.tile([128, TOK], f32, tag="g", name="bMu")
                nc.tensor.matmul(bMu_p[:, 0:OWN], ones1[:], mur2b[:, 0:OWN],
                                 start=True, stop=True)
                bMu = ac.tile([128, TOK], bf16, tag="bM", bufs=2, name="bMub")
                nc.vector.tensor_copy(out=bMu[:, 0:OWN], in_=bMu_p[:, 0:OWN])
                for c in range(4):
                    nc.vector.tensor_sub(out=tok[c][:, 0:OWN],
                                         in0=tok[c][:, 0:OWN],
                                         in1=bMu[:, 0:OWN])
                xsq2 = []
                for kp in range(2):
                    t = ac.tile([128, 2, TOK], bf16, tag="xsq", bufs=4,
                                name=f"xsq2{kp}")
                    nc.scalar.activation(out=t[:, :, 0:OWN],
                                         in_=t28[kp][:, :, 0:OWN],
                                         func=AF.Square)
                    xsq2.append(t)
                Q2 = pgen.tile([1, TOK], f32, tag="g", name="Q2")
                for i in range(4):
                    nc.tensor.matmul(Q2[:, 0:OWN], oi512[:],
                                     xsq2[i // 2][:, i % 2, 0:OWN],
                                     start=(i == 0), stop=(i == 3))

                # --- B-combine -> fusion delta -> fusion-col LN2 prep ---
                PT = ac.tile([128, 130], f32, tag="cmbB", bufs=3, name="PT")
                nc.vector.tensor_reduce(out=PT[:],
                                        in_=RB[:].rearrange("p r f -> p f r"),
                                        axis=mybir.AxisListType.X,
                                        op=OP.add)
                linv = ac.tile([128, 2], f32, tag="linv", bufs=2, name="linv")
                nc.vector.reciprocal(out=linv[:], in_=PT[:, 0:2])
                of = []
                for t in range(2):
                    s = ac.tile([128, 64], bf16, tag=f"of{t}", bufs=1,
                                name=f"of{t}")
                    nc.vector.tensor_scalar_mul(out=s[:],
                                                in0=PT[:, 2 + 64 * t:66 + 64 * t],
                                                scalar1=linv[:, t:t + 1])
                    of.append(s)
                ofI8 = [ac.tile([128, 2, FUS], f8, tag=f"ofI{kp}", bufs=1,
                                name=f"ofI{kp}") for kp in range(2)]
                for kc in range(4):
                    pt = pacc.tile([128, 32], bf16, tag="a", name=f"ofIp{kc}")
                    for half in range(2):
                        h = 2 * kc + half
                        t, i = h // 4, h % 4
                        nc.tensor.transpose(
                            pt[64 * half:64 * half + 64, :],
                            of[t][32 * i:32 * i + 32, :],
                            ident[32 * i:32 * i + 32, 32 * i:32 * i + 32],
                            tile_position=(32 * i, 64 * half))
                    nc.vector.tensor_copy(out=ofI8[kc // 2][:, kc % 2, :],
                                          in_=pt[:, 0:FUS])
                dfp = pacc.tile([FUS, D], f32, tag="a", name="dfp")
                for kp in range(2):
                    nc.tensor.matmul(dfp[:], ofI8[kp][:], wo8[:, kp, :, :],
                                     start=(kp == 0), stop=(kp == 1),
                                     perf_mode=DR)
                dfb = ac.tile([FUS, D], bf16, tag="dfb", bufs=2, name="dfb")
                nc.scalar.activation(out=dfb[:], in_=dfp[:], func=AF.Copy,
                                     scale=1.0 / so)
                for c in range(4):
                    pt = pacc.tile([128, FUS], bf16, tag="a", name=f"dft{c}")
                    nc.tensor.transpose(pt[:], dfb[0:FUS, 128 * c:128 * (c + 1)],
                                        ident[0:FUS, 0:FUS])
                    nc.vector.tensor_add(out=tok[c][:, OWN:TOK],
                                         in0=tok[c][:, OWN:TOK], in1=pt[:])
                    nc.vector.tensor_copy(out=t28[c // 2][:, c % 2, OWN:TOK],
                                          in_=tok[c][:, OWN:TOK])
                for i in range(4):
                    nc.tensor.matmul(S2[:, OWN:TOK], ones8p[:],
                                     t28[i // 2][:, i % 2, OWN:TOK],
                                     start=(i == 0), stop=(i == 3))
                for kp in range(2):
                    nc.scalar.activation(out=xsq2[kp][:, :, OWN:TOK],
                                         in_=t28[kp][:, :, OWN:TOK],
                                         func=AF.Square)
                for i in range(4):
                    nc.tensor.matmul(Q2[:, OWN:TOK], oi512[:],
                                     xsq2[i // 2][:, i % 2, OWN:TOK],
                                     start=(i == 0), stop=(i == 3))

                # --- fusion-col centering + stats2 tail + normalize-cast ---
                nc.scalar.activation(out=mur2b[:, OWN:TOK], in_=S2[:, OWN:TOK],
                                     func=AF.Copy, scale=1.0 / 64)
                nc.tensor.matmul(bMu_p[:, OWN:TOK], ones1[:],
                                 mur2b[:, OWN:TOK], start=True, stop=True)
                nc.vector.tensor_copy(out=bMu[:, OWN:TOK],
                                      in_=bMu_p[:, OWN:TOK])
                for c in range(4):
                    nc.vector.tensor_sub(out=tok[c][:, OWN:TOK],
                                         in0=tok[c][:, OWN:TOK],
                                         in1=bMu[:, OWN:TOK])
                m2_2 = ac.tile([1, TOK], f32, tag="rowf", bufs=6, name="m2_2")
                nc.scalar.activation(out=m2_2[:], in_=S2[:], func=AF.Square,
                                     scale=1.0 / 64)
                var2 = ac.tile([1, TOK], f32, tag="rowf", bufs=6, name="var2")
                nc.vector.tensor_sub(out=var2[:], in0=Q2[:], in1=m2_2[:])
                rstd2 = ac.tile([1, TOK], f32, tag="rowf", bufs=6,
                                name="rstd2")
                nc.scalar.activation(out=rstd2[:], in_=var2[:], func=AF.Ln,
                                     bias=epsc[0:1, :])
                nc.scalar.activation(out=rstd2[:], in_=rstd2[:], func=AF.Exp,
                                     scale=-0.5)
                rstd2b = ac.tile([1, TOK], bf16, tag="rowb", bufs=4,
                                 name="rstd2b")
                nc.vector.tensor_copy(out=rstd2b[:], in_=rstd2[:])
                bR2_p = pgen.tile([128, TOK], f32, tag="g", name="bR2")
                nc.tensor.matmul(bR2_p[:], ones1[:], rstd2b[:],
                                 start=True, stop=True)
                bR2 = ac.tile([128, TOK], bf16, tag="bR", bufs=2, name="bR2b")
                nc.vector.tensor_copy(out=bR2[:], in_=bR2_p[:])
                # dummy gelu: preload the gelu table while FF1 matmuls run
                dt2 = ac.tile([1, 1], f32, tag="dtab", bufs=2, name="dt2")
                nc.scalar.activation(out=dt2[:], in_=rstd2b[0:1, 0:1],
                                     func=AF.Gelu)
                xc8 = [ac.tile([128, 2, TOK], f8, tag=f"xc8_{kp}", bufs=1,
                               name=f"xc8_{kp}") for kp in range(2)]
                for c in range(4):
                    nc.vector.tensor_mul(out=xc8[c // 2][:, c % 2, :],
                                         in0=tok[c][:], in1=bR2[:])

                # --- FF1 / GEGLU / FF2 ---
                gt8 = [ac.tile([128, 2, TOK], f8, tag=f"gt{jp}", bufs=1,
                               name=f"gt{jp}") for jp in range(6)]
                nc.vector.memset(gt8[5][:, 1, :], 0.0)
                for j in range(11):
                    px = pgen.tile([128, TOK], f32, tag="g", name=f"fx{j}")
                    pg = pacc.tile([128, TOK], f32, tag="a", name=f"fg{j}")
                    for kp in range(2):
                        nc.tensor.matmul(px[:], w18[:, kp, :, 128 * j:128 * (j + 1)],
                                         xc8[kp][:], start=(kp == 0),
                                         stop=(kp == 1), perf_mode=DR)
                    for kp in range(2):
                        nc.tensor.matmul(
                            pg[:],
                            w18[:, kp, :, IFFP + 128 * j:IFFP + 128 * (j + 1)],
                            xc8[kp][:], start=(kp == 0), stop=(kp == 1),
                            perf_mode=DR)
                    gg = ac.tile([128, TOK], bf16, tag="gg", bufs=3,
                                 name=f"gg{j}")
                    nc.scalar.activation(out=gg[:], in_=pg[:], func=AF.Gelu,
                                         scale=1.0 / s1g)
                    nc.vector.tensor_mul(out=gt8[j // 2][:, j % 2, :],
                                         in0=gg[:], in1=px[:])
                # dummy exp: preload nlexp for the next layer during FF2
                dt3 = ac.tile([1, 1], f32, tag="dtab", bufs=2, name="dt3")
                nc.scalar.activation(out=dt3[:], in_=gt8[5][0:1, 0, 0:1],
                                     func=AF.Exp)
                psO_prev = []
                for c in range(4):
                    psO = pacc.tile([128, TOK], f32, tag="a", name=f"fo{c}")
                    for jp in range(6):
                        nc.tensor.matmul(psO[:], w28[:, jp, :, 128 * c:128 * (c + 1)],
                                         gt8[jp][:], start=(jp == 0),
                                         stop=(jp == 5), perf_mode=DR)
                    psO_prev.append(psO)

            # ---------- pool ----------
            pwkv8 = wp.tile([128, 2, 2, 2 * D], f8, tag="pwkv8", bufs=1,
                            name="pwkv8")
            nc.sync.dma_start(out=pwkv8[:], in_=pwkv8_t)
            pwo8 = wp.tile([128, 2, 2, D], f8, tag="pwo8", bufs=1,
                           name="pwo8")
            nc.sync.dma_start(out=pwo8[:], in_=pwo8_t)
            prow = wp.tile([1, 2 * D], bf16, tag="prow", bufs=1, name="prow")
            nc.sync.dma_start(out=prow[:], in_=prow_t)
            pq2s = load_cols(pq2_t, 4, "pq2")

            # final LN (rank-1 folded) over tok + last FF residual
            tf8 = [ac.tile([128, 2, TOK], f8, tag=f"tr8{kp}", bufs=1,
                           name=f"tf8_{kp}") for kp in range(2)]
            for c in range(4):
                t = ac.tile([128, TOK], bf16, tag="psot", bufs=2,
                            name=f"fpsot{c}")
                nc.scalar.activation(out=t[:], in_=psO_prev[c][:],
                                     func=AF.Copy, scale=cinvf)
                nc.vector.tensor_add(out=tok[c][:], in0=tok[c][:], in1=t[:])
                nc.vector.tensor_copy(out=tf8[c // 2][:, c % 2, :],
                                      in_=tok[c][:])
            Sf = pgen.tile([1, TOK], f32, tag="g", name="Sf")
            for i in range(4):
                nc.tensor.matmul(Sf[:], ones8p[:], tf8[i // 2][:, i % 2, :],
                                 start=(i == 0), stop=(i == 3))
            Sfb = ac.tile([1, TOK], bf16, tag="rowb", bufs=4, name="Sfb")
            nc.scalar.activation(out=Sfb[:], in_=Sf[:], func=AF.Copy)
            xsqf = []
            for kp in range(2):
                t = ac.tile([128, 2, TOK], bf16, tag="xsq", bufs=4,
                            name=f"xsqf{kp}")
                if kp == 0:
                    nc.vector.tensor_mul(out=t[:], in0=tf8[kp][:],
                                         in1=tf8[kp][:])
                else:
                    nc.scalar.activation(out=t[:], in_=tf8[kp][:],
                                         func=AF.Square)
                xsqf.append(t)
            Qf_ = pgen.tile([1, TOK], f32, tag="g", name="Qf_")
            for i in range(4):
                nc.tensor.matmul(Qf_[:], oi512[:], xsqf[i // 2][:, i % 2, :],
                                 start=(i == 0), stop=(i == 3))
            m2f = ac.tile([1, TOK], f32, tag="rowf", bufs=6, name="m2f")
            nc.scalar.activation(out=m2f[:], in_=Sf[:], func=AF.Square,
                                 scale=1.0 / 64)
            varf = ac.tile([1, TOK], f32, tag="rowf", bufs=6, name="varf")
            nc.vector.tensor_sub(out=varf[:], in0=Qf_[:], in1=m2f[:])
            rstdf = ac.tile([1, TOK], f32, tag="rowf", bufs=6, name="rstdf")
            nc.scalar.activation(out=rstdf[:], in_=varf[:], func=AF.Ln,
                                 bias=epsc[0:1, :])
            nc.scalar.activation(out=rstdf[:], in_=rstdf[:], func=AF.Exp,
                                 scale=-0.5)
            rstdfb = ac.tile([1, TOK], bf16, tag="rowb", bufs=4,
                             name="rstdfb")
            nc.vector.tensor_copy(out=rstdfb[:], in_=rstdf[:])
            bRf_p = pgen.tile([128, TOK], f32, tag="g", name="bRf")
            nc.tensor.matmul(bRf_p[:], ones1[:], rstdfb[:], start=True,
                             stop=True)
            bRf = ac.tile([128, TOK], bf16, tag="bR", bufs=2, name="bRfb")
            nc.vector.tensor_copy(out=bRf[:], in_=bRf_p[:])
            rTf_p = pacc.tile([128, 4], f32, tag="a", name="rTfp")
            for c, (a_, b_) in enumerate(tok_chunks):
                nc.tensor.matmul(rTf_p[0:b_ - a_, c:c + 1],
                                 rstdfb[:, a_:b_],
                                 ones1[0:1, 0:1], start=True, stop=True)
            rTf = ac.tile([128, 4], f32, tag="rT", bufs=2, name="rTf")
            nc.scalar.activation(out=rTf[:], in_=rTf_p[:], func=AF.Copy,
                                 scale=1.0 / spl)

            # V_pool + vsums -> pool exchange
            Vp = []
            for i, (a, b) in enumerate(tok_chunks):
                m = b - a
                ps = pacc.tile([128, D], f32, tag="a", name=f"pv{i}")
                for kp in range(2):
                    nc.tensor.matmul(ps[0:m, :], tf8[kp][:, :, a:b],
                                     pwkv8[:, kp, :, D:2 * D],
                                     start=(kp == 0), stop=False,
                                     perf_mode=DR)
                nc.tensor.matmul(ps[0:m, :], Sfb[:, a:b], prow[:, D:2 * D],
                                 start=False, stop=True)
                s = ac.tile([128, D], bf16, tag=f"V{i}", bufs=1,
                            name=f"pVb{i}")
                nc.scalar.activation(out=s[0:m, :], in_=ps[0:m, :],
                                     func=AF.Copy,
                                     scale=rTf[0:m, i:i + 1])
                Vp.append(s)
            PpA = ac.tile([128, 4], f32, tag="PA", bufs=2, name="PpA")
            for c in range(4):
                ps = pgen.tile([128, 1], f32, tag="g", name=f"pvs{c}")
                for j in range(3):
                    nc.tensor.matmul(ps[:], Vp[j][:, 128 * c:128 * (c + 1)],
                                     ones128[:], start=(j == 0), stop=(j == 2))
                nc.vector.tensor_copy(out=PpA[:, c:c + 1], in_=ps[:])
            pvfu = ac.tile([128, 4], f32, tag="vfu", bufs=2, name="pvfu")
            for c in range(4):
                ps = pgen.tile([128, 1], f32, tag="g", name=f"pvf{c}")
                nc.tensor.matmul(ps[:], Vp[3][0:FUS, 128 * c:128 * (c + 1)],
                                 ones128[0:FUS, :], start=True, stop=True)
                nc.vector.tensor_copy(out=pvfu[:, c:c + 1], in_=ps[:])
            pinP = dramp.tile([128, 4], f32, tag="pinA", bufs=2, name="pinP")
            nc.sync.dma_start(out=pinP[:], in_=PpA[:])
            RP = ac.tile([128, 4, 4], f32, tag="RA", bufs=2, name="RP")
            if use_cc:
                poutP = dramp.tile([4 * 128, 4], f32, tag="poutA", bufs=2,
                                   name="poutP")
                nc.gpsimd.collective_compute(
                    "AllGather", OP.bypass, replica_groups=rg,
                    ins=[pinP.opt()], outs=[poutP.opt()])
                nc.sync.dma_start(
                    out=RP[:], in_=poutP.rearrange("(r p) f -> p r f", r=4))
            else:
                nc.sync.dma_start(
                    out=RP[:],
                    in_=pinP.rearrange("(r p) f -> p r f", r=1)
                    .to_broadcast((128, 4, 4)))

            # uniform pooled vector u
            pvs = ac.tile([128, 4], f32, tag="vsb", bufs=2, name="pvs")
            nc.vector.tensor_reduce(out=pvs[:],
                                    in_=RP[:].rearrange("p r f -> p f r"),
                                    axis=mybir.AxisListType.X,
                                    op=OP.add)
            nc.vector.tensor_add(out=pvs[:], in0=pvs[:], in1=pvfu[:])
            pvsb8 = ac.tile([128, 4], f8, tag="vsb8", bufs=2,
                            name="pvsb8")
            nc.vector.tensor_copy(out=pvsb8[:], in_=pvs[:])
            pup = pgen.tile([128, 4], f32, tag="g", name="pup")
            for c in range(4):
                for kc in range(4):
                    nc.tensor.matmul(pup[:, c:c + 1],
                                     pwo8[:, kc // 2, kc % 2,
                                          128 * c:128 * (c + 1)],
                                     pvsb8[:, kc:kc + 1],
                                     start=(kc == 0), stop=(kc == 3))
            pus = ac.tile([128, 4], f32, tag="dus", bufs=2, name="pus")
            nc.scalar.activation(out=pus[:], in_=pup[:], func=AF.Copy,
                                 scale=1.0 / (spo * NALL))
            nc.sync.dma_start(out=out_u.rearrange("(c p) one -> p (c one)", c=4),
                              in_=pus[:])

            # fusion-key attention for return token 2
            kf = []
            for mc in range(4):
                ps = pgen.tile([128, FUS], f32, tag="g", name=f"pkf{mc}")
                for kp in range(2):
                    nc.tensor.matmul(ps[:],
                                     pwkv8[:, kp, :, 128 * mc:128 * (mc + 1)],
                                     tf8[kp][:, :, OWN:TOK],
                                     start=(kp == 0), stop=False,
                                     perf_mode=DR)
                nc.tensor.matmul(ps[:], prow[:, 128 * mc:128 * (mc + 1)],
                                 Sfb[:, OWN:TOK], start=False, stop=True)
                s = ac.tile([128, FUS], bf16, tag=f"kf{mc}", bufs=1,
                            name=f"kfb{mc}")
                nc.vector.tensor_mul(out=s[:], in0=ps[:],
                                     in1=bRf[:, OWN:TOK])
                kf.append(s)
            q2 = []
            for mc in range(4):
                s = ac.tile([128, 32], bf16, tag=f"qf{mc}", bufs=1,
                            name=f"q2b{mc}")
                nc.vector.memset(s[:, 1:32], 0.0)
                nc.vector.tensor_copy(out=s[:, 0:1], in_=pq2s[mc][:])
                q2.append(s)
            e2, l2 = [], []
            for t in range(2):
                sp = pgen.tile([128, FUS], f32, tag="g", name=f"ps2{t}")
                for i in range(4):
                    h = 4 * t + i
                    ch, base = h // 2, (h % 2) * 64
                    nc.tensor.matmul(sp[32 * i:32 * i + 32, :],
                                     q2[ch][base:base + 64, 0:32],
                                     kf[ch][base:base + 64, :],
                                     start=True, stop=True,
                                     tile_position=(base, 32 * i))
                e = ac.tile([128, FUS], bf16, tag=f"e2{t}", bufs=1,
                            name=f"e2{t}")
                la = ac.tile([128, 1], f32, tag=f"la{t}", bufs=2,
                             name=f"pla{t}")
                nc.scalar.activation(out=e[:], in_=sp[:], func=AF.Exp,
                                     scale=1.0 / spl, accum_out=la[:])
                e2.append(e)
                l2.append(la)
            e2T = []
            for t in range(2):
                pt = pgen.tile([FUS, 128], bf16, tag="g", name=f"pet{t}")
                nc.tensor.transpose(pt[:], e2[t][:], ident[:])
                s = ac.tile([FUS, 128], bf16, tag=f"e2T{t}", bufs=1,
                            name=f"e2Tb{t}")
                nc.vector.tensor_copy(out=s[:], in_=pt[:])
                e2T.append(s)
            of2 = []
            for t in range(2):
                acc = pacc.tile([128, 64], f32, tag="a", name=f"pacc2{t}")
                for i in range(4):
                    h = 4 * t + i
                    nc.tensor.matmul(acc[32 * i:32 * i + 32, :],
                                     e2T[t][:, 32 * i:32 * i + 32],
                                     Vp[3][0:FUS, DH * h:DH * (h + 1)],
                                     start=True, stop=True,
                                     tile_position=(0, 32 * i))
                li = ac.tile([128, 1], f32, tag="linv", bufs=2,
                             name=f"pli{t}")
                nc.vector.reciprocal(out=li[:], in_=l2[t][:])
                s = ac.tile([128, 64], bf16, tag=f"of{t}", bufs=1,
                            name=f"pof{t}")
                nc.vector.tensor_scalar_mul(out=s[:], in0=acc[:],
                                            scalar1=li[:])
                of2.append(s)
            ofI2 = [ac.tile([128, 1], f8, tag=f"ofI2{kc}", bufs=1,
                            name=f"ofI2{kc}") for kc in range(4)]
            for kc in range(4):
                pt = pacc.tile([128, 32], bf16, tag="a", name=f"ofI2p{kc}")
                for half in range(2):
                    h = 2 * kc + half
                    t, i = h // 4, h % 4
                    nc.tensor.transpose(
                        pt[64 * half:64 * half + 64, :],
                        of2[t][32 * i:32 * i + 32, :],
                        ident[32 * i:32 * i + 32, 32 * i:32 * i + 32],
                        tile_position=(32 * i, 64 * half))
                nc.vector.tensor_copy(out=ofI2[kc][:], in_=pt[:, 0:1])
            P2 = pacc.tile([1, D], f32, tag="a", name="P2")
            for kc in range(4):
                nc.tensor.matmul(P2[:], ofI2[kc][:],
                                 pwo8[:, kc // 2, kc % 2, :],
                                 start=(kc == 0), stop=(kc == 3))
            p2s = ac.tile([1, D], f32, tag="p2s", bufs=1, name="p2s")
            nc.scalar.activation(out=p2s[:], in_=P2[:], func=AF.Copy,
                                 scale=1.0 / spo)
            nc.sync.dma_start(out=out_f, in_=p2s[:])

    nc.compile()
    _built[key] = nc
    return nc


def _prep_inputs(inputs):
    """Host-side prep: fold gains, pick fp8 scales, pack weights per layer."""
    I = {k: np.asarray(v, np.float64) for k, v in inputs.items()}
    f32 = np.float32

    def col(x):
        return np.ascontiguousarray(np.asarray(x, f32).reshape(-1, 1))

    scale_dh = DH ** -0.5
    g = I["layers_attn_g"][:, :, None]
    wqf = I["layers_wq"] * scale_dh * g            # [L, 512, 512]
    wkvf = I["layers_wkv"] * g                     # [L, 512, 1024]
    wof = I["layers_wo"]                           # [L, 512, 512]
    w1f = I["layers_ff_w1"] * I["layers_ff_g"][:, :, None]  # [L, 512, 2730]
    w2f = I["layers_ff_w2"]                        # [L, 1365, 512]

    sq = _pow2_scale(wqf)
    skv = _pow2_scale(wkvf)
    so = _pow2_scale(wof)
    s1x = 16.0
    s1g = _pow2_scale(w1f[:, :, IFF:])
    s2 = _pow2_scale(w2f)
    pkvf = I["pool_wkv"] * I["final_g"][:, None]
    spl = _pow2_scale(pkvf)
    spo = _pow2_scale(I["pool_wo"])
    sef = {}
    for mod in ("rna", "atac"):
        ewf = I[f"{mod}_w"] * I[f"{mod}_ln1_g"][:, None]
        sef[mod] = (ewf, _pow2_scale(ewf))
    se = min(sef["rna"][1], sef["atac"][1])
    scales = (sq, skv, so, s1x, s1g, s2, se, spl, spo)

    # packed per-layer fp8 weights
    wpk = np.zeros((DEPTH, 128, FCOLS), F8)
    rows = np.zeros((DEPTH, 3, D), np.float64)
    for l in range(DEPTH):
        wpk[l, :, SEG_WQ:SEG_WKV] = _pack_pairs(wqf[l], sq).reshape(128, -1)
        wpk[l, :, SEG_WKV:SEG_WO] = _pack_pairs(wkvf[l], skv).reshape(128, -1)
        wpk[l, :, SEG_WO:SEG_W1] = _pack_pairs(wof[l], so).reshape(128, -1)
        w1p = np.zeros((D, 2 * IFFP), np.float64)
        w1p[:, 0:IFF] = w1f[l][:, 0:IFF] * s1x
        w1p[:, IFFP:IFFP + IFF] = w1f[l][:, IFF:] * s1g
        wpk[l, :, SEG_W1:SEG_W2] = _pack_pairs(w1p, 1.0).reshape(128, -1)
        w2p = np.zeros((IFF2, D), np.float64)
        w2p[0:IFF, :] = w2f[l] * s2
        wpk[l, :, SEG_W2:FCOLS] = _pack_pairs(w2p, 1.0).reshape(128, -1)
        # rank-1 rows: -scale*colsum/64
        rows[l, 0] = -skv * wkvf[l][:, 0:D].sum(0) / 64     # wksum
        rows[l, 1] = -skv * wkvf[l][:, D:2 * D].sum(0) / 64  # wvsum
        rows[l, 2] = -sq * wqf[l].sum(0) / 64                # wqsum
    rows_b = np.ascontiguousarray(rows.reshape(1, -1)).astype(BF)

    prow = np.zeros((1, 2 * D), np.float64)
    prow[0, 0:D] = -spl * pkvf[:, 0:D].sum(0) / 64
    prow[0, D:2 * D] = -spl * pkvf[:, D:2 * D].sum(0) / 64

    pwkv8 = _pack_pairs(pkvf, spl)
    pwo8 = _pack_pairs(I["pool_wo"], spo)

    # pool query for return token 2 (host, tiny)
    ret = I["return_tokens"].astype(f32)
    gp = I["pool_g"].astype(f32)
    mu = ret.mean(-1, keepdims=True)
    var = ((ret - mu) ** 2).mean(-1, keepdims=True)
    retn = (ret - mu) / np.sqrt(var + 1e-5) * gp
    q2 = (retn[2] @ I["pool_wq"].astype(f32)) * scale_dh

    fus8 = I["fusion_tokens"].T.reshape(4, 128, FUS).transpose(1, 0, 2)

    shared = {
        "wpk": wpk,
        "rows": rows_b,
        "prow": prow.astype(BF),
        "pwkv8": pwkv8,
        "pwo8": pwo8,
        "pool_q2": col(q2),
        "fus_t": np.ascontiguousarray(fus8).astype(np.float32),
    }

    in_maps = []
    for c in range(N_CORES):
        b, q = c // 4, c % 4
        mod = "rna" if q < 2 else "atac"
        x = I[mod][b, (q % 2) * OWN:(q % 2 + 1) * OWN, :]   # [384, 1024]
        m = dict(shared)
        # x8: [128, 4kp, 2sub, 384]
        xT = np.ascontiguousarray(x.T)                      # [1024, 384]
        m["x8"] = _pack_pairs(xT, 1.0)
        ewf = sef[mod][0]
        m["ew8"] = _pack_pairs(ewf, se)
        m["emb_b"] = col(I[f"{mod}_b"]
                         + I[f"{mod}_ln1_b"] @ I[f"{mod}_w"])
        m["erow"] = (-se * ewf.sum(0) / 128).reshape(1, -1).astype(BF)
        m["eln2_g"] = col(I[f"{mod}_ln2_g"])
        m["eln2_b"] = col(I[f"{mod}_ln2_b"])
        in_maps.append(m)
    return in_maps, ret.astype(f32), scales


def kernel(**inputs):
    from concourse import bass_utils
    in_maps, ret, scales = _prep_inputs(inputs)
    nc = build(num_devices=N_CORES, use_cc=True, scales=scales)
    res = bass_utils.run_bass_kernel_spmd(nc, in_maps,
                                          core_ids=list(range(N_CORES)))
    out = np.zeros((B, 3, D), np.float32)
    for b in range(2):
        r = res.results[4 * b]
        u = r["out_u"][:, 0]
        f = r["out_f"][0]
        out[b, 0] = u + ret[0]
        out[b, 1] = u + ret[1]
        out[b, 2] = f + ret[2]
    return out


# revision 17
# speedup vs baseline: 1.5496x; 1.0113x over previous
"""BioZorro sparse-attention kernel for 8 Trainium2 NeuronCores.

Sharding: 8 cores = 2 batches x 4 token-quarters (384 own tokens each).
The zorro mask makes all non-fusion query rows fully masked -> uniform
softmax -> their attention output is mean(V); only the 16 fusion tokens
attend (over the 1536 non-fusion keys). Cross-core data per layer is two
small AllGathers: (A) V column sums (2KB) issued early, (B) fusion
flash-softmax partials (66KB), plus one tiny AllGather for pooling.

Compute layout: residual stream feature-major (tok^T [512, 400] f32).
All heavy matmuls run in fp8e4 DoubleRow (contract 256/instr, 2x rate):
activations are cast to paired [128,2,T] tiles; weights are host-packed
into one fp8 buffer per layer (single DMA, double-buffered). LayerNorms
are folded into consumers: raw-cast -> matmul immediately; the -mu
correction enters PSUM as a rank-1 matmul (host-precomputed column sums
x the device S row); rstd is applied at PSUM eviction (column-broadcast
or per-token scalars). Per-tensor power-of-2 fp8 scales are descaled via
free immediate-scale slots (exp/gelu/copy activations).
"""
import sys
sys.path.insert(0, "/opt/trn_rl_repo")
import numpy as np
import ml_dtypes

BF = ml_dtypes.bfloat16
F8 = ml_dtypes.float8_e4m3
OWN, FUS, TOK = 384, 16, 400
D, RIN, H, DH, IFF, DEPTH = 512, 1024, 8, 64, 1365, 4
NALL = 1552
B, NR, NA = 2, 768, 768
N_CORES = 8
IFFP = 1408           # x/gate block padding (11 x 128)
IFF2 = 1536           # FF2 contract padding (6 x 256)

# fp8 packed-weight segment offsets (cols in the per-layer [128, FCOLS])
SEG_WQ = 0            # [2kp][2sub][512]
SEG_WKV = 2048        # [2kp][2sub][1024]
SEG_WO = SEG_WKV + 4096   # [2kp][2sub][512]
SEG_W1 = SEG_WO + 2048    # [2kp][2sub][2*1408]
SEG_W2 = SEG_W1 + 11264   # [6jp][2sub][512]
FCOLS = SEG_W2 + 6144

_built = {}


def _pow2_scale(w, target=120.0):
    m = float(np.abs(w).max())
    if m <= 0:
        return 1.0
    return float(2.0 ** np.floor(np.log2(target / m)))


def _pack_pairs(w, scale):
    """[K, N] f64 -> [128, K//256, 2, N] fp8 DoubleRow lhsT layout."""
    K, N = w.shape
    assert K % 256 == 0
    out = (w * scale).astype(F8).reshape(K // 128, 128, N)
    # chunk k = rows 128k..128k+128; pair kp = (2kp, 2kp+1)
    out = out.transpose(1, 0, 2).reshape(128, K // 256, 2, N)
    return np.ascontiguousarray(out)


def build(num_devices=8, use_cc=True, scales=None):
    key = (num_devices, use_cc, scales)
    if key in _built:
        return _built[key]
    import concourse.tile as tile
    from concourse import bacc, mybir
    from concourse.masks import make_identity

    # Force Exp to resolve to natural_log_exp_and_others so Ln/Exp/Square
    # live in one ACT table set (Gelu still needs its own set; those two
    # swaps per layer are prefetched off the critical path with dummy ops).
    if not getattr(bacc, "_act_tables_patched", False):
        _orig_gat = bacc.get_activation_tables

        def _patched_gat(arch):
            tabs = _orig_gat(arch)
            exp_t = mybir.ActivationFunctionType.Exp
            for nm, fns in tabs.items():
                if nm != "natural_log_exp_and_others":
                    fns.discard(exp_t)
            return tabs

        bacc.get_activation_tables = _patched_gat
        bacc._act_tables_patched = True

    sq, skv, so, s1x, s1g, s2, se, spl, spo = scales
    f32 = mybir.dt.float32
    bf16 = mybir.dt.bfloat16
    f8 = mybir.dt.float8e4
    AF = mybir.ActivationFunctionType
    OP = mybir.AluOpType
    DR = mybir.MatmulPerfMode.DoubleRow

    nc = bacc.Bacc("TRN2", target_bir_lowering=False, debug=False,
                   enable_asserts=True, num_devices=num_devices)

    def din(name, shape, dt=f32):
        return nc.dram_tensor(name, shape, dt, kind="ExternalInput").ap()

    x8_t = din("x8", [128, 4, 2, OWN], f8)
    ew8_t = din("ew8", [128, 4, 2, D], f8)
    ebias_t = din("emb_b", [D, 1])
    eg2_t = din("eln2_g", [D, 1])
    eb2_t = din("eln2_b", [D, 1])
    fus_t = din("fus_t", [128, 4, FUS], f32)
    wpk_t = din("wpk", [DEPTH, 128, FCOLS], f8)
    # host rank-1 rows: per layer [wksum, wvsum, wqsum] each [512]
    rows_t = din("rows", [1, DEPTH * 3 * D], bf16)
    erow_t = din("erow", [1, D], bf16)        # embed -se*colsum(ew')/128
    prow_t = din("prow", [1, 2 * D], bf16)    # pool [pwksum, pwvsum]
    pwkv8_t = din("pwkv8", [128, 2, 2, 2 * D], f8)
    pwo8_t = din("pwo8", [128, 2, 2, D], f8)
    pq2_t = din("pool_q2", [D, 1])
    out_u = nc.dram_tensor("out_u", [D, 1], f32, kind="ExternalOutput").ap()
    out_f = nc.dram_tensor("out_f", [1, D], f32, kind="ExternalOutput").ap()

    with tile.TileContext(nc) as tc:
        with tc.tile_pool(name="cst", bufs=1) as cst, \
             tc.tile_pool(name="wp", bufs=2) as wp, \
             tc.tile_pool(name="ac", bufs=2) as ac, \
             tc.tile_pool(name="pgen", bufs=4, space="PSUM") as pgen, \
             tc.tile_pool(name="pacc", bufs=4, space="PSUM") as pacc, \
             tc.tile_pool(name="dramp", bufs=2, space="DRAM") as dramp:

            ident = cst.tile([128, 128], bf16, name="ident")
            make_identity(nc, ident[:])
            ones128 = cst.tile([128, 1], bf16, name="ones128")
            nc.vector.memset(ones128[:], 1.0)
            ones1 = cst.tile([1, 128], bf16, name="ones1")
            nc.vector.memset(ones1[:], 1.0)
            epsc = cst.tile([128, 1], f32, name="epsc")
            nc.vector.memset(epsc[:], 1e-5)
            oi512 = cst.tile([128, 1], bf16, name="oi512")
            nc.vector.memset(oi512[:], 1.0 / 512)
            ones8p = cst.tile([128, 1], f8, name="ones8p")
            nc.vector.memset(ones8p[:], 0.125)
            cinvf = 1.0 / (s1x * s2)

            def load_cols(dram_ap, n, tag, rows=128):
                ts = []
                for c in range(n):
                    t = wp.tile([rows, 1], f32, tag=f"{tag}{c}", bufs=1,
                                name=f"{tag}{c}")
                    nc.sync.dma_start(out=t[:],
                                      in_=dram_ap[rows * c:rows * (c + 1), :])
                    ts.append(t)
                return ts

            # ---------- one-time loads ----------
            x8 = ac.tile([128, 4, 2, OWN], f8, tag="x8", bufs=1, name="x8")
            nc.sync.dma_start(out=x8[:], in_=x8_t)
            ew8 = wp.tile([128, 4, 2, D], f8, tag="ew8", bufs=1, name="ew8")
            nc.sync.dma_start(out=ew8[:], in_=ew8_t)
            rows = wp.tile([1, DEPTH * 3 * D], bf16, tag="rows", bufs=1,
                           name="rows")
            nc.sync.dma_start(out=rows[:], in_=rows_t)
            erow = wp.tile([1, D], bf16, tag="erow", bufs=1, name="erow")
            nc.sync.dma_start(out=erow[:], in_=erow_t)
            ebs = load_cols(ebias_t, 4, "ebias")
            eg2s = load_cols(eg2_t, 4, "eg2")
            eb2s = load_cols(eb2_t, 4, "eb2")

            # layer-0 weights DMA starts immediately
            wts = []
            w0 = wp.tile([128, FCOLS], f8, tag="wpk", bufs=2, name="wpk0")
            nc.sync.dma_start(out=w0[:], in_=wpk_t[0])

            # dummy exp to preload the nlexp ACT table during initial DMAs
            dtab = ac.tile([1, 1], f32, tag="dtab", bufs=2, name="dtab")
            nc.scalar.activation(out=dtab[:], in_=epsc[0:1, :], func=AF.Exp)

            # ---------- embed ----------
            # stats over raw x8 (LN1 folded into the embed matmul)
            Se = pgen.tile([1, OWN], f32, tag="g", name="Se")
            for i in range(8):
                nc.tensor.matmul(Se[:], ones8p[:], x8[:, i // 2, i % 2, :],
                                 start=(i == 0), stop=(i == 7))
            Seb = ac.tile([1, OWN], bf16, tag="rowb", bufs=4, name="Seb")
            nc.scalar.activation(out=Seb[:], in_=Se[:], func=AF.Copy)
            xsqe = []
            for kp in range(4):
                t = ac.tile([128, 2, OWN], bf16, tag="xsq", bufs=4,
                            name=f"xsqe{kp}")
                nc.scalar.activation(out=t[:], in_=x8[:, kp, :, :],
                                     func=AF.Square)
                xsqe.append(t)
            Qe = pgen.tile([1, OWN], f32, tag="g", name="Qe")
            oi1024 = cst.tile([128, 1], bf16, name="oi1024")
            nc.vector.memset(oi1024[:], 1.0 / 1024)
            for i in range(8):
                nc.tensor.matmul(Qe[:], oi1024[:], xsqe[i // 2][:, i % 2, :],
                                 start=(i == 0), stop=(i == 7))
            m2e = ac.tile([1, OWN], f32, tag="rowf", bufs=6, name="m2e")
            nc.scalar.activation(out=m2e[:], in_=Se[:], func=AF.Square,
                                 scale=1.0 / 128)
            vare = ac.tile([1, OWN], f32, tag="rowf", bufs=6, name="vare")
            nc.vector.tensor_sub(out=vare[:], in0=Qe[:], in1=m2e[:])
            rstde = ac.tile([1, OWN], f32, tag="rowf", bufs=6, name="rstde")
            nc.scalar.activation(out=rstde[:], in_=vare[:], func=AF.Ln,
                                 bias=epsc[0:1, :])
            nc.scalar.activation(out=rstde[:], in_=rstde[:], func=AF.Exp,
                                 scale=-0.5)
            rstdeb = ac.tile([1, OWN], bf16, tag="rowb", bufs=4, name="rstdeb")
            nc.scalar.activation(out=rstdeb[:], in_=rstde[:], func=AF.Copy,
                                 scale=1.0 / se)
            bRe_p = pgen.tile([128, OWN], f32, tag="g", name="bRe")
            nc.tensor.matmul(bRe_p[:], ones1[:], rstdeb[:], start=True,
                             stop=True)
            bRe = ac.tile([128, OWN], bf16, tag="bR", bufs=2, name="bReb")
            nc.vector.tensor_copy(out=bRe[:], in_=bRe_p[:])

            hb = []
            for mc in range(4):
                ps = pgen.tile([128, OWN], f32, tag="g", name=f"embp{mc}")
                for kp in range(4):
                    nc.tensor.matmul(ps[:], ew8[:, kp, :, 128 * mc:128 * (mc + 1)],
                                     x8[:, kp, :, :], start=(kp == 0),
                                     stop=False, perf_mode=DR)
                nc.tensor.matmul(ps[:], erow[:, 128 * mc:128 * (mc + 1)],
                                 Seb[:], start=False, stop=True)
                t1 = ac.tile([128, OWN], bf16, tag="embt", bufs=2,
                             name=f"embt{mc}")
                nc.vector.tensor_mul(out=t1[:], in0=ps[:], in1=bRe[:])
                t2 = ac.tile([128, OWN], bf16, tag=f"hb{mc}", bufs=1,
                             name=f"hb{mc}")
                nc.vector.tensor_scalar_add(out=t2[:], in0=t1[:],
                                            scalar1=ebs[mc][:])
                hb.append(t2)

            # embed LN2 (explicit normalize into f32 tok)
            S2e = pgen.tile([1, OWN], f32, tag="g", name="S2e")
            for c in range(4):
                nc.tensor.matmul(S2e[:], oi512[:], hb[c][:],
                                 start=(c == 0), stop=(c == 3))
            x2e = []
            for c in range(4):
                t = ac.tile([128, OWN], bf16, tag="xsq", bufs=4,
                            name=f"x2e{c}")
                nc.vector.tensor_mul(out=t[:], in0=hb[c][:], in1=hb[c][:])
                x2e.append(t)
            Q2e = pgen.tile([1, OWN], f32, tag="g", name="Q2e")
            for c in range(4):
                nc.tensor.matmul(Q2e[:], oi512[:], x2e[c][:],
                                 start=(c == 0), stop=(c == 3))
            m22 = ac.tile([1, OWN], f32, tag="rowf", bufs=6, name="m22")
            nc.scalar.activation(out=m22[:], in_=S2e[:], func=AF.Square)
            var2 = ac.tile([1, OWN], f32, tag="rowf", bufs=6, name="var2e")
            nc.vector.tensor_sub(out=var2[:], in0=Q2e[:], in1=m22[:])
            rstd2e = ac.tile([1, OWN], f32, tag="rowf", bufs=6, name="rstd2e")
            nc.scalar.activation(out=rstd2e[:], in_=var2[:], func=AF.Ln,
                                 bias=epsc[0:1, :])
            nc.scalar.activation(out=rstd2e[:], in_=rstd2e[:], func=AF.Exp,
                                 scale=-0.5)
            pr2 = ac.tile([1, 2 * OWN], bf16, tag="rowb2", bufs=2, name="pr2e")
            nc.vector.tensor_copy(out=pr2[:, 0:OWN], in_=rstd2e[:])
            mre = ac.tile([1, OWN], f32, tag="rowf", bufs=6, name="mre")
            nc.vector.tensor_mul(out=mre[:], in0=S2e[:], in1=rstd2e[:])
            nc.vector.tensor_copy(out=pr2[:, OWN:2 * OWN], in_=mre[:])
            bR2e_p = pgen.tile([128, OWN], f32, tag="g", name="bR2e")
            nc.tensor.matmul(bR2e_p[:], ones1[:], pr2[:, 0:OWN], start=True,
                             stop=True)
            bR2e = ac.tile([128, OWN], bf16, tag="bR", bufs=2, name="bR2eb")
            nc.vector.tensor_copy(out=bR2e[:], in_=bR2e_p[:])
            bM2e_p = pgen.tile([128, OWN], f32, tag="g", name="bM2e")
            nc.tensor.matmul(bM2e_p[:], ones1[:], pr2[:, OWN:2 * OWN],
                             start=True, stop=True)
            bM2e = ac.tile([128, OWN], bf16, tag="bM", bufs=2, name="bM2eb")
            nc.vector.tensor_copy(out=bM2e[:], in_=bM2e_p[:])

            tok = [ac.tile([128, TOK], f32, tag=f"tok{c}", bufs=1,
                           name=f"tok{c}") for c in range(4)]
            for c in range(4):
                t1 = ac.tile([128, OWN], bf16, tag="embt", bufs=2,
                             name=f"eln{c}")
                nc.vector.tensor_mul(out=t1[:], in0=hb[c][:], in1=bR2e[:])
                nc.vector.tensor_sub(out=t1[:], in0=t1[:], in1=bM2e[:])
                nc.vector.tensor_scalar(out=tok[c][:, 0:OWN], in0=t1[:],
                                        scalar1=eg2s[c][:], scalar2=eb2s[c][:],
                                        op0=OP.mult, op1=OP.add)
                nc.sync.dma_start(out=tok[c][:, OWN:TOK], in_=fus_t[:, c, :])

            tok_chunks = [(0, 128), (128, 256), (256, 384), (384, 400)]
            rg = [[0, 1, 2, 3], [4, 5, 6, 7]]
            psO_prev = None

            # rank-1 row views per layer
            def lrow(l, which):
                base = (l * 3 + which) * D
                return rows[:, base:base + D]

            # ---------- layers ----------
            for l in range(DEPTH):
                wT = w0 if l == 0 else wp.tile([128, FCOLS], f8, tag="wpk",
                                               bufs=2, name=f"wpk{l}")
                if l > 0:
                    nc.sync.dma_start(out=wT[:], in_=wpk_t[l])
                wq8 = wT[:, SEG_WQ:SEG_WKV].rearrange(
                    "p (kp s o) -> p kp s o", kp=2, s=2)
                wkv8 = wT[:, SEG_WKV:SEG_WO].rearrange(
                    "p (kp s o) -> p kp s o", kp=2, s=2)
                wo8 = wT[:, SEG_WO:SEG_W1].rearrange(
                    "p (kp s o) -> p kp s o", kp=2, s=2)
                w18 = wT[:, SEG_W1:SEG_W2].rearrange(
                    "p (kp s o) -> p kp s o", kp=2, s=2)
                w28 = wT[:, SEG_W2:FCOLS].rearrange(
                    "p (jp s o) -> p jp s o", jp=6, s=2)

                # --- LN1 prep: residual + fp8 raw cast ---
                tr8 = [ac.tile([128, 2, TOK], f8, tag=f"tr8{kp}", bufs=1,
                               name=f"tr8_{kp}") for kp in range(2)]
                for c in range(4):
                    if psO_prev is not None:
                        t = ac.tile([128, TOK], bf16, tag="psot", bufs=2,
                                    name=f"psot{c}")
                        nc.scalar.activation(out=t[:], in_=psO_prev[c][:],
                                             func=AF.Copy, scale=cinvf)
                        nc.vector.tensor_add(out=tok[c][:], in0=tok[c][:],
                                             in1=t[:])
                    nc.vector.tensor_copy(out=tr8[c // 2][:, c % 2, :],
                                          in_=tok[c][:])
                psO_prev = None

                # --- stats1 (rank-1 style folded LN) ---
                S1 = pgen.tile([1, TOK], f32, tag="g", name="S1")
                for i in range(4):
                    nc.tensor.matmul(S1[:], ones8p[:], tr8[i // 2][:, i % 2, :],
                                     start=(i == 0), stop=(i == 3))
                S1b = ac.tile([1, TOK], bf16, tag="rowb", bufs=4, name="S1b")
                nc.scalar.activation(out=S1b[:], in_=S1[:], func=AF.Copy)
                m2 = ac.tile([1, TOK], f32, tag="rowf", bufs=6, name="m2")
                nc.scalar.activation(out=m2[:], in_=S1[:], func=AF.Square,
                                     scale=1.0 / 64)
                xsq = []
                for kp in range(2):
                    t = ac.tile([128, 2, TOK], bf16, tag="xsq", bufs=4,
                                name=f"xsq{kp}")
                    if kp == 0:
                        nc.vector.tensor_mul(out=t[:], in0=tr8[kp][:],
                                             in1=tr8[kp][:])
                    else:
                        nc.scalar.activation(out=t[:], in_=tr8[kp][:],
                                             func=AF.Square)
                    xsq.append(t)

                # --- V raw (+ early evict, rstd folded into consumers) ---
                V = []
                for i, (a, b) in enumerate(tok_chunks):
                    m = b - a
                    ps = pacc.tile([128, D], f32, tag="a", name=f"v{i}")
                    for kp in range(2):
                        nc.tensor.matmul(ps[0:m, :], tr8[kp][:, :, a:b],
                                         wkv8[:, kp, :, D:2 * D],
                                         start=(kp == 0), stop=False,
                                         perf_mode=DR)
                    nc.tensor.matmul(ps[0:m, :], S1b[:, a:b],
                                     lrow(l, 1), start=False, stop=True)
                    sV = ac.tile([128, D], bf16, tag=f"V{i}", bufs=1,
                                 name=f"Vb{i}")
                    nc.scalar.activation(out=sV[0:m, :], in_=ps[0:m, :],
                                         func=AF.Copy, scale=1.0 / skv)
                    V.append(sV)

                # --- K^T, Qf raw matmuls ---
                ktp, qfp = [], []
                for mc in range(4):
                    ps = pacc.tile([128, OWN], f32, tag="a", name=f"kt{mc}")
                    for kp in range(2):
                        nc.tensor.matmul(ps[:],
                                         wkv8[:, kp, :, 128 * mc:128 * (mc + 1)],
                                         tr8[kp][:, :, 0:OWN],
                                         start=(kp == 0), stop=False,
                                         perf_mode=DR)
                    nc.tensor.matmul(ps[:], lrow(l, 0)[:, 128 * mc:128 * (mc + 1)],
                                     S1b[:, 0:OWN], start=False, stop=True)
                    ktp.append(ps)
                # --- stats1 tail ---
                Q1 = pgen.tile([1, TOK], f32, tag="g", name="Q1")
                for i in range(4):
                    nc.tensor.matmul(Q1[:], oi512[:], xsq[i // 2][:, i % 2, :],
                                     start=(i == 0), stop=(i == 3))
                var = ac.tile([1, TOK], f32, tag="rowf", bufs=6, name="var")
                nc.vector.tensor_sub(out=var[:], in0=Q1[:], in1=m2[:])
                rstd = ac.tile([1, TOK], f32, tag="rowf", bufs=6, name="rstd")
                nc.scalar.activation(out=rstd[:], in_=var[:], func=AF.Ln,
                                     bias=epsc[0:1, :])
                nc.scalar.activation(out=rstd[:], in_=rstd[:], func=AF.Exp,
                                     scale=-0.5)
                rstdb = ac.tile([1, TOK], bf16, tag="rowb", bufs=4,
                                name="rstdb")
                nc.vector.tensor_copy(out=rstdb[:], in_=rstd[:])
                bR1_p = pgen.tile([128, TOK], f32, tag="g", name="bR1")
                nc.tensor.matmul(bR1_p[:], ones1[:], rstdb[:], start=True,
                                 stop=True)
                bR1 = ac.tile([128, TOK], bf16, tag="bR", bufs=2, name="bR1b")
                nc.vector.tensor_copy(out=bR1[:], in_=bR1_p[:])
                rT_p = pgen.tile([128, 4], f32, tag="g", name="rTp")
                for c, (a_, b_) in enumerate(tok_chunks):
                    nc.tensor.matmul(rT_p[0:b_ - a_, c:c + 1],
                                     rstdb[:, a_:b_],
                                     ones1[0:1, 0:1], start=True, stop=True)
                rT = ac.tile([128, 4], bf16, tag="rT", bufs=2, name="rT")
                nc.vector.tensor_copy(out=rT[:], in_=rT_p[:])
                rTs = ac.tile([128, 4], f32, tag="rTs", bufs=2, name="rTs")
                nc.vector.tensor_copy(out=rTs[:], in_=rT_p[:])

                # --- Qf raw (after stats-tail matmuls; "a" slots free
                # once kt evictions land) ---
                for mc in range(4):
                    ps = pacc.tile([128, FUS], f32, tag="a", name=f"qf{mc}")
                    for kp in range(2):
                        nc.tensor.matmul(ps[:],
                                         wq8[:, kp, :, 128 * mc:128 * (mc + 1)],
                                         tr8[kp][:, :, OWN:TOK],
                                         start=(kp == 0), stop=False,
                                         perf_mode=DR)
                    nc.tensor.matmul(ps[:], lrow(l, 2)[:, 128 * mc:128 * (mc + 1)],
                                     S1b[:, OWN:TOK], start=False, stop=True)
                    qfp.append(ps)

                # --- vsums (rstd via rhs) -> exchange A ---
                PA = ac.tile([128, 4], f32, tag="PA", bufs=2, name="PA")
                for c in range(4):
                    ps = pgen.tile([128, 1], f32, tag="g", name=f"vs{c}")
                    for j in range(3):
                        nc.tensor.matmul(ps[:], V[j][:, 128 * c:128 * (c + 1)],
                                         rT[:, j:j + 1], start=(j == 0),
                                         stop=(j == 2))
                    nc.vector.tensor_copy(out=PA[:, c:c + 1], in_=ps[:])
                vfu = ac.tile([128, 4], f32, tag="vfu", bufs=2, name="vfu")
                for c in range(4):
                    ps = pgen.tile([128, 1], f32, tag="g", name=f"vf{c}")
                    nc.tensor.matmul(ps[:], V[3][0:FUS, 128 * c:128 * (c + 1)],
                                     rT[0:FUS, 3:4], start=True, stop=True)
                    nc.vector.tensor_copy(out=vfu[:, c:c + 1], in_=ps[:])
                pinA = dramp.tile([128, 4], f32, tag="pinA", bufs=2,
                                  name="pinA")
                nc.sync.dma_start(out=pinA[:], in_=PA[:])
                RA = ac.tile([128, 4, 4], f32, tag="RA", bufs=2, name="RA")
                if use_cc:
                    poutA = dramp.tile([4 * 128, 4], f32, tag="poutA", bufs=2,
                                       name="poutA")
                    nc.gpsimd.collective_compute(
                        "AllGather", OP.bypass, replica_groups=rg,
                        ins=[pinA.opt()], outs=[poutA.opt()])
                    nc.sync.dma_start(
                        out=RA[:], in_=poutA.rearrange("(r p) f -> p r f", r=4))
                else:
                    nc.sync.dma_start(
                        out=RA[:],
                        in_=pinA.rearrange("(r p) f -> p r f", r=1)
                        .to_broadcast((128, 4, 4)))

                # --- kt/qf evictions (x rstd columns) ---
                kt = []
                for mc in range(4):
                    sK = ac.tile([128, OWN], bf16, tag=f"kt{mc}", bufs=1,
                                 name=f"ktb{mc}")
                    nc.vector.tensor_mul(out=sK[:], in0=ktp[mc][:],
                                         in1=bR1[:, 0:OWN])
                    kt.append(sK)
                qf = []
                for mc in range(4):
                    sQ = ac.tile([128, 32], bf16, tag=f"qf{mc}", bufs=1,
                                 name=f"qfb{mc}")
                    nc.vector.memset(sQ[:, FUS:32], 0.0)
                    nc.vector.tensor_mul(out=sQ[:, 0:FUS], in0=qfp[mc][:],
                                         in1=bR1[:, OWN:TOK])
                    qf.append(sQ)

                # --- scores + exp ---
                E, lacc = [], []
                for t in range(2):
                    sp = pgen.tile([128, OWN], f32, tag="g", name=f"sp{t}")
                    for i in range(4):
                        h = 4 * t + i
                        ch, base = h // 2, (h % 2) * 64
                        nc.tensor.matmul(sp[32 * i:32 * i + 32, :],
                                         qf[ch][base:base + 64, 0:32],
                                         kt[ch][base:base + 64, :],
                                         start=True, stop=True,
                                         tile_position=(base, 32 * i))
                    e = ac.tile([128, OWN], bf16, tag=f"e{t}", bufs=1,
                                name=f"e{t}")
                    la = ac.tile([128, 1], f32, tag=f"la{t}", bufs=2,
                                 name=f"la{t}")
                    nc.scalar.activation(out=e[:], in_=sp[:], func=AF.Exp,
                                         scale=1.0 / (sq * skv),
                                         accum_out=la[:])
                    E.append(e)
                    lacc.append(la)

                # --- E^T + ACC -> exchange B ---
                ET = [[None] * 3 for _ in range(2)]
                for t in range(2):
                    for j in range(3):
                        pt = pgen.tile([128, 128], bf16, tag="g",
                                       name=f"et{t}{j}")
                        nc.tensor.transpose(pt[:], E[t][:, 128 * j:128 * (j + 1)],
                                            ident[:])
                        s = ac.tile([128, 128], bf16, tag=f"ET{t}{j}", bufs=1,
                                    name=f"ETb{t}{j}")
                        nc.vector.tensor_scalar_mul(out=s[:], in0=pt[:],
                                                    scalar1=rTs[:, j:j + 1])
                        ET[t][j] = s
                PB = ac.tile([128, 130], f32, tag="PB", bufs=2, name="PB")
                nc.vector.tensor_copy(out=PB[:, 0:1], in_=lacc[0][:])
                nc.vector.tensor_copy(out=PB[:, 1:2], in_=lacc[1][:])
                for t in range(2):
                    acc = pacc.tile([128, 64], f32, tag="a", name=f"acc{t}")
                    for i in range(4):
                        h = 4 * t + i
                        for j in range(3):
                            nc.tensor.matmul(acc[32 * i:32 * i + 32, :],
                                             ET[t][j][:, 32 * i:32 * i + 32],
                                             V[j][:, DH * h:DH * (h + 1)],
                                             start=(j == 0), stop=(j == 2),
                                             tile_position=(0, 32 * i))
                    nc.vector.tensor_copy(out=PB[:, 2 + 64 * t:66 + 64 * t],
                                          in_=acc[:])
                pinB = dramp.tile([128, 130], f32, tag="pinB", bufs=2,
                                  name="pinB")
                nc.sync.dma_start(out=pinB[:], in_=PB[:])
                RB = ac.tile([128, 4, 130], f32, tag="RB", bufs=2, name="RB")
                if use_cc:
                    poutB = dramp.tile([4 * 128, 130], f32, tag="poutB",
                                       bufs=2, name="poutB")
                    nc.gpsimd.collective_compute(
                        "AllGather", OP.bypass, replica_groups=rg,
                        ins=[pinB.opt()], outs=[poutB.opt()])
                    nc.sync.dma_start(
                        out=RB[:], in_=poutB.rearrange("(r p) f -> p r f", r=4))
                else:
                    nc.sync.dma_start(
                        out=RB[:],
                        in_=pinB.rearrange("(r p) f -> p r f", r=1)
                        .to_broadcast((128, 4, 130)))

                # --- A-combine -> uniform delta -> own-col LN2 prep ---
                vsb = ac.tile([128, 4], f32, tag="vsb", bufs=2, name="vsb")
                nc.vector.tensor_reduce(out=vsb[:],
                                        in_=RA[:].rearrange("p r f -> p f r"),
                                        axis=mybir.AxisListType.X,
                                        op=OP.add)
                nc.vector.tensor_add(out=vsb[:], in0=vsb[:], in1=vfu[:])
                vsb8 = ac.tile([128, 4], f8, tag="vsb8", bufs=2,
                               name="vsb8")
                nc.vector.tensor_copy(out=vsb8[:], in_=vsb[:])
                dup = pgen.tile([128, 4], f32, tag="g", name="dup")
                for c in range(4):
                    for kc in range(4):
                        nc.tensor.matmul(dup[:, c:c + 1],
                                         wo8[:, kc // 2, kc % 2,
                                             128 * c:128 * (c + 1)],
                                         vsb8[:, kc:kc + 1],
                                         start=(kc == 0), stop=(kc == 3))
                dus = ac.tile([128, 4], f32, tag="dus", bufs=2, name="dus")
                nc.scalar.activation(out=dus[:], in_=dup[:], func=AF.Copy,
                                     scale=1.0 / (so * NALL))
                t28 = [ac.tile([128, 2, TOK], f8, tag=f"t28_{kp}", bufs=1,
                               name=f"t28_{kp}") for kp in range(2)]
                for c in range(4):
                    nc.vector.tensor_scalar_add(out=tok[c][:, 0:OWN],
                                                in0=tok[c][:, 0:OWN],
                                                scalar1=dus[:, c:c + 1])
                    nc.vector.tensor_copy(out=t28[c // 2][:, c % 2, 0:OWN],
                                          in_=tok[c][:, 0:OWN])
                S2 = pgen.tile([1, TOK], f32, tag="g", name="S2")
                for i in range(4):
                    nc.tensor.matmul(S2[:, 0:OWN], ones8p[:],
                                     t28[i // 2][:, i % 2, 0:OWN],
                                     start=(i == 0), stop=(i == 3))
                mur2b = ac.tile([1, TOK], bf16, tag="rowb", bufs=4,
                                name="mur2b")
                nc.scalar.activation(out=mur2b[:, 0:OWN], in_=S2[:, 0:OWN],
                                     func=AF.Copy, scale=1.0 / 64)
                bMu_p = pgen.tile([128, TOK], f32, tag="g", name="bMu")
                nc.tensor.matmul(bMu_p[:, 0:OWN], ones1[:], mur2b[:, 0:OWN],
                                 start=True, stop=True)
                bMu = ac.tile([128, TOK], bf16, tag="bM", bufs=2, name="bMub")
                nc.vector.tensor_copy(out=bMu[:, 0:OWN], in_=bMu_p[:, 0:OWN])
                for c in range(4):
                    nc.vector.tensor_sub(out=tok[c][:, 0:OWN],
                                         in0=tok[c][:, 0:OWN],
                                         in1=bMu[:, 0:OWN])
                xsq2 = []
                for kp in range(2):
                    t = ac.tile([128, 2, TOK], bf16, tag="xsq", bufs=4,
                                name=f"xsq2{kp}")
                    nc.scalar.activation(out=t[:, :, 0:OWN],
                                         in_=t28[kp][:, :, 0:OWN],
                                         func=AF.Square)
                    xsq2.append(t)
                Q2 = pgen.tile([1, TOK], f32, tag="g", name="Q2")
                for i in range(4):
                    nc.tensor.matmul(Q2[:, 0:OWN], oi512[:],
                                     xsq2[i // 2][:, i % 2, 0:OWN],
                                     start=(i == 0), stop=(i == 3))

                # --- B-combine -> fusion delta -> fusion-col LN2 prep ---
                PT = ac.tile([128, 130], f32, tag="cmbB", bufs=3, name="PT")
                nc.vector.tensor_reduce(out=PT[:],
                                        in_=RB[:].rearrange("p r f -> p f r"),
                                        axis=mybir.AxisListType.X,
                                        op=OP.add)
                linv = ac.tile([128, 2], f32, tag="linv", bufs=2, name="linv")
                nc.vector.reciprocal(out=linv[:], in_=PT[:, 0:2])
                of = []
                for t in range(2):
                    s = ac.tile([128, 64], bf16, tag=f"of{t}", bufs=1,
                                name=f"of{t}")
                    nc.vector.tensor_scalar_mul(out=s[:],
                                                in0=PT[:, 2 + 64 * t:66 + 64 * t],
                                                scalar1=linv[:, t:t + 1])
                    of.append(s)
                ofI8 = [ac.tile([128, 2, FUS], f8, tag=f"ofI{kp}", bufs=1,
                                name=f"ofI{kp}") for kp in range(2)]
                for kc in range(4):
                    pt = pacc.tile([128, 32], bf16, tag="a", name=f"ofIp{kc}")
                    for half in range(2):
                        h = 2 * kc + half
                        t, i = h // 4, h % 4
                        nc.tensor.transpose(
                            pt[64 * half:64 * half + 64, :],
                            of[t][32 * i:32 * i + 32, :],
                            ident[32 * i:32 * i + 32, 32 * i:32 * i + 32],
                            tile_position=(32 * i, 64 * half))
                    nc.vector.tensor_copy(out=ofI8[kc // 2][:, kc % 2, :],
                                          in_=pt[:, 0:FUS])
                dfp = pacc.tile([FUS, D], f32, tag="a", name="dfp")
                for kp in range(2):
                    nc.tensor.matmul(dfp[:], ofI8[kp][:], wo8[:, kp, :, :],
                                     start=(kp == 0), stop=(kp == 1),
                                     perf_mode=DR)
                dfb = ac.tile([FUS, D], bf16, tag="dfb", bufs=2, name="dfb")
                nc.scalar.activation(out=dfb[:], in_=dfp[:], func=AF.Copy,
                                     scale=1.0 / so)
                for c in range(4):
                    pt = pacc.tile([128, FUS], bf16, tag="a", name=f"dft{c}")
                    nc.tensor.transpose(pt[:], dfb[0:FUS, 128 * c:128 * (c + 1)],
                                        ident[0:FUS, 0:FUS])
                    nc.vector.tensor_add(out=tok[c][:, OWN:TOK],
                                         in0=tok[c][:, OWN:TOK], in1=pt[:])
                    nc.vector.tensor_copy(out=t28[c // 2][:, c % 2, OWN:TOK],
                                          in_=tok[c][:, OWN:TOK])
                for i in range(4):
                    nc.tensor.matmul(S2[:, OWN:TOK], ones8p[:],
                                     t28[i // 2][:, i % 2, OWN:TOK],
                                     start=(i == 0), stop=(i == 3))
                for kp in range(2):
                    nc.scalar.activation(out=xsq2[kp][:, :, OWN:TOK],
                                         in_=t28[kp][:, :, OWN:TOK],
                                         func=AF.Square)
                for i in range(4):
                    nc.tensor.matmul(Q2[:, OWN:TOK], oi512[:],
                                     xsq2[i // 2][:, i % 2, OWN:TOK],
                                     start=(i == 0), stop=(i == 3))

                # --- fusion-col centering + stats2 tail + normalize-cast ---
                nc.scalar.activation(out=mur2b[:, OWN:TOK], in_=S2[:, OWN:TOK],
                                     func=AF.Copy, scale=1.0 / 64)
                nc.tensor.matmul(bMu_p[:, OWN:TOK], ones1[:],
                                 mur2b[:, OWN:TOK], start=True, stop=True)
                nc.vector.tensor_copy(out=bMu[:, OWN:TOK],
                                      in_=bMu_p[:, OWN:TOK])
                for c in range(4):
                    nc.vector.tensor_sub(out=tok[c][:, OWN:TOK],
                                         in0=tok[c][:, OWN:TOK],
                                         in1=bMu[:, OWN:TOK])
                m2_2 = ac.tile([1, TOK], f32, tag="rowf", bufs=6, name="m2_2")
                nc.scalar.activation(out=m2_2[:], in_=S2[:], func=AF.Square,
                                     scale=1.0 / 64)
                var2 = ac.tile([1, TOK], f32, tag="rowf", bufs=6, name="var2")
                nc.vector.tensor_sub(out=var2[:], in0=Q2[:], in1=m2_2[:])
                rstd2 = ac.tile([1, TOK], f32, tag="rowf", bufs=6,
                                name="rstd2")
                nc.scalar.activation(out=rstd2[:], in_=var2[:], func=AF.Ln,
                                     bias=epsc[0:1, :])
                nc.scalar.activation(out=rstd2[:], in_=rstd2[:], func=AF.Exp,
                                     scale=-0.5)
                rstd2b = ac.tile([1, TOK], bf16, tag="rowb", bufs=4,
                                 name="rstd2b")
                nc.vector.tensor_copy(out=rstd2b[:], in_=rstd2[:])
                bR2_p = pgen.tile([128, TOK], f32, tag="g", name="bR2")
                nc.tensor.matmul(bR2_p[:], ones1[:], rstd2b[:],
                                 start=True, stop=True)
                bR2 = ac.tile([128, TOK], bf16, tag="bR", bufs=2, name="bR2b")
                nc.vector.tensor_copy(out=bR2[:], in_=bR2_p[:])
                # dummy gelu: preload the gelu table while FF1 matmuls run
                dt2 = ac.tile([1, 1], f32, tag="dtab", bufs=2, name="dt2")
                nc.scalar.activation(out=dt2[:], in_=rstd2b[0:1, 0:1],
                                     func=AF.Gelu)
                xc8 = [ac.tile([128, 2, TOK], f8, tag=f"xc8_{kp}", bufs=1,
                               name=f"xc8_{kp}") for kp in range(2)]
                for c in range(4):
                    nc.vector.tensor_mul(out=xc8[c // 2][:, c % 2, :],
                                         in0=tok[c][:], in1=bR2[:])

                # --- FF1 / GEGLU / FF2 ---
                gt8 = [ac.tile([128, 2, TOK], f8, tag=f"gt{jp}", bufs=1,
                               name=f"gt{jp}") for jp in range(6)]
                nc.vector.memset(gt8[5][:, 1, :], 0.0)
                for j in range(11):
                    px = pgen.tile([128, TOK], f32, tag="g", name=f"fx{j}")
                    pg = pacc.tile([128, TOK], f32, tag="a", name=f"fg{j}")
                    for kp in range(2):
                        nc.tensor.matmul(px[:], w18[:, kp, :, 128 * j:128 * (j + 1)],
                                         xc8[kp][:], start=(kp == 0),
                                         stop=(kp == 1), perf_mode=DR)
                    for kp in range(2):
                        nc.tensor.matmul(
                            pg[:],
                            w18[:, kp, :, IFFP + 128 * j:IFFP + 128 * (j + 1)],
                            xc8[kp][:], start=(kp == 0), stop=(kp == 1),
                            perf_mode=DR)
                    gg = ac.tile([128, TOK], bf16, tag="gg", bufs=3,
                                 name=f"gg{j}")
                    nc.scalar.activation(out=gg[:], in_=pg[:], func=AF.Gelu,
                                         scale=1.0 / s1g)
                    nc.vector.tensor_mul(out=gt8[j // 2][:, j % 2, :],
                                         in0=gg[:], in1=px[:])
                # dummy exp: preload nlexp for the next layer during FF2
                dt3 = ac.tile([1, 1], f32, tag="dtab", bufs=2, name="dt3")
                nc.scalar.activation(out=dt3[:], in_=gt8[5][0:1, 0, 0:1],
                                     func=AF.Exp)
                psO_prev = []
                for c in range(4):
                    psO = pacc.tile([128, TOK], f32, tag="a", name=f"fo{c}")
                    for jp in range(6):
                        nc.tensor.matmul(psO[:], w28[:, jp, :, 128 * c:128 * (c + 1)],
                                         gt8[jp][:], start=(jp == 0),
                                         stop=(jp == 5), perf_mode=DR)
                    psO_prev.append(psO)

            # ---------- pool ----------
            pwkv8 = wp.tile([128, 2, 2, 2 * D], f8, tag="pwkv8", bufs=1,
                            name="pwkv8")
            nc.sync.dma_start(out=pwkv8[:], in_=pwkv8_t)
            pwo8 = wp.tile([128, 2, 2, D], f8, tag="pwo8", bufs=1,
                           name="pwo8")
            nc.sync.dma_start(out=pwo8[:], in_=pwo8_t)
            prow = wp.tile([1, 2 * D], bf16, tag="prow", bufs=1, name="prow")
            nc.sync.dma_start(out=prow[:], in_=prow_t)
            pq2s = load_cols(pq2_t, 4, "pq2")

            # final LN (rank-1 folded) over tok + last FF residual
            tf8 = [ac.tile([128, 2, TOK], f8, tag=f"tr8{kp}", bufs=1,
                           name=f"tf8_{kp}") for kp in range(2)]
            for c in range(4):
                t = ac.tile([128, TOK], bf16, tag="psot", bufs=2,
                            name=f"fpsot{c}")
                nc.scalar.activation(out=t[:], in_=psO_prev[c][:],
                                     func=AF.Copy, scale=cinvf)
                nc.vector.tensor_add(out=tok[c][:], in0=tok[c][:], in1=t[:])
                nc.vector.tensor_copy(out=tf8[c // 2][:, c % 2, :],
                                      in_=tok[c][:])
            Sf = pgen.tile([1, TOK], f32, tag="g", name="Sf")
            for i in range(4):
                nc.tensor.matmul(Sf[:], ones8p[:], tf8[i // 2][:, i % 2, :],
                                 start=(i == 0), stop=(i == 3))
            Sfb = ac.tile([1, TOK], bf16, tag="rowb", bufs=4, name="Sfb")
            nc.scalar.activation(out=Sfb[:], in_=Sf[:], func=AF.Copy)
            xsqf = []
            for kp in range(2):
                t = ac.tile([128, 2, TOK], bf16, tag="xsq", bufs=4,
                            name=f"xsqf{kp}")
                if kp == 0:
                    nc.vector.tensor_mul(out=t[:], in0=tf8[kp][:],
                                         in1=tf8[kp][:])
                else:
                    nc.scalar.activation(out=t[:], in_=tf8[kp][:],
                                         func=AF.Square)
                xsqf.append(t)
            Qf_ = pgen.tile([1, TOK], f32, tag="g", name="Qf_")
            for i in range(4):
                nc.tensor.matmul(Qf_[:], oi512[:], xsqf[i // 2][:, i % 2, :],
                                 start=(i == 0), stop=(i == 3))
            m2f = ac.tile([1, TOK], f32, tag="rowf", bufs=6, name="m2f")
            nc.scalar.activation(out=m2f[:], in_=Sf[:], func=AF.Square,
                                 scale=1.0 / 64)
            varf = ac.tile([1, TOK], f32, tag="rowf", bufs=6, name="varf")
            nc.vector.tensor_sub(out=varf[:], in0=Qf_[:], in1=m2f[:])
            rstdf = ac.tile([1, TOK], f32, tag="rowf", bufs=6, name="rstdf")
            nc.scalar.activation(out=rstdf[:], in_=varf[:], func=AF.Ln,
                                 bias=epsc[0:1, :])
            nc.scalar.activation(out=rstdf[:], in_=rstdf[:], func=AF.Exp,
                                 scale=-0.5)
            rstdfb = ac.tile([1, TOK], bf16, tag="rowb", bufs=4,
                             name="rstdfb")
            nc.vector.tensor_copy(out=rstdfb[:], in_=rstdf[:])
            bRf_p = pgen.tile([128, TOK], f32, tag="g", name="bRf")
            nc.tensor.matmul(bRf_p[:], ones1[:], rstdfb[:], start=True,
                             stop=True)
            bRf = ac.tile([128, TOK], bf16, tag="bR", bufs=2, name="bRfb")
            nc.vector.tensor_copy(out=bRf[:], in_=bRf_p[:])
            rTf_p = pacc.tile([128, 4], f32, tag="a", name="rTfp")
            for c, (a_, b_) in enumerate(tok_chunks):
                nc.tensor.matmul(rTf_p[0:b_ - a_, c:c + 1],
                                 rstdfb[:, a_:b_],
                                 ones1[0:1, 0:1], start=True, stop=True)
            rTf = ac.tile([128, 4], f32, tag="rT", bufs=2, name="rTf")
            nc.scalar.activation(out=rTf[:], in_=rTf_p[:], func=AF.Copy,
                                 scale=1.0 / spl)

            # V_pool + vsums -> pool exchange
            Vp = []
            for i, (a, b) in enumerate(tok_chunks):
                m = b - a
                ps = pacc.tile([128, D], f32, tag="a", name=f"pv{i}")
                for kp in range(2):
                    nc.tensor.matmul(ps[0:m, :], tf8[kp][:, :, a:b],
                                     pwkv8[:, kp, :, D:2 * D],
                                     start=(kp == 0), stop=False,
                                     perf_mode=DR)
                nc.tensor.matmul(ps[0:m, :], Sfb[:, a:b], prow[:, D:2 * D],
                                 start=False, stop=True)
                s = ac.tile([128, D], bf16, tag=f"V{i}", bufs=1,
                            name=f"pVb{i}")
                nc.scalar.activation(out=s[0:m, :], in_=ps[0:m, :],
                                     func=AF.Copy,
                                     scale=rTf[0:m, i:i + 1])
                Vp.append(s)
            PpA = ac.tile([128, 4], f32, tag="PA", bufs=2, name="PpA")
            for c in range(4):
                ps = pgen.tile([128, 1], f32, tag="g", name=f"pvs{c}")
                for j in range(3):
                    nc.tensor.matmul(ps[:], Vp[j][:, 128 * c:128 * (c + 1)],
                                     ones128[:], start=(j == 0), stop=(j == 2))
                nc.vector.tensor_copy(out=PpA[:, c:c + 1], in_=ps[:])
            pvfu = ac.tile([128, 4], f32, tag="vfu", bufs=2, name="pvfu")
            for c in range(4):
                ps = pgen.tile([128, 1], f32, tag="g", name=f"pvf{c}")
                nc.tensor.matmul(ps[:], Vp[3][0:FUS, 128 * c:128 * (c + 1)],
                                 ones128[0:FUS, :], start=True, stop=True)
                nc.vector.tensor_copy(out=pvfu[:, c:c + 1], in_=ps[:])
            pinP = dramp.tile([128, 4], f32, tag="pinA", bufs=2, name="pinP")
            nc.sync.dma_start(out=pinP[:], in_=PpA[:])
            RP = ac.tile([128, 4, 4], f32, tag="RA", bufs=2, name="RP")
            if use_cc:
                poutP = dramp.tile([4 * 128, 4], f32, tag="poutA", bufs=2,
                                   name="poutP")
                nc.gpsimd.collective_compute(
                    "AllGather", OP.bypass, replica_groups=rg,
                    ins=[pinP.opt()], outs=[poutP.opt()])
                nc.sync.dma_start(
                    out=RP[:], in_=poutP.rearrange("(r p) f -> p r f", r=4))
            else:
                nc.sync.dma_start(
                    out=RP[:],
                    in_=pinP.rearrange("(r p) f -> p r f", r=1)
                    .to_broadcast((128, 4, 4)))

            # uniform pooled vector u
            pvs = ac.tile([128, 4], f32, tag="vsb", bufs=2, name="pvs")
            nc.vector.tensor_reduce(out=pvs[:],
                                    in_=RP[:].rearrange("p r f -> p f r"),
                                    axis=mybir.AxisListType.X,
                                    op=OP.add)
            nc.vector.tensor_add(out=pvs[:], in0=pvs[:], in1=pvfu[:])
            pvsb8 = ac.tile([128, 4], f8, tag="vsb8", bufs=2,
                            name="pvsb8")
            nc.vector.tensor_copy(out=pvsb8[:], in_=pvs[:])
            pup = pgen.tile([128, 4], f32, tag="g", name="pup")
            for c in range(4):
                for kc in range(4):
                    nc.tensor.matmul(pup[:, c:c + 1],
                                     pwo8[:, kc // 2, kc % 2,
                                          128 * c:128 * (c + 1)],
                                     pvsb8[:, kc:kc + 1],
                                     start=(kc == 0), stop=(kc == 3))
            pus = ac.tile([128, 4], f32, tag="dus", bufs=2, name="pus")
            nc.scalar.activation(out=pus[:], in_=pup[:], func=AF.Copy,
                                 scale=1.0 / (spo * NALL))
            nc.sync.dma_start(out=out_u.rearrange("(c p) one -> p (c one)", c=4),
                              in_=pus[:])

            # fusion-key attention for return token 2
            kf = []
            for mc in range(4):
                ps = pgen.tile([128, FUS], f32, tag="g", name=f"pkf{mc}")
                for kp in range(2):
                    nc.tensor.matmul(ps[:],
                                     pwkv8[:, kp, :, 128 * mc:128 * (mc + 1)],
                                     tf8[kp][:, :, OWN:TOK],
                                     start=(kp == 0), stop=False,
                                     perf_mode=DR)
                nc.tensor.matmul(ps[:], prow[:, 128 * mc:128 * (mc + 1)],
                                 Sfb[:, OWN:TOK], start=False, stop=True)
                s = ac.tile([128, FUS], bf16, tag=f"kf{mc}", bufs=1,
                            name=f"kfb{mc}")
                nc.vector.tensor_mul(out=s[:], in0=ps[:],
                                     in1=bRf[:, OWN:TOK])
                kf.append(s)
            q2 = []
            for mc in range(4):
                s = ac.tile([128, 32], bf16, tag=f"qf{mc}", bufs=1,
                            name=f"q2b{mc}")
                nc.vector.memset(s[:, 1:32], 0.0)
                nc.vector.tensor_copy(out=s[:, 0:1], in_=pq2s[mc][:])
                q2.append(s)
            e2, l2 = [], []
            for t in range(2):
                sp = pgen.tile([128, FUS], f32, tag="g", name=f"ps2{t}")
                for i in range(4):
                    h = 4 * t + i
                    ch, base = h // 2, (h % 2) * 64
                    nc.tensor.matmul(sp[32 * i:32 * i + 32, :],
                                     q2[ch][base:base + 64, 0:32],
                                     kf[ch][base:base + 64, :],
                                     start=True, stop=True,
                                     tile_position=(base, 32 * i))
                e = ac.tile([128, FUS], bf16, tag=f"e2{t}", bufs=1,
                            name=f"e2{t}")
                la = ac.tile([128, 1], f32, tag=f"la{t}", bufs=2,
                             name=f"pla{t}")
                nc.scalar.activation(out=e[:], in_=sp[:], func=AF.Exp,
                                     scale=1.0 / spl, accum_out=la[:])
                e2.append(e)
                l2.append(la)
            e2T = []
            for t in range(2):
                pt = pgen.tile([FUS, 128], bf16, tag="g", name=f"pet{t}")
                nc.tensor.transpose(pt[:], e2[t][:], ident[:])
                s = ac.tile([FUS, 128], bf16, tag=f"e2T{t}", bufs=1,
                            name=f"e2Tb{t}")
                nc.vector.tensor_copy(out=s[:], in_=pt[:])
                e2T.append(s)
            of2 = []
            for t in range(2):
                acc = pacc.tile([128, 64], f32, tag="a", name=f"pacc2{t}")
                for i in range(4):
                    h = 4 * t + i
                    nc.tensor.matmul(acc[32 * i:32 * i + 32, :],
                                     e2T[t][:, 32 * i:32 * i + 32],
                                     Vp[3][0:FUS, DH * h:DH * (h + 1)],
                                     start=True, stop=True,
                                     tile_position=(0, 32 * i))
                li = ac.tile([128, 1], f32, tag="linv", bufs=2,
                             name=f"pli{t}")
                nc.vector.reciprocal(out=li[:], in_=l2[t][:])
                s = ac.tile([128, 64], bf16, tag=f"of{t}", bufs=1,
                            name=f"pof{t}")
                nc.vector.tensor_scalar_mul(out=s[:], in0=acc[:],
                                            scalar1=li[:])
                of2.append(s)
            ofI2 = [ac.tile([128, 1], f8, tag=f"ofI2{kc}", bufs=1,
                            name=f"ofI2{kc}") for kc in range(4)]
            for kc in range(4):
                pt = pacc.tile([128, 32], bf16, tag="a", name=f"ofI2p{kc}")
                for half in range(2):
                    h = 2 * kc + half
                    t, i = h // 4, h % 4
                    nc.tensor.transpose(
                        pt[64 * half:64 * half + 64, :],
                        of2[t][32 * i:32 * i + 32, :],
                        ident[32 * i:32 * i + 32, 32 * i:32 * i + 32],
                        tile_position=(32 * i, 64 * half))
                nc.vector.tensor_copy(out=ofI2[kc][:], in_=pt[:, 0:1])
            P2 = pacc.tile([1, D], f32, tag="a", name="P2")
            for kc in range(4):
                nc.tensor.matmul(P2[:], ofI2[kc][:],
                                 pwo8[:, kc // 2, kc % 2, :],
                                 start=(kc == 0), stop=(kc == 3))
            p2s = ac.tile([1, D], f32, tag="p2s", bufs=1, name="p2s")
            nc.scalar.activation(out=p2s[:], in_=P2[:], func=AF.Copy,
                                 scale=1.0 / spo)
            nc.sync.dma_start(out=out_f, in_=p2s[:])

    nc.compile()
    _built[key] = nc
    return nc


def _prep_inputs(inputs):
    """Host-side prep: fold gains, pick fp8 scales, pack weights per layer."""
    I = {k: np.asarray(v, np.float64) for k, v in inputs.items()}
    f32 = np.float32

    def col(x):
        return np.ascontiguousarray(np.asarray(x, f32).reshape(-1, 1))

    scale_dh = DH ** -0.5
    g = I["layers_attn_g"][:, :, None]
    wqf = I["layers_wq"] * scale_dh * g            # [L, 512, 512]
    wkvf = I["layers_wkv"] * g                     # [L, 512, 1024]
    wof = I["layers_wo"]                           # [L, 512, 512]
    w1f = I["layers_ff_w1"] * I["layers_ff_g"][:, :, None]  # [L, 512, 2730]
    w2f = I["layers_ff_w2"]                        # [L, 1365, 512]

    sq = _pow2_scale(wqf)
    skv = _pow2_scale(wkvf)
    so = _pow2_scale(wof)
    s1x = 16.0
    s1g = _pow2_scale(w1f[:, :, IFF:])
    s2 = _pow2_scale(w2f)
    pkvf = I["pool_wkv"] * I["final_g"][:, None]
    spl = _pow2_scale(pkvf)
    spo = _pow2_scale(I["pool_wo"])
    sef = {}
    for mod in ("rna", "atac"):
        ewf = I[f"{mod}_w"] * I[f"{mod}_ln1_g"][:, None]
        sef[mod] = (ewf, _pow2_scale(ewf))
    se = min(sef["rna"][1], sef["atac"][1])
    scales = (sq, skv, so, s1x, s1g, s2, se, spl, spo)

    # packed per-layer fp8 weights
    wpk = np.zeros((DEPTH, 128, FCOLS), F8)
    rows = np.zeros((DEPTH, 3, D), np.float64)
    for l in range(DEPTH):
        wpk[l, :, SEG_WQ:SEG_WKV] = _pack_pairs(wqf[l], sq).reshape(128, -1)
        wpk[l, :, SEG_WKV:SEG_WO] = _pack_pairs(wkvf[l], skv).reshape(128, -1)
        wpk[l, :, SEG_WO:SEG_W1] = _pack_pairs(wof[l], so).reshape(128, -1)
        w1p = np.zeros((D, 2 * IFFP), np.float64)
        w1p[:, 0:IFF] = w1f[l][:, 0:IFF] * s1x
        w1p[:, IFFP:IFFP + IFF] = w1f[l][:, IFF:] * s1g
        wpk[l, :, SEG_W1:SEG_W2] = _pack_pairs(w1p, 1.0).reshape(128, -1)
        w2p = np.zeros((IFF2, D), np.float64)
        w2p[0:IFF, :] = w2f[l] * s2
        wpk[l, :, SEG_W2:FCOLS] = _pack_pairs(w2p, 1.0).reshape(128, -1)
        # rank-1 rows: -scale*colsum/64
        rows[l, 0] = -skv * wkvf[l][:, 0:D].sum(0) / 64     # wksum
        rows[l, 1] = -skv * wkvf[l][:, D:2 * D].sum(0) / 64  # wvsum
        rows[l, 2] = -sq * wqf[l].sum(0) / 64                # wqsum
    rows_b = np.ascontiguousarray(rows.reshape(1, -1)).astype(BF)

    prow = np.zeros((1, 2 * D), np.float64)
    prow[0, 0:D] = -spl * pkvf[:, 0:D].sum(0) / 64
    prow[0, D:2 * D] = -spl * pkvf[:, D:2 * D].sum(0) / 64

    pwkv8 = _pack_pairs(pkvf, spl)
    pwo8 = _pack_pairs(I["pool_wo"], spo)

    # pool query for return token 2 (host, tiny)
    ret = I["return_tokens"].astype(f32)
    gp = I["pool_g"].astype(f32)
    mu = ret.mean(-1, keepdims=True)
    var = ((ret - mu) ** 2).mean(-1, keepdims=True)
    retn = (ret - mu) / np.sqrt(var + 1e-5) * gp
    q2 = (retn[2] @ I["pool_wq"].astype(f32)) * scale_dh

    fus8 = I["fusion_tokens"].T.reshape(4, 128, FUS).transpose(1, 0, 2)

    shared = {
        "wpk": wpk,
        "rows": rows_b,
        "prow": prow.astype(BF),
        "pwkv8": pwkv8,
        "pwo8": pwo8,
        "pool_q2": col(q2),
        "fus_t": np.ascontiguousarray(fus8).astype(np.float32),
    }

    in_maps = []
    for c in range(N_CORES):
        b, q = c // 4, c % 4
        mod = "rna" if q < 2 else "atac"
        x = I[mod][b, (q % 2) * OWN:(q % 2 + 1) * OWN, :]   # [384, 1024]
        m = dict(shared)
        # x8: [128, 4kp, 2sub, 384]
        xT = np.ascontiguousarray(x.T)                      # [1024, 384]
        m["x8"] = _pack_pairs(xT, 1.0)
        ewf = sef[mod][0]
        m["ew8"] = _pack_pairs(ewf, se)
        m["emb_b"] = col(I[f"{mod}_b"]
                         + I[f"{mod}_ln1_b"] @ I[f"{mod}_w"])
        m["erow"] = (-se * ewf.sum(0) / 128).reshape(1, -1).astype(BF)
        m["eln2_g"] = col(I[f"{mod}_ln2_g"])
        m["eln2_b"] = col(I[f"{mod}_ln2_b"])
        in_maps.append(m)
    return in_maps, ret.astype(f32), scales


def kernel(**inputs):
    from concourse import bass_utils
    in_maps, ret, scales = _prep_inputs(inputs)
    nc = build(num_devices=N_CORES, use_cc=True, scales=scales)
    res = bass_utils.run_bass_kernel_spmd(nc, in_maps,
                                          core_ids=list(range(N_CORES)))
    out = np.zeros((B, 3, D), np.float32)
    for b in range(2):
        r = res.results[4 * b]
        u = r["out_u"][:, 0]
        f = r["out_f"][0]
        out[b, 0] = u + ret[0]
        out[b, 1] = u + ret[1]
        out[b, 2] = f + ret[2]
    return out


# revision 23
# speedup vs baseline: 1.5731x; 1.0151x over previous
"""BioZorro sparse-attention kernel for 8 Trainium2 NeuronCores.

Sharding: 8 cores = 2 batches x 4 token-quarters (384 own tokens each).
The zorro mask makes all non-fusion query rows fully masked -> uniform
softmax -> their attention output is mean(V); only the 16 fusion tokens
attend (over the 1536 non-fusion keys). Cross-core data per layer is two
small AllGathers: (A) V column sums (2KB) issued early, (B) fusion
flash-softmax partials (66KB), plus one tiny AllGather for pooling.

Compute layout: residual stream feature-major (tok^T [512, 400] f32).
All heavy matmuls run in fp8e4 DoubleRow (contract 256/instr, 2x rate):
activations are cast to paired [128,2,T] tiles; weights are host-packed
into one fp8 buffer per layer (single DMA, double-buffered). LayerNorms
are folded into consumers: raw-cast -> matmul immediately; the -mu
correction enters PSUM as a rank-1 matmul (host-precomputed column sums
x the device S row); rstd is applied at PSUM eviction (column-broadcast
or per-token scalars). Per-tensor power-of-2 fp8 scales are descaled via
free immediate-scale slots (exp/gelu/copy activations).
"""
import sys
sys.path.insert(0, "/opt/trn_rl_repo")
import numpy as np
import ml_dtypes

BF = ml_dtypes.bfloat16
F8 = ml_dtypes.float8_e4m3
OWN, FUS, TOK = 384, 16, 400
D, RIN, H, DH, IFF, DEPTH = 512, 1024, 8, 64, 1365, 4
NALL = 1552
B, NR, NA = 2, 768, 768
N_CORES = 8
IFFP = 1408           # x/gate block padding (11 x 128)
IFF2 = 1536           # FF2 contract padding (6 x 256)

# fp8 packed-weight segment offsets (cols in the per-layer [128, FCOLS])
SEG_WQ = 0            # [2kp][2sub][512]
SEG_WKV = 2048        # [2kp][2sub][1024]
SEG_WO = SEG_WKV + 4096   # [2kp][2sub][512]
SEG_W1 = SEG_WO + 2048    # [2kp][2sub][2*1408]
SEG_W2 = SEG_W1 + 11264   # [6jp][2sub][512]
FCOLS = SEG_W2 + 6144

_built = {}


def _pow2_scale(w, target=120.0):
    m = float(np.abs(w).max())
    if m <= 0:
        return 1.0
    return float(2.0 ** np.floor(np.log2(target / m)))


def _pack_pairs(w, scale):
    """[K, N] f64 -> [128, K//256, 2, N] fp8 DoubleRow lhsT layout."""
    K, N = w.shape
    assert K % 256 == 0
    out = (w * scale).astype(F8).reshape(K // 128, 128, N)
    # chunk k = rows 128k..128k+128; pair kp = (2kp, 2kp+1)
    out = out.transpose(1, 0, 2).reshape(128, K // 256, 2, N)
    return np.ascontiguousarray(out)


def build(num_devices=8, use_cc=True, scales=None):
    key = (num_devices, use_cc, scales)
    if key in _built:
        return _built[key]
    import concourse.tile as tile
    from concourse import bacc, mybir
    from concourse.masks import make_identity

    # Force Exp to resolve to natural_log_exp_and_others so Ln/Exp/Square
    # live in one ACT table set (Gelu still needs its own set; those two
    # swaps per layer are prefetched off the critical path with dummy ops).
    if not getattr(bacc, "_act_tables_patched", False):
        _orig_gat = bacc.get_activation_tables

        def _patched_gat(arch):
            tabs = _orig_gat(arch)
            exp_t = mybir.ActivationFunctionType.Exp
            for nm, fns in tabs.items():
                if nm != "natural_log_exp_and_others":
                    fns.discard(exp_t)
            return tabs

        bacc.get_activation_tables = _patched_gat
        bacc._act_tables_patched = True

    sq, skv, so, s1x, s1g, s2, se, spl, spo = scales
    f32 = mybir.dt.float32
    bf16 = mybir.dt.bfloat16
    f8 = mybir.dt.float8e4
    AF = mybir.ActivationFunctionType
    OP = mybir.AluOpType
    DR = mybir.MatmulPerfMode.DoubleRow

    nc = bacc.Bacc("TRN2", target_bir_lowering=False, debug=False,
                   enable_asserts=True, num_devices=num_devices)

    def din(name, shape, dt=f32):
        return nc.dram_tensor(name, shape, dt, kind="ExternalInput").ap()

    x8_t = din("x8", [128, 4, 2, OWN], f8)
    ew8_t = din("ew8", [128, 4, 2, D], f8)
    ebias_t = din("emb_b", [D, 1])
    eg2_t = din("eln2_g", [D, 1])
    eb2_t = din("eln2_b", [D, 1])
    fus_t = din("fus_t", [128, 4, FUS], f32)
    wpk_t = din("wpk", [DEPTH, 128, FCOLS], f8)
    # host rank-1 rows: per layer [wksum, wvsum, wqsum] each [512]
    rows_t = din("rows", [1, DEPTH * 3 * D], bf16)
    erow_t = din("erow", [1, D], bf16)        # embed -se*colsum(ew')/128
    prow_t = din("prow", [1, 2 * D], bf16)    # pool [pwksum, pwvsum]
    pwkv8_t = din("pwkv8", [128, 2, 2, 2 * D], f8)
    pwo8_t = din("pwo8", [128, 2, 2, D], f8)
    pq2_t = din("pool_q2", [D, 1])
    out_u = nc.dram_tensor("out_u", [D, 1], f32, kind="ExternalOutput").ap()
    out_f = nc.dram_tensor("out_f", [1, D], f32, kind="ExternalOutput").ap()

    with tile.TileContext(nc) as tc:
        with tc.tile_pool(name="cst", bufs=1) as cst, \
             tc.tile_pool(name="wp", bufs=2) as wp, \
             tc.tile_pool(name="ac", bufs=2) as ac, \
             tc.tile_pool(name="pgen", bufs=4, space="PSUM") as pgen, \
             tc.tile_pool(name="pacc", bufs=4, space="PSUM") as pacc, \
             tc.tile_pool(name="dramp", bufs=2, space="DRAM") as dramp:

            ident = cst.tile([128, 128], bf16, name="ident")
            make_identity(nc, ident[:])
            ones128 = cst.tile([128, 1], bf16, name="ones128")
            nc.vector.memset(ones128[:], 1.0)
            ones1 = cst.tile([1, 128], bf16, name="ones1")
            nc.vector.memset(ones1[:], 1.0)
            epsc = cst.tile([128, 1], f32, name="epsc")
            nc.vector.memset(epsc[:], 1e-5)
            oi512 = cst.tile([128, 1], bf16, name="oi512")
            nc.vector.memset(oi512[:], 1.0 / 512)
            ones8p = cst.tile([128, 1], f8, name="ones8p")
            nc.vector.memset(ones8p[:], 0.125)
            cinvf = 1.0 / (s1x * s2)

            def load_cols(dram_ap, n, tag, rows=128):
                ts = []
                for c in range(n):
                    t = wp.tile([rows, 1], f32, tag=f"{tag}{c}", bufs=1,
                                name=f"{tag}{c}")
                    nc.sync.dma_start(out=t[:],
                                      in_=dram_ap[rows * c:rows * (c + 1), :])
                    ts.append(t)
                return ts

            # ---------- one-time loads ----------
            x8 = ac.tile([128, 4, 2, OWN], f8, tag="x8", bufs=1, name="x8")
            nc.sync.dma_start(out=x8[:], in_=x8_t)
            ew8 = wp.tile([128, 4, 2, D], f8, tag="ew8", bufs=1, name="ew8")
            nc.sync.dma_start(out=ew8[:], in_=ew8_t)
            rows = wp.tile([1, DEPTH * 3 * D], bf16, tag="rows", bufs=1,
                           name="rows")
            nc.sync.dma_start(out=rows[:], in_=rows_t)
            erow = wp.tile([1, D], bf16, tag="erow", bufs=1, name="erow")
            nc.sync.dma_start(out=erow[:], in_=erow_t)
            ebs = load_cols(ebias_t, 4, "ebias")
            eg2s = load_cols(eg2_t, 4, "eg2")
            eb2s = load_cols(eb2_t, 4, "eb2")

            # layer-0 weights DMA starts immediately
            wts = []
            w0 = wp.tile([128, FCOLS], f8, tag="wpk", bufs=2, name="wpk0")
            nc.sync.dma_start(out=w0[:], in_=wpk_t[0])

            # dummy exp to preload the nlexp ACT table during initial DMAs
            dtab = ac.tile([1, 1], f32, tag="dtab", bufs=2, name="dtab")
            nc.scalar.activation(out=dtab[:], in_=epsc[0:1, :], func=AF.Exp)

            # ---------- embed ----------
            # stats over raw x8 (LN1 folded into the embed matmul)
            Se = pgen.tile([1, OWN], f32, tag="g", name="Se")
            for i in range(8):
                nc.tensor.matmul(Se[:], ones8p[:], x8[:, i // 2, i % 2, :],
                                 start=(i == 0), stop=(i == 7))
            Seb = ac.tile([1, OWN], bf16, tag="rowb", bufs=4, name="Seb")
            nc.scalar.activation(out=Seb[:], in_=Se[:], func=AF.Copy)
            xsqe = []
            for kp in range(4):
                t = ac.tile([128, 2, OWN], bf16, tag="xsq", bufs=4,
                            name=f"xsqe{kp}")
                nc.scalar.activation(out=t[:], in_=x8[:, kp, :, :],
                                     func=AF.Square)
                xsqe.append(t)
            Qe = pgen.tile([1, OWN], f32, tag="g", name="Qe")
            oi1024 = cst.tile([128, 1], bf16, name="oi1024")
            nc.vector.memset(oi1024[:], 1.0 / 1024)
            for i in range(8):
                nc.tensor.matmul(Qe[:], oi1024[:], xsqe[i // 2][:, i % 2, :],
                                 start=(i == 0), stop=(i == 7))
            m2e = ac.tile([1, OWN], f32, tag="rowf", bufs=6, name="m2e")
            nc.scalar.activation(out=m2e[:], in_=Se[:], func=AF.Square,
                                 scale=1.0 / 128)
            vare = ac.tile([1, OWN], f32, tag="rowf", bufs=6, name="vare")
            nc.vector.tensor_sub(out=vare[:], in0=Qe[:], in1=m2e[:])
            rstde = ac.tile([1, OWN], f32, tag="rowf", bufs=6, name="rstde")
            nc.scalar.activation(out=rstde[:], in_=vare[:], func=AF.Ln,
                                 bias=epsc[0:1, :])
            nc.scalar.activation(out=rstde[:], in_=rstde[:], func=AF.Exp,
                                 scale=-0.5)
            rstdeb = ac.tile([1, OWN], bf16, tag="rowb", bufs=4, name="rstdeb")
            nc.scalar.activation(out=rstdeb[:], in_=rstde[:], func=AF.Copy,
                                 scale=1.0 / se)
            bRe_p = pgen.tile([128, OWN], f32, tag="g", name="bRe")
            nc.tensor.matmul(bRe_p[:], ones1[:], rstdeb[:], start=True,
                             stop=True)
            bRe = ac.tile([128, OWN], bf16, tag="bR", bufs=2, name="bReb")
            nc.vector.tensor_copy(out=bRe[:], in_=bRe_p[:])

            hb = []
            for mc in range(4):
                ps = pgen.tile([128, OWN], f32, tag="g", name=f"embp{mc}")
                for kp in range(4):
                    nc.tensor.matmul(ps[:], ew8[:, kp, :, 128 * mc:128 * (mc + 1)],
                                     x8[:, kp, :, :], start=(kp == 0),
                                     stop=False, perf_mode=DR)
                nc.tensor.matmul(ps[:], erow[:, 128 * mc:128 * (mc + 1)],
                                 Seb[:], start=False, stop=True)
                t1 = ac.tile([128, OWN], bf16, tag="embt", bufs=2,
                             name=f"embt{mc}")
                nc.vector.tensor_mul(out=t1[:], in0=ps[:], in1=bRe[:])
                t2 = ac.tile([128, OWN], bf16, tag=f"hb{mc}", bufs=1,
                             name=f"hb{mc}")
                nc.vector.tensor_scalar_add(out=t2[:], in0=t1[:],
                                            scalar1=ebs[mc][:])
                hb.append(t2)

            # embed LN2 (explicit normalize into f32 tok)
            S2e = pgen.tile([1, OWN], f32, tag="g", name="S2e")
            for c in range(4):
                nc.tensor.matmul(S2e[:], oi512[:], hb[c][:],
                                 start=(c == 0), stop=(c == 3))
            x2e = []
            for c in range(4):
                t = ac.tile([128, OWN], bf16, tag="xsq", bufs=4,
                            name=f"x2e{c}")
                nc.vector.tensor_mul(out=t[:], in0=hb[c][:], in1=hb[c][:])
                x2e.append(t)
            Q2e = pgen.tile([1, OWN], f32, tag="g", name="Q2e")
            for c in range(4):
                nc.tensor.matmul(Q2e[:], oi512[:], x2e[c][:],
                                 start=(c == 0), stop=(c == 3))
            m22 = ac.tile([1, OWN], f32, tag="rowf", bufs=6, name="m22")
            nc.scalar.activation(out=m22[:], in_=S2e[:], func=AF.Square)
            var2 = ac.tile([1, OWN], f32, tag="rowf", bufs=6, name="var2e")
            nc.vector.tensor_sub(out=var2[:], in0=Q2e[:], in1=m22[:])
            rstd2e = ac.tile([1, OWN], f32, tag="rowf", bufs=6, name="rstd2e")
            nc.scalar.activation(out=rstd2e[:], in_=var2[:], func=AF.Ln,
                                 bias=epsc[0:1, :])
            nc.scalar.activation(out=rstd2e[:], in_=rstd2e[:], func=AF.Exp,
                                 scale=-0.5)
            pr2 = ac.tile([1, 2 * OWN], bf16, tag="rowb2", bufs=2, name="pr2e")
            nc.vector.tensor_copy(out=pr2[:, 0:OWN], in_=rstd2e[:])
            mre = ac.tile([1, OWN], f32, tag="rowf", bufs=6, name="mre")
            nc.vector.tensor_mul(out=mre[:], in0=S2e[:], in1=rstd2e[:])
            nc.vector.tensor_copy(out=pr2[:, OWN:2 * OWN], in_=mre[:])
            bR2e_p = pgen.tile([128, OWN], f32, tag="g", name="bR2e")
            nc.tensor.matmul(bR2e_p[:], ones1[:], pr2[:, 0:OWN], start=True,
                             stop=True)
            bR2e = ac.tile([128, OWN], bf16, tag="bR", bufs=2, name="bR2eb")
            nc.vector.tensor_copy(out=bR2e[:], in_=bR2e_p[:])
            bM2e_p = pgen.tile([128, OWN], f32, tag="g", name="bM2e")
            nc.tensor.matmul(bM2e_p[:], ones1[:], pr2[:, OWN:2 * OWN],
                             start=True, stop=True)
            bM2e = ac.tile([128, OWN], bf16, tag="bM", bufs=2, name="bM2eb")
            nc.vector.tensor_copy(out=bM2e[:], in_=bM2e_p[:])

            tok = [ac.tile([128, TOK], f32, tag=f"tok{c}", bufs=1,
                           name=f"tok{c}") for c in range(4)]
            for c in range(4):
                t1 = ac.tile([128, OWN], bf16, tag="embt", bufs=2,
                             name=f"eln{c}")
                nc.vector.tensor_mul(out=t1[:], in0=hb[c][:], in1=bR2e[:])
                nc.vector.tensor_sub(out=t1[:], in0=t1[:], in1=bM2e[:])
                nc.vector.tensor_scalar(out=tok[c][:, 0:OWN], in0=t1[:],
                                        scalar1=eg2s[c][:], scalar2=eb2s[c][:],
                                        op0=OP.mult, op1=OP.add)
                nc.sync.dma_start(out=tok[c][:, OWN:TOK], in_=fus_t[:, c, :])

            tok_chunks = [(0, 128), (128, 256), (256, 384), (384, 400)]
            rg = [[0, 1, 2, 3], [4, 5, 6, 7]]
            psO_prev = None

            # rank-1 row views per layer
            def lrow(l, which):
                base = (l * 3 + which) * D
                return rows[:, base:base + D]

            # ---------- layers ----------
            for l in range(DEPTH):
                wT = w0 if l == 0 else wp.tile([128, FCOLS], f8, tag="wpk",
                                               bufs=2, name=f"wpk{l}")
                if l > 0:
                    nc.sync.dma_start(out=wT[:], in_=wpk_t[l])
                wq8 = wT[:, SEG_WQ:SEG_WKV].rearrange(
                    "p (kp s o) -> p kp s o", kp=2, s=2)
                wkv8 = wT[:, SEG_WKV:SEG_WO].rearrange(
                    "p (kp s o) -> p kp s o", kp=2, s=2)
                wo8 = wT[:, SEG_WO:SEG_W1].rearrange(
                    "p (kp s o) -> p kp s o", kp=2, s=2)
                w18 = wT[:, SEG_W1:SEG_W2].rearrange(
                    "p (kp s o) -> p kp s o", kp=2, s=2)
                w28 = wT[:, SEG_W2:FCOLS].rearrange(
                    "p (jp s o) -> p jp s o", jp=6, s=2)

                # --- LN1 prep: residual + fp8 raw cast ---
                tr8 = [ac.tile([128, 2, TOK], f8, tag=f"tr8{kp}", bufs=1,
                               name=f"tr8_{kp}") for kp in range(2)]
                for c in range(4):
                    if psO_prev is not None:
                        t = ac.tile([128, TOK], bf16, tag="psot", bufs=2,
                                    name=f"psot{c}")
                        nc.scalar.activation(out=t[:], in_=psO_prev[c][:],
                                             func=AF.Copy, scale=cinvf)
                        nc.vector.tensor_add(out=tok[c][:], in0=tok[c][:],
                                             in1=t[:])
                    nc.vector.tensor_copy(out=tr8[c // 2][:, c % 2, :],
                                          in_=tok[c][:])
                psO_prev = None

                # --- stats1 (rank-1 style folded LN) ---
                S1 = pgen.tile([1, TOK], f32, tag="g", name="S1")
                for i in range(4):
                    nc.tensor.matmul(S1[:], ones8p[:], tr8[i // 2][:, i % 2, :],
                                     start=(i == 0), stop=(i == 3))
                S1b = ac.tile([1, TOK], bf16, tag="rowb", bufs=4, name="S1b")
                nc.scalar.activation(out=S1b[:], in_=S1[:], func=AF.Copy)
                m2 = ac.tile([1, TOK], f32, tag="rowf", bufs=6, name="m2")
                nc.scalar.activation(out=m2[:], in_=S1[:], func=AF.Square,
                                     scale=1.0 / 64)
                xsq = []
                for kp in range(2):
                    t = ac.tile([128, 2, TOK], bf16, tag="xsq", bufs=4,
                                name=f"xsq{kp}")
                    if kp == 0:
                        nc.vector.tensor_mul(out=t[:], in0=tr8[kp][:],
                                             in1=tr8[kp][:])
                    else:
                        nc.scalar.activation(out=t[:], in_=tr8[kp][:],
                                             func=AF.Square)
                    xsq.append(t)

                # --- V raw (+ early evict, rstd folded into consumers) ---
                V = []
                for i, (a, b) in enumerate(tok_chunks):
                    m = b - a
                    ps = pacc.tile([128, D], f32, tag="a", name=f"v{i}")
                    for kp in range(2):
                        nc.tensor.matmul(ps[0:m, :], tr8[kp][:, :, a:b],
                                         wkv8[:, kp, :, D:2 * D],
                                         start=(kp == 0), stop=False,
                                         perf_mode=DR)
                    nc.tensor.matmul(ps[0:m, :], S1b[:, a:b],
                                     lrow(l, 1), start=False, stop=True)
                    sV = ac.tile([128, D], bf16, tag=f"V{i}", bufs=1,
                                 name=f"Vb{i}")
                    nc.scalar.activation(out=sV[0:m, :], in_=ps[0:m, :],
                                         func=AF.Copy, scale=1.0 / skv)
                    V.append(sV)

                # --- K^T, Qf raw matmuls ---
                ktp, qfp = [], []
                for mc in range(4):
                    ps = pacc.tile([128, OWN], f32, tag="a", name=f"kt{mc}")
                    for kp in range(2):
                        nc.tensor.matmul(ps[:],
                                         wkv8[:, kp, :, 128 * mc:128 * (mc + 1)],
                                         tr8[kp][:, :, 0:OWN],
                                         start=(kp == 0), stop=False,
                                         perf_mode=DR)
                    nc.tensor.matmul(ps[:], lrow(l, 0)[:, 128 * mc:128 * (mc + 1)],
                                     S1b[:, 0:OWN], start=False, stop=True)
                    ktp.append(ps)
                # --- stats1 tail ---
                Q1 = pgen.tile([1, TOK], f32, tag="g", name="Q1")
                for i in range(4):
                    nc.tensor.matmul(Q1[:], oi512[:], xsq[i // 2][:, i % 2, :],
                                     start=(i == 0), stop=(i == 3))
                var = ac.tile([1, TOK], f32, tag="rowf", bufs=6, name="var")
                nc.vector.tensor_sub(out=var[:], in0=Q1[:], in1=m2[:])
                rstd = ac.tile([1, TOK], f32, tag="rowf", bufs=6, name="rstd")
                nc.scalar.activation(out=rstd[:], in_=var[:], func=AF.Ln,
                                     bias=epsc[0:1, :])
                nc.scalar.activation(out=rstd[:], in_=rstd[:], func=AF.Exp,
                                     scale=-0.5)
                rstdb = ac.tile([1, TOK], bf16, tag="rowb", bufs=4,
                                name="rstdb")
                nc.vector.tensor_copy(out=rstdb[:], in_=rstd[:])
                bR1_p = pgen.tile([128, TOK], f32, tag="g", name="bR1")
                nc.tensor.matmul(bR1_p[:], ones1[:], rstdb[:], start=True,
                                 stop=True)
                bR1 = ac.tile([128, TOK], bf16, tag="bR", bufs=2, name="bR1b")
                nc.vector.tensor_copy(out=bR1[:], in_=bR1_p[:])
                rT_p = pgen.tile([128, 4], f32, tag="g", name="rTp")
                for c, (a_, b_) in enumerate(tok_chunks):
                    nc.tensor.matmul(rT_p[0:b_ - a_, c:c + 1],
                                     rstdb[:, a_:b_],
                                     ones1[0:1, 0:1], start=True, stop=True)
                rT = ac.tile([128, 4], bf16, tag="rT", bufs=2, name="rT")
                nc.vector.tensor_copy(out=rT[:], in_=rT_p[:])
                rTs = ac.tile([128, 4], f32, tag="rTs", bufs=2, name="rTs")
                nc.vector.tensor_copy(out=rTs[:], in_=rT_p[:])

                # --- Qf raw (after stats-tail matmuls; "a" slots free
                # once kt evictions land) ---
                for mc in range(4):
                    ps = pacc.tile([128, FUS], f32, tag="a", name=f"qf{mc}")
                    for kp in range(2):
                        nc.tensor.matmul(ps[:],
                                         wq8[:, kp, :, 128 * mc:128 * (mc + 1)],
                                         tr8[kp][:, :, OWN:TOK],
                                         start=(kp == 0), stop=False,
                                         perf_mode=DR)
                    nc.tensor.matmul(ps[:], lrow(l, 2)[:, 128 * mc:128 * (mc + 1)],
                                     S1b[:, OWN:TOK], start=False, stop=True)
                    qfp.append(ps)

                # --- vsums (rstd via rhs) -> exchange A ---
                PA = ac.tile([128, 4], f32, tag="PA", bufs=2, name="PA")
                for c in range(4):
                    ps = pgen.tile([128, 1], f32, tag="g", name=f"vs{c}")
                    for j in range(3):
                        nc.tensor.matmul(ps[:], V[j][:, 128 * c:128 * (c + 1)],
                                         rT[:, j:j + 1], start=(j == 0),
                                         stop=(j == 2))
                    nc.vector.tensor_copy(out=PA[:, c:c + 1], in_=ps[:])
                vfu = ac.tile([128, 4], f32, tag="vfu", bufs=2, name="vfu")
                for c in range(4):
                    ps = pgen.tile([128, 1], f32, tag="g", name=f"vf{c}")
                    nc.tensor.matmul(ps[:], V[3][0:FUS, 128 * c:128 * (c + 1)],
                                     rT[0:FUS, 3:4], start=True, stop=True)
                    nc.vector.tensor_copy(out=vfu[:, c:c + 1], in_=ps[:])
                pinA = dramp.tile([128, 4], f32, tag="pinA", bufs=2,
                                  name="pinA")
                nc.sync.dma_start(out=pinA[:], in_=PA[:])
                RA = ac.tile([128, 4, 4], f32, tag="RA", bufs=2, name="RA")
                if use_cc:
                    poutA = dramp.tile([4 * 128, 4], f32, tag="poutA", bufs=2,
                                       name="poutA")
                    nc.gpsimd.collective_compute(
                        "AllGather", OP.bypass, replica_groups=rg,
                        ins=[pinA.opt()], outs=[poutA.opt()])
                    nc.sync.dma_start(
                        out=RA[:], in_=poutA.rearrange("(r p) f -> p r f", r=4))
                else:
                    nc.sync.dma_start(
                        out=RA[:],
                        in_=pinA.rearrange("(r p) f -> p r f", r=1)
                        .to_broadcast((128, 4, 4)))

                # --- kt/qf evictions (x rstd columns) ---
                kt = []
                for mc in range(4):
                    sK = ac.tile([128, OWN], bf16, tag=f"kt{mc}", bufs=1,
                                 name=f"ktb{mc}")
                    nc.vector.tensor_mul(out=sK[:], in0=ktp[mc][:],
                                         in1=bR1[:, 0:OWN])
                    kt.append(sK)
                qf = []
                for mc in range(4):
                    sQ = ac.tile([128, 32], bf16, tag=f"qf{mc}", bufs=1,
                                 name=f"qfb{mc}")
                    nc.vector.memset(sQ[:, FUS:32], 0.0)
                    nc.vector.tensor_mul(out=sQ[:, 0:FUS], in0=qfp[mc][:],
                                         in1=bR1[:, OWN:TOK])
                    qf.append(sQ)

                # --- scores + exp ---
                E, lacc = [], []
                for t in range(2):
                    sp = pgen.tile([128, OWN], f32, tag="g", name=f"sp{t}")
                    for i in range(4):
                        h = 4 * t + i
                        ch, base = h // 2, (h % 2) * 64
                        nc.tensor.matmul(sp[32 * i:32 * i + 32, :],
                                         qf[ch][base:base + 64, 0:32],
                                         kt[ch][base:base + 64, :],
                                         start=True, stop=True,
                                         tile_position=(base, 32 * i))
                    e = ac.tile([128, OWN], bf16, tag=f"e{t}", bufs=1,
                                name=f"e{t}")
                    la = ac.tile([128, 1], f32, tag=f"la{t}", bufs=2,
                                 name=f"la{t}")
                    nc.scalar.activation(out=e[:], in_=sp[:], func=AF.Exp,
                                         scale=1.0 / (sq * skv),
                                         accum_out=la[:])
                    E.append(e)
                    lacc.append(la)

                # --- E^T + ACC -> exchange B ---
                ET = [[None] * 3 for _ in range(2)]
                for t in range(2):
                    for j in range(3):
                        pt = pgen.tile([128, 128], bf16, tag="g",
                                       name=f"et{t}{j}")
                        nc.tensor.transpose(pt[:], E[t][:, 128 * j:128 * (j + 1)],
                                            ident[:])
                        s = ac.tile([128, 128], bf16, tag=f"ET{t}{j}", bufs=1,
                                    name=f"ETb{t}{j}")
                        nc.vector.tensor_scalar_mul(out=s[:], in0=pt[:],
                                                    scalar1=rTs[:, j:j + 1])
                        ET[t][j] = s
                PB = ac.tile([128, 130], f32, tag="PB", bufs=2, name="PB")
                nc.vector.tensor_copy(out=PB[:, 0:1], in_=lacc[0][:])
                nc.vector.tensor_copy(out=PB[:, 1:2], in_=lacc[1][:])
                for t in range(2):
                    acc = pacc.tile([128, 64], f32, tag="a", name=f"acc{t}")
                    for i in range(4):
                        h = 4 * t + i
                        for j in range(3):
                            nc.tensor.matmul(acc[32 * i:32 * i + 32, :],
                                             ET[t][j][:, 32 * i:32 * i + 32],
                                             V[j][:, DH * h:DH * (h + 1)],
                                             start=(j == 0), stop=(j == 2),
                                             tile_position=(0, 32 * i))
                    nc.vector.tensor_copy(out=PB[:, 2 + 64 * t:66 + 64 * t],
                                          in_=acc[:])
                pinB = dramp.tile([128, 130], f32, tag="pinB", bufs=2,
                                  name="pinB")
                nc.sync.dma_start(out=pinB[:], in_=PB[:])
                RB = ac.tile([128, 4, 130], f32, tag="RB", bufs=2, name="RB")
                if use_cc:
                    poutB = dramp.tile([4 * 128, 130], f32, tag="poutB",
                                       bufs=2, name="poutB")
                    nc.gpsimd.collective_compute(
                        "AllGather", OP.bypass, replica_groups=rg,
                        ins=[pinB.opt()], outs=[poutB.opt()])
                    nc.sync.dma_start(
                        out=RB[:], in_=poutB.rearrange("(r p) f -> p r f", r=4))
                else:
                    nc.sync.dma_start(
                        out=RB[:],
                        in_=pinB.rearrange("(r p) f -> p r f", r=1)
                        .to_broadcast((128, 4, 130)))

                # --- A-combine -> uniform delta -> own-col LN2 prep ---
                vsb = ac.tile([128, 4], f32, tag="vsb", bufs=2, name="vsb")
                nc.vector.tensor_reduce(out=vsb[:],
                                        in_=RA[:].rearrange("p r f -> p f r"),
                                        axis=mybir.AxisListType.X,
                                        op=OP.add)
                nc.vector.tensor_add(out=vsb[:], in0=vsb[:], in1=vfu[:])
                vsb8 = ac.tile([128, 4], f8, tag="vsb8", bufs=2,
                               name="vsb8")
                nc.vector.tensor_copy(out=vsb8[:], in_=vsb[:])
                dup = pgen.tile([128, 4], f32, tag="g", name="dup")
                for c in range(4):
                    for kc in range(4):
                        nc.tensor.matmul(dup[:, c:c + 1],
                                         wo8[:, kc // 2, kc % 2,
                                             128 * c:128 * (c + 1)],
                                         vsb8[:, kc:kc + 1],
                                         start=(kc == 0), stop=(kc == 3))
                dus = ac.tile([128, 4], f32, tag="dus", bufs=2, name="dus")
                nc.scalar.activation(out=dus[:], in_=dup[:], func=AF.Copy,
                                     scale=1.0 / (so * NALL))
                t28 = [ac.tile([128, 2, TOK], f8, tag=f"t28_{kp}", bufs=1,
                               name=f"t28_{kp}") for kp in range(2)]
                for c in range(4):
                    nc.vector.tensor_scalar_add(out=tok[c][:, 0:OWN],
                                                in0=tok[c][:, 0:OWN],
                                                scalar1=dus[:, c:c + 1])
                    nc.vector.tensor_copy(out=t28[c // 2][:, c % 2, 0:OWN],
                                          in_=tok[c][:, 0:OWN])
                S2 = pgen.tile([1, TOK], f32, tag="g", name="S2")
                for i in range(4):
                    nc.tensor.matmul(S2[:, 0:OWN], ones8p[:],
                                     t28[i // 2][:, i % 2, 0:OWN],
                                     start=(i == 0), stop=(i == 3))
                mur2b = ac.tile([1, TOK], bf16, tag="rowb", bufs=4,
                                name="mur2b")
                nc.scalar.activation(out=mur2b[:, 0:OWN], in_=S2[:, 0:OWN],
                                     func=AF.Copy, scale=1.0 / 64)
                bMu_p = pgen.tile([128, TOK], f32, tag="g", name="bMu")
                nc.tensor.matmul(bMu_p[:, 0:OWN], ones1[:], mur2b[:, 0:OWN],
                                 start=True, stop=True)
                bMu = ac.tile([128, TOK], bf16, tag="bM", bufs=2, name="bMub")
                nc.vector.tensor_copy(out=bMu[:, 0:OWN], in_=bMu_p[:, 0:OWN])
                for c in range(4):
                    nc.vector.tensor_sub(out=tok[c][:, 0:OWN],
                                         in0=tok[c][:, 0:OWN],
                                         in1=bMu[:, 0:OWN])
                xsq2 = []
                for kp in range(2):
                    t = ac.tile([128, 2, TOK], bf16, tag="xsq", bufs=4,
                                name=f"xsq2{kp}")
                    nc.scalar.activation(out=t[:, :, 0:OWN],
                                         in_=t28[kp][:, :, 0:OWN],
                                         func=AF.Square)
                    xsq2.append(t)
                Q2 = pgen.tile([1, TOK], f32, tag="g", name="Q2")
                for i in range(4):
                    nc.tensor.matmul(Q2[:, 0:OWN], oi512[:],
                                     xsq2[i // 2][:, i % 2, 0:OWN],
                                     start=(i == 0), stop=(i == 3))

                # --- B-combine -> fusion delta -> fusion-col LN2 prep ---
                PT = ac.tile([128, 130], f32, tag="cmbB", bufs=3, name="PT")
                nc.vector.tensor_reduce(out=PT[:],
                                        in_=RB[:].rearrange("p r f -> p f r"),
                                        axis=mybir.AxisListType.X,
                                        op=OP.add)
                linv = ac.tile([128, 2], f32, tag="linv", bufs=2, name="linv")
                nc.vector.reciprocal(out=linv[:], in_=PT[:, 0:2])
                of = []
                for t in range(2):
                    s = ac.tile([128, 64], bf16, tag=f"of{t}", bufs=1,
                                name=f"of{t}")
                    nc.vector.tensor_scalar_mul(out=s[:],
                                                in0=PT[:, 2 + 64 * t:66 + 64 * t],
                                                scalar1=linv[:, t:t + 1])
                    of.append(s)
                ofI8 = [ac.tile([128, 2, FUS], f8, tag=f"ofI{kp}", bufs=1,
                                name=f"ofI{kp}") for kp in range(2)]
                for kc in range(4):
                    pt = pacc.tile([128, 32], bf16, tag="a", name=f"ofIp{kc}")
                    for half in range(2):
                        h = 2 * kc + half
                        t, i = h // 4, h % 4
                        nc.tensor.transpose(
                            pt[64 * half:64 * half + 64, :],
                            of[t][32 * i:32 * i + 32, :],
                            ident[32 * i:32 * i + 32, 32 * i:32 * i + 32],
                            tile_position=(32 * i, 64 * half))
                    nc.scalar.activation(out=ofI8[kc // 2][:, kc % 2, :],
                                         in_=pt[:, 0:FUS], func=AF.Copy,
                                         scale=1.0 / so)
                for c in range(4):
                    dfp = pacc.tile([128, FUS], f32, tag="a", name=f"dfp{c}")
                    for kp in range(2):
                        nc.tensor.matmul(dfp[:],
                                         wo8[:, kp, :, 128 * c:128 * (c + 1)],
                                         ofI8[kp][:],
                                         start=(kp == 0), stop=(kp == 1),
                                         perf_mode=DR)
                    nc.vector.tensor_add(out=tok[c][:, OWN:TOK],
                                         in0=tok[c][:, OWN:TOK], in1=dfp[:])
                    nc.vector.tensor_copy(out=t28[c // 2][:, c % 2, OWN:TOK],
                                          in_=tok[c][:, OWN:TOK])
                for i in range(4):
                    nc.tensor.matmul(S2[:, OWN:TOK], ones8p[:],
                                     t28[i // 2][:, i % 2, OWN:TOK],
                                     start=(i == 0), stop=(i == 3))
                for kp in range(2):
                    nc.scalar.activation(out=xsq2[kp][:, :, OWN:TOK],
                                         in_=t28[kp][:, :, OWN:TOK],
                                         func=AF.Square)
                for i in range(4):
                    nc.tensor.matmul(Q2[:, OWN:TOK], oi512[:],
                                     xsq2[i // 2][:, i % 2, OWN:TOK],
                                     start=(i == 0), stop=(i == 3))

                # --- own-col stats2 tail (overlaps exchange B) ---
                m2_2 = ac.tile([1, TOK], f32, tag="rowf", bufs=6, name="m2_2")
                nc.scalar.activation(out=m2_2[:, 0:OWN], in_=S2[:, 0:OWN],
                                     func=AF.Square, scale=1.0 / 64)
                var2 = ac.tile([1, TOK], f32, tag="rowf", bufs=6, name="var2")
                nc.vector.tensor_sub(out=var2[:, 0:OWN], in0=Q2[:, 0:OWN],
                                     in1=m2_2[:, 0:OWN])
                rstd2 = ac.tile([1, TOK], f32, tag="rowf", bufs=6,
                                name="rstd2")
                nc.scalar.activation(out=rstd2[:, 0:OWN], in_=var2[:, 0:OWN],
                                     func=AF.Ln, bias=epsc[0:1, :])
                nc.scalar.activation(out=rstd2[:, 0:OWN], in_=rstd2[:, 0:OWN],
                                     func=AF.Exp, scale=-0.5)
                rstd2b = ac.tile([1, TOK], bf16, tag="rowb", bufs=4,
                                 name="rstd2b")
                nc.vector.tensor_copy(out=rstd2b[:, 0:OWN],
                                      in_=rstd2[:, 0:OWN])
                bR2_p = pgen.tile([128, TOK], f32, tag="g", name="bR2")
                nc.tensor.matmul(bR2_p[:, 0:OWN], ones1[:], rstd2b[:, 0:OWN],
                                 start=True, stop=True)
                bR2 = ac.tile([128, TOK], bf16, tag="bR", bufs=2, name="bR2b")
                nc.vector.tensor_copy(out=bR2[:, 0:OWN], in_=bR2_p[:, 0:OWN])
                xc8 = [ac.tile([128, 2, TOK], f8, tag=f"xc8_{kp}", bufs=1,
                               name=f"xc8_{kp}") for kp in range(2)]
                for c in range(4):
                    nc.vector.tensor_mul(out=xc8[c // 2][:, c % 2, 0:OWN],
                                         in0=tok[c][:, 0:OWN],
                                         in1=bR2[:, 0:OWN])

                # --- fusion-col centering + tail (post exchange B) ---
                nc.scalar.activation(out=mur2b[:, OWN:TOK], in_=S2[:, OWN:TOK],
                                     func=AF.Copy, scale=1.0 / 64)
                nc.tensor.matmul(bMu_p[:, OWN:TOK], ones1[:],
                                 mur2b[:, OWN:TOK], start=True, stop=True)
                nc.vector.tensor_copy(out=bMu[:, OWN:TOK],
                                      in_=bMu_p[:, OWN:TOK])
                for c in range(4):
                    nc.vector.tensor_sub(out=tok[c][:, OWN:TOK],
                                         in0=tok[c][:, OWN:TOK],
                                         in1=bMu[:, OWN:TOK])
                nc.scalar.activation(out=m2_2[:, OWN:TOK], in_=S2[:, OWN:TOK],
                                     func=AF.Square, scale=1.0 / 64)
                nc.vector.tensor_sub(out=var2[:, OWN:TOK], in0=Q2[:, OWN:TOK],
                                     in1=m2_2[:, OWN:TOK])
                nc.scalar.activation(out=rstd2[:, OWN:TOK],
                                     in_=var2[:, OWN:TOK],
                                     func=AF.Ln, bias=epsc[0:1, :])
                nc.scalar.activation(out=rstd2[:, OWN:TOK],
                                     in_=rstd2[:, OWN:TOK],
                                     func=AF.Exp, scale=-0.5)
                nc.vector.tensor_copy(out=rstd2b[:, OWN:TOK],
                                      in_=rstd2[:, OWN:TOK])
                nc.tensor.matmul(bR2_p[:, OWN:TOK], ones1[:],
                                 rstd2b[:, OWN:TOK], start=True, stop=True)
                nc.vector.tensor_copy(out=bR2[:, OWN:TOK],
                                      in_=bR2_p[:, OWN:TOK])
                # dummy gelu: preload the gelu table while FF1 matmuls run
                dt2 = ac.tile([1, 1], f32, tag="dtab", bufs=2, name="dt2")
                nc.scalar.activation(out=dt2[:], in_=rstd2b[0:1, 0:1],
                                     func=AF.Gelu)
                for c in range(4):
                    nc.vector.tensor_mul(out=xc8[c // 2][:, c % 2, OWN:TOK],
                                         in0=tok[c][:, OWN:TOK],
                                         in1=bR2[:, OWN:TOK])

                # --- FF1 / GEGLU / FF2 ---
                gt8 = [ac.tile([128, 2, TOK], f8, tag=f"gt{jp}", bufs=1,
                               name=f"gt{jp}") for jp in range(6)]
                nc.vector.memset(gt8[5][:, 1, :], 0.0)
                for j in range(11):
                    px = pgen.tile([128, TOK], f32, tag="g", name=f"fx{j}")
                    pg = pacc.tile([128, TOK], f32, tag="a", name=f"fg{j}")
                    for kp in range(2):
                        nc.tensor.matmul(px[:], w18[:, kp, :, 128 * j:128 * (j + 1)],
                                         xc8[kp][:], start=(kp == 0),
                                         stop=(kp == 1), perf_mode=DR)
                    for kp in range(2):
                        nc.tensor.matmul(
                            pg[:],
                            w18[:, kp, :, IFFP + 128 * j:IFFP + 128 * (j + 1)],
                            xc8[kp][:], start=(kp == 0), stop=(kp == 1),
                            perf_mode=DR)
                    gg = ac.tile([128, TOK], bf16, tag="gg", bufs=3,
                                 name=f"gg{j}")
                    nc.scalar.activation(out=gg[:], in_=pg[:], func=AF.Gelu,
                                         scale=1.0 / s1g)
                    nc.vector.tensor_mul(out=gt8[j // 2][:, j % 2, :],
                                         in0=gg[:], in1=px[:])
                # dummy exp: preload nlexp for the next layer during FF2
                dt3 = ac.tile([1, 1], f32, tag="dtab", bufs=2, name="dt3")
                nc.scalar.activation(out=dt3[:], in_=gt8[5][0:1, 0, 0:1],
                                     func=AF.Exp)
                psO_prev = []
                for c in range(4):
                    psO = pacc.tile([128, TOK], f32, tag="a", name=f"fo{c}")
                    for jp in range(6):
                        nc.tensor.matmul(psO[:], w28[:, jp, :, 128 * c:128 * (c + 1)],
                                         gt8[jp][:], start=(jp == 0),
                                         stop=(jp == 5), perf_mode=DR)
                    psO_prev.append(psO)

            # ---------- pool ----------
            pwkv8 = wp.tile([128, 2, 2, 2 * D], f8, tag="pwkv8", bufs=1,
                            name="pwkv8")
            nc.sync.dma_start(out=pwkv8[:], in_=pwkv8_t)
            pwo8 = wp.tile([128, 2, 2, D], f8, tag="pwo8", bufs=1,
                           name="pwo8")
            nc.sync.dma_start(out=pwo8[:], in_=pwo8_t)
            prow = wp.tile([1, 2 * D], bf16, tag="prow", bufs=1, name="prow")
            nc.sync.dma_start(out=prow[:], in_=prow_t)
            pq2s = load_cols(pq2_t, 4, "pq2")

            # final LN (rank-1 folded) over tok + last FF residual
            tf8 = [ac.tile([128, 2, TOK], f8, tag=f"tr8{kp}", bufs=1,
                           name=f"tf8_{kp}") for kp in range(2)]
            for c in range(4):
                t = ac.tile([128, TOK], bf16, tag="psot", bufs=2,
                            name=f"fpsot{c}")
                nc.scalar.activation(out=t[:], in_=psO_prev[c][:],
                                     func=AF.Copy, scale=cinvf)
                nc.vector.tensor_add(out=tok[c][:], in0=tok[c][:], in1=t[:])
                nc.vector.tensor_copy(out=tf8[c // 2][:, c % 2, :],
                                      in_=tok[c][:])
            Sf = pgen.tile([1, TOK], f32, tag="g", name="Sf")
            for i in range(4):
                nc.tensor.matmul(Sf[:], ones8p[:], tf8[i // 2][:, i % 2, :],
                                 start=(i == 0), stop=(i == 3))
            Sfb = ac.tile([1, TOK], bf16, tag="rowb", bufs=4, name="Sfb")
            nc.scalar.activation(out=Sfb[:], in_=Sf[:], func=AF.Copy)
            xsqf = []
            for kp in range(2):
                t = ac.tile([128, 2, TOK], bf16, tag="xsq", bufs=4,
                            name=f"xsqf{kp}")
                if kp == 0:
                    nc.vector.tensor_mul(out=t[:], in0=tf8[kp][:],
                                         in1=tf8[kp][:])
                else:
                    nc.scalar.activation(out=t[:], in_=tf8[kp][:],
                                         func=AF.Square)
                xsqf.append(t)
            Qf_ = pgen.tile([1, TOK], f32, tag="g", name="Qf_")
            for i in range(4):
                nc.tensor.matmul(Qf_[:], oi512[:], xsqf[i // 2][:, i % 2, :],
                                 start=(i == 0), stop=(i == 3))
            m2f = ac.tile([1, TOK], f32, tag="rowf", bufs=6, name="m2f")
            nc.scalar.activation(out=m2f[:], in_=Sf[:], func=AF.Square,
                                 scale=1.0 / 64)
            varf = ac.tile([1, TOK], f32, tag="rowf", bufs=6, name="varf")
            nc.vector.tensor_sub(out=varf[:], in0=Qf_[:], in1=m2f[:])
            rstdf = ac.tile([1, TOK], f32, tag="rowf", bufs=6, name="rstdf")
            nc.scalar.activation(out=rstdf[:], in_=varf[:], func=AF.Ln,
                                 bias=epsc[0:1, :])
            nc.scalar.activation(out=rstdf[:], in_=rstdf[:], func=AF.Exp,
                                 scale=-0.5)
            rstdfb = ac.tile([1, TOK], bf16, tag="rowb", bufs=4,
                             name="rstdfb")
            nc.vector.tensor_copy(out=rstdfb[:], in_=rstdf[:])
            bRf_p = pgen.tile([128, TOK], f32, tag="g", name="bRf")
            nc.tensor.matmul(bRf_p[:], ones1[:], rstdfb[:], start=True,
                             stop=True)
            bRf = ac.tile([128, TOK], bf16, tag="bR", bufs=2, name="bRfb")
            nc.vector.tensor_copy(out=bRf[:], in_=bRf_p[:])
            rTf_p = pacc.tile([128, 4], f32, tag="a", name="rTfp")
            for c, (a_, b_) in enumerate(tok_chunks):
                nc.tensor.matmul(rTf_p[0:b_ - a_, c:c + 1],
                                 rstdfb[:, a_:b_],
                                 ones1[0:1, 0:1], start=True, stop=True)
            rTf = ac.tile([128, 4], f32, tag="rT", bufs=2, name="rTf")
            nc.scalar.activation(out=rTf[:], in_=rTf_p[:], func=AF.Copy,
                                 scale=1.0 / spl)

            # V_pool + vsums -> pool exchange
            Vp = []
            for i, (a, b) in enumerate(tok_chunks):
                m = b - a
                ps = pacc.tile([128, D], f32, tag="a", name=f"pv{i}")
                for kp in range(2):
                    nc.tensor.matmul(ps[0:m, :], tf8[kp][:, :, a:b],
                                     pwkv8[:, kp, :, D:2 * D],
                                     start=(kp == 0), stop=False,
                                     perf_mode=DR)
                nc.tensor.matmul(ps[0:m, :], Sfb[:, a:b], prow[:, D:2 * D],
                                 start=False, stop=True)
                s = ac.tile([128, D], bf16, tag=f"V{i}", bufs=1,
                            name=f"pVb{i}")
                nc.scalar.activation(out=s[0:m, :], in_=ps[0:m, :],
                                     func=AF.Copy,
                                     scale=rTf[0:m, i:i + 1])
                Vp.append(s)
            PpA = ac.tile([128, 4], f32, tag="PA", bufs=2, name="PpA")
            for c in range(4):
                ps = pgen.tile([128, 1], f32, tag="g", name=f"pvs{c}")
                for j in range(3):
                    nc.tensor.matmul(ps[:], Vp[j][:, 128 * c:128 * (c + 1)],
                                     ones128[:], start=(j == 0), stop=(j == 2))
                nc.vector.tensor_copy(out=PpA[:, c:c + 1], in_=ps[:])
            pvfu = ac.tile([128, 4], f32, tag="vfu", bufs=2, name="pvfu")
            for c in range(4):
                ps = pgen.tile([128, 1], f32, tag="g", name=f"pvf{c}")
                nc.tensor.matmul(ps[:], Vp[3][0:FUS, 128 * c:128 * (c + 1)],
                                 ones128[0:FUS, :], start=True, stop=True)
                nc.vector.tensor_copy(out=pvfu[:, c:c + 1], in_=ps[:])
            pinP = dramp.tile([128, 4], f32, tag="pinA", bufs=2, name="pinP")
            nc.sync.dma_start(out=pinP[:], in_=PpA[:])
            RP = ac.tile([128, 4, 4], f32, tag="RA", bufs=2, name="RP")
            if use_cc:
                poutP = dramp.tile([4 * 128, 4], f32, tag="poutA", bufs=2,
                                   name="poutP")
                nc.gpsimd.collective_compute(
                    "AllGather", OP.bypass, replica_groups=rg,
                    ins=[pinP.opt()], outs=[poutP.opt()])
                nc.sync.dma_start(
                    out=RP[:], in_=poutP.rearrange("(r p) f -> p r f", r=4))
            else:
                nc.sync.dma_start(
                    out=RP[:],
                    in_=pinP.rearrange("(r p) f -> p r f", r=1)
                    .to_broadcast((128, 4, 4)))

            # uniform pooled vector u
            pvs = ac.tile([128, 4], f32, tag="vsb", bufs=2, name="pvs")
            nc.vector.tensor_reduce(out=pvs[:],
                                    in_=RP[:].rearrange("p r f -> p f r"),
                                    axis=mybir.AxisListType.X,
                                    op=OP.add)
            nc.vector.tensor_add(out=pvs[:], in0=pvs[:], in1=pvfu[:])
            pvsb8 = ac.tile([128, 4], f8, tag="vsb8", bufs=2,
                            name="pvsb8")
            nc.vector.tensor_copy(out=pvsb8[:], in_=pvs[:])
            pup = pgen.tile([128, 4], f32, tag="g", name="pup")
            for c in range(4):
                for kc in range(4):
                    nc.tensor.matmul(pup[:, c:c + 1],
                                     pwo8[:, kc // 2, kc % 2,
                                          128 * c:128 * (c + 1)],
                                     pvsb8[:, kc:kc + 1],
                                     start=(kc == 0), stop=(kc == 3))
            pus = ac.tile([128, 4], f32, tag="dus", bufs=2, name="pus")
            nc.scalar.activation(out=pus[:], in_=pup[:], func=AF.Copy,
                                 scale=1.0 / (spo * NALL))
            nc.sync.dma_start(out=out_u.rearrange("(c p) one -> p (c one)", c=4),
                              in_=pus[:])

            # fusion-key attention for return token 2
            kf = []
            for mc in range(4):
                ps = pgen.tile([128, FUS], f32, tag="g", name=f"pkf{mc}")
                for kp in range(2):
                    nc.tensor.matmul(ps[:],
                                     pwkv8[:, kp, :, 128 * mc:128 * (mc + 1)],
                                     tf8[kp][:, :, OWN:TOK],
                                     start=(kp == 0), stop=False,
                                     perf_mode=DR)
                nc.tensor.matmul(ps[:], prow[:, 128 * mc:128 * (mc + 1)],
                                 Sfb[:, OWN:TOK], start=False, stop=True)
                s = ac.tile([128, FUS], bf16, tag=f"kf{mc}", bufs=1,
                            name=f"kfb{mc}")
                nc.vector.tensor_mul(out=s[:], in0=ps[:],
                                     in1=bRf[:, OWN:TOK])
                kf.append(s)
            q2 = []
            for mc in range(4):
                s = ac.tile([128, 32], bf16, tag=f"qf{mc}", bufs=1,
                            name=f"q2b{mc}")
                nc.vector.memset(s[:, 1:32], 0.0)
                nc.vector.tensor_copy(out=s[:, 0:1], in_=pq2s[mc][:])
                q2.append(s)
            e2, l2 = [], []
            for t in range(2):
                sp = pgen.tile([128, FUS], f32, tag="g", name=f"ps2{t}")
                for i in range(4):
                    h = 4 * t + i
                    ch, base = h // 2, (h % 2) * 64
                    nc.tensor.matmul(sp[32 * i:32 * i + 32, :],
                                     q2[ch][base:base + 64, 0:32],
                                     kf[ch][base:base + 64, :],
                                     start=True, stop=True,
                                     tile_position=(base, 32 * i))
                e = ac.tile([128, FUS], bf16, tag=f"e2{t}", bufs=1,
                            name=f"e2{t}")
                la = ac.tile([128, 1], f32, tag=f"la{t}", bufs=2,
                             name=f"pla{t}")
                nc.scalar.activation(out=e[:], in_=sp[:], func=AF.Exp,
                                     scale=1.0 / spl, accum_out=la[:])
                e2.append(e)
                l2.append(la)
            e2T = []
            for t in range(2):
                pt = pgen.tile([FUS, 128], bf16, tag="g", name=f"pet{t}")
                nc.tensor.transpose(pt[:], e2[t][:], ident[:])
                s = ac.tile([FUS, 128], bf16, tag=f"e2T{t}", bufs=1,
                            name=f"e2Tb{t}")
                nc.vector.tensor_copy(out=s[:], in_=pt[:])
                e2T.append(s)
            of2 = []
            for t in range(2):
                acc = pacc.tile([128, 64], f32, tag="a", name=f"pacc2{t}")
                for i in range(4):
                    h = 4 * t + i
                    nc.tensor.matmul(acc[32 * i:32 * i + 32, :],
                                     e2T[t][:, 32 * i:32 * i + 32],
                                     Vp[3][0:FUS, DH * h:DH * (h + 1)],
                                     start=True, stop=True,
                                     tile_position=(0, 32 * i))
                li = ac.tile([128, 1], f32, tag="linv", bufs=2,
                             name=f"pli{t}")
                nc.vector.reciprocal(out=li[:], in_=l2[t][:])
                s = ac.tile([128, 64], bf16, tag=f"of{t}", bufs=1,
                            name=f"pof{t}")
                nc.vector.tensor_scalar_mul(out=s[:], in0=acc[:],
                                            scalar1=li[:])
                of2.append(s)
            ofI2 = [ac.tile([128, 1], f8, tag=f"ofI2{kc}", bufs=1,
                            name=f"ofI2{kc}") for kc in range(4)]
            for kc in range(4):
                pt = pacc.tile([128, 32], bf16, tag="a", name=f"ofI2p{kc}")
                for half in range(2):
                    h = 2 * kc + half
                    t, i = h // 4, h % 4
                    nc.tensor.transpose(
                        pt[64 * half:64 * half + 64, :],
                        of2[t][32 * i:32 * i + 32, :],
                        ident[32 * i:32 * i + 32, 32 * i:32 * i + 32],
                        tile_position=(32 * i, 64 * half))
                nc.vector.tensor_copy(out=ofI2[kc][:], in_=pt[:, 0:1])
            P2 = pacc.tile([1, D], f32, tag="a", name="P2")
            for kc in range(4):
                nc.tensor.matmul(P2[:], ofI2[kc][:],
                                 pwo8[:, kc // 2, kc % 2, :],
                                 start=(kc == 0), stop=(kc == 3))
            p2s = ac.tile([1, D], f32, tag="p2s", bufs=1, name="p2s")
            nc.scalar.activation(out=p2s[:], in_=P2[:], func=AF.Copy,
                                 scale=1.0 / spo)
            nc.sync.dma_start(out=out_f, in_=p2s[:])

    nc.compile()
    _built[key] = nc
    return nc


def _prep_inputs(inputs):
    """Host-side prep: fold gains, pick fp8 scales, pack weights per layer."""
    I = {k: np.asarray(v, np.float64) for k, v in inputs.items()}
    f32 = np.float32

    def col(x):
        return np.ascontiguousarray(np.asarray(x, f32).reshape(-1, 1))

    scale_dh = DH ** -0.5
    g = I["layers_attn_g"][:, :, None]
    wqf = I["layers_wq"] * scale_dh * g            # [L, 512, 512]
    wkvf = I["layers_wkv"] * g                     # [L, 512, 1024]
    wof = I["layers_wo"]                           # [L, 512, 512]
    w1f = I["layers_ff_w1"] * I["layers_ff_g"][:, :, None]  # [L, 512, 2730]
    w2f = I["layers_ff_w2"]                        # [L, 1365, 512]

    sq = _pow2_scale(wqf)
    skv = _pow2_scale(wkvf)
    so = _pow2_scale(wof)
    s1x = 16.0
    s1g = _pow2_scale(w1f[:, :, IFF:])
    s2 = _pow2_scale(w2f)
    pkvf = I["pool_wkv"] * I["final_g"][:, None]
    spl = _pow2_scale(pkvf)
    spo = _pow2_scale(I["pool_wo"])
    sef = {}
    for mod in ("rna", "atac"):
        ewf = I[f"{mod}_w"] * I[f"{mod}_ln1_g"][:, None]
        sef[mod] = (ewf, _pow2_scale(ewf))
    se = min(sef["rna"][1], sef["atac"][1])
    scales = (sq, skv, so, s1x, s1g, s2, se, spl, spo)

    # packed per-layer fp8 weights
    wpk = np.zeros((DEPTH, 128, FCOLS), F8)
    rows = np.zeros((DEPTH, 3, D), np.float64)
    for l in range(DEPTH):
        wpk[l, :, SEG_WQ:SEG_WKV] = _pack_pairs(wqf[l], sq).reshape(128, -1)
        wpk[l, :, SEG_WKV:SEG_WO] = _pack_pairs(wkvf[l], skv).reshape(128, -1)
        wpk[l, :, SEG_WO:SEG_W1] = _pack_pairs(wof[l], so).reshape(128, -1)
        w1p = np.zeros((D, 2 * IFFP), np.float64)
        w1p[:, 0:IFF] = w1f[l][:, 0:IFF] * s1x
        w1p[:, IFFP:IFFP + IFF] = w1f[l][:, IFF:] * s1g
        wpk[l, :, SEG_W1:SEG_W2] = _pack_pairs(w1p, 1.0).reshape(128, -1)
        w2p = np.zeros((IFF2, D), np.float64)
        w2p[0:IFF, :] = w2f[l] * s2
        wpk[l, :, SEG_W2:FCOLS] = _pack_pairs(w2p, 1.0).reshape(128, -1)
        # rank-1 rows: -scale*colsum/64
        rows[l, 0] = -skv * wkvf[l][:, 0:D].sum(0) / 64     # wksum
        rows[l, 1] = -skv * wkvf[l][:, D:2 * D].sum(0) / 64  # wvsum
        rows[l, 2] = -sq * wqf[l].sum(0) / 64                # wqsum
    rows_b = np.ascontiguousarray(rows.reshape(1, -1)).astype(BF)

    prow = np.zeros((1, 2 * D), np.float64)
    prow[0, 0:D] = -spl * pkvf[:, 0:D].sum(0) / 64
    prow[0, D:2 * D] = -spl * pkvf[:, D:2 * D].sum(0) / 64

    pwkv8 = _pack_pairs(pkvf, spl)
    pwo8 = _pack_pairs(I["pool_wo"], spo)

    # pool query for return token 2 (host, tiny)
    ret = I["return_tokens"].astype(f32)
    gp = I["pool_g"].astype(f32)
    mu = ret.mean(-1, keepdims=True)
    var = ((ret - mu) ** 2).mean(-1, keepdims=True)
    retn = (ret - mu) / np.sqrt(var + 1e-5) * gp
    q2 = (retn[2] @ I["pool_wq"].astype(f32)) * scale_dh

    fus8 = I["fusion_tokens"].T.reshape(4, 128, FUS).transpose(1, 0, 2)

    shared = {
        "wpk": wpk,
        "rows": rows_b,
        "prow": prow.astype(BF),
        "pwkv8": pwkv8,
        "pwo8": pwo8,
        "pool_q2": col(q2),
        "fus_t": np.ascontiguousarray(fus8).astype(np.float32),
    }

    in_maps = []
    for c in range(N_CORES):
        b, q = c // 4, c % 4
        mod = "rna" if q < 2 else "atac"
        x = I[mod][b, (q % 2) * OWN:(q % 2 + 1) * OWN, :]   # [384, 1024]
        m = dict(shared)
        # x8: [128, 4kp, 2sub, 384]
        xT = np.ascontiguousarray(x.T)                      # [1024, 384]
        m["x8"] = _pack_pairs(xT, 1.0)
        ewf = sef[mod][0]
        m["ew8"] = _pack_pairs(ewf, se)
        m["emb_b"] = col(I[f"{mod}_b"]
                         + I[f"{mod}_ln1_b"] @ I[f"{mod}_w"])
        m["erow"] = (-se * ewf.sum(0) / 128).reshape(1, -1).astype(BF)
        m["eln2_g"] = col(I[f"{mod}_ln2_g"])
        m["eln2_b"] = col(I[f"{mod}_ln2_b"])
        in_maps.append(m)
    return in_maps, ret.astype(f32), scales


def kernel(**inputs):
    from concourse import bass_utils
    in_maps, ret, scales = _prep_inputs(inputs)
    nc = build(num_devices=N_CORES, use_cc=True, scales=scales)
    res = bass_utils.run_bass_kernel_spmd(nc, in_maps,
                                          core_ids=list(range(N_CORES)))
    out = np.zeros((B, 3, D), np.float32)
    for b in range(2):
        r = res.results[4 * b]
        u = r["out_u"][:, 0]
        f = r["out_f"][0]
        out[b, 0] = u + ret[0]
        out[b, 1] = u + ret[1]
        out[b, 2] = f + ret[2]
    return out
